# revision 28
# baseline (speedup 1.0000x reference)
"""D4 dispersion energy kernel for 8 Trainium2 NeuronCores.

Strategy (v2):
- Host (numpy, integer/permutation work only): sort the edge list by (dst
  atom, j-range bucket), pad each (atom,bucket) edge run to a multiple of 8
  ("groups"), lay slots out in a fixed chunk/call/partition grid, and
  pre-permute all per-edge input data into that slot order.
- Device (all float math):
  * pass A computes per-edge coordination-number contributions and
    tree-reduces them into group sums, then dma_scatter_add's the group sums
    directly into a dense per-atom ncoord table;
  * stage 2 computes per-atom Gaussian weights / zeta / effective alpha
    table A~ from this core's atom slice (bf16, packed 23 floats/atom);
  * one AllGather shares the packed bf16 A~ rows into a 256B-stride table;
  * pass B gathers A~ rows for edge sources via dma_gather (bf16, 46B
    payload), applies Becke-Johnson damping, tree-reduces into group rows and
    dma_scatter_add's them into a dense per-atom B table;
  * E_i = -0.5*HARTREE * <A~_i, B_i>.
"""
import math
import numpy as np

import concourse.bass as bass
import concourse.bacc as bacc
import concourse.tile as tile
from concourse import mybir
from concourse.library_config import mlp as mlp_library

F32 = mybir.dt.float32
BF16 = mybir.dt.bfloat16
I16 = mybir.dt.int16

Z = 87
NREF = 7
NC = 5
NW = 23
BOHR = 0.5291772105638411
HARTREE = 27.211386024367243
K4, K5, K6, KK = 4.10451, 19.08857, 254.5553148552, 7.5
E3 = float(np.exp(3.0))
CPFAC = 3.0 / (2.0 * np.pi)

NCORES = 8
P = 128
ACOLS = 80              # atom columns per partition -> NA = 128*80
NA = P * ACOLS          # atoms per core (10240)
NPAD = NCORES * NA      # padded atom count (81920)
ACH = 16                # atom columns per stage-2 chunk (2048 atoms)
NACH = ACOLS // ACH     # atom chunks per core (5)
CALL = 32768            # idxs per dma_gather call (one per chunk)
TCH = 256               # slots per partition per compute chunk
GS = 4                  # slots per group
CHSLOTS = P * TCH       # slots per compute chunk (32768)
GCH = CHSLOTS // GS     # groups per chunk (4096)
TPG = TCH // GS         # group cells per partition per chunk (32)

# j-range buckets (dma_gather idx is int16)
NBUCK = 3
BBASE = [0, 27307, 54614]
BSIZE = [27307, 27307, NPAD - 54614]

SROWW = 320             # per-species row width (f32); 1280 B, 256-aligned
BTW = 64                # per-atom table row width (f32); 256 B stride


def _wrap16(idx_lin):
    """int linear idx list -> [128, ceil(n/16)] int16 wrapped tile."""
    n = len(idx_lin)
    m = (n + 15) // 16
    pad = np.zeros(m * 16, np.int16)
    pad[:n] = idx_lin.astype(np.int16)
    core = pad.reshape(m, 16).T  # [16, m]
    return np.tile(core, (8, 1)).reshape(128, m)


def preprocess(species, edge_index, lengths, partial_charges):
    """Build per-core host-side data. Returns (per_core list of dicts, meta)."""
    n_at = species.shape[0]
    species = np.asarray(species).astype(np.int32)
    idx_i = np.asarray(edge_index[0]).astype(np.int64)
    idx_j = np.asarray(edge_index[1]).astype(np.int64)
    lengths = np.asarray(lengths).astype(np.float32)
    charges = np.asarray(partial_charges).astype(np.float32)

    spec_pad = np.zeros(NPAD, np.int32)
    spec_pad[:n_at] = species
    chg_pad = np.zeros(NPAD, np.float32)
    chg_pad[:n_at] = charges

    # bucket of each edge by j range
    jb = np.searchsorted(np.array(BBASE[1:]), idx_j, side="right")  # 0..2
    key = idx_i * NBUCK + jb
    order = np.argsort(key, kind="stable")
    si = idx_i[order]
    sj = idx_j[order]
    sl = lengths[order]
    sjb = jb[order]

    # count edges per (atom, bucket)
    cnt = np.bincount(idx_i * NBUCK + jb, minlength=NPAD * NBUCK).reshape(NPAD, NBUCK)
    grp = (cnt + GS - 1) // GS  # groups per (atom,bucket)
    # CSR offsets into sorted edge array for (atom,bucket)
    flat_cnt = cnt.reshape(-1)
    edge_off = np.zeros(NPAD * NBUCK + 1, np.int64)
    np.cumsum(flat_cnt, out=edge_off[1:])

    # group quota per bucket (max over cores, rounded to chunk multiple)
    grp_cb = grp.reshape(NCORES, NA, NBUCK).sum(axis=1)  # [core, bucket]
    NGBS = []
    for b in range(NBUCK):
        m = int(grp_cb[:, b].max())
        NGBS.append(((m + GCH - 1) // GCH) * GCH)
    NG = sum(NGBS)                       # groups per core
    SLOTS = NG * GS                      # slots per core
    NCH = SLOTS // CHSLOTS               # compute chunks
    assert SLOTS % CHSLOTS == 0
    # chunk -> bucket map (buckets are whole chunks)
    ch_bucket = []
    for b in range(NBUCK):
        ch_bucket += [b] * (NGBS[b] * GS // CHSLOTS)
    gb_off = np.concatenate([[0], np.cumsum(NGBS)])  # group offset per bucket

    meta = dict(NGBS=NGBS, NG=NG, SLOTS=SLOTS, NCH=NCH, ch_bucket=ch_bucket)

    per_core = []
    for c in range(NCORES):
        a0 = c * NA
        g_c = grp[a0 : a0 + NA]                 # [NA, NBUCK]
        gofs = np.zeros((NA + 1, NBUCK), np.int64)
        np.cumsum(g_c, axis=0, out=gofs[1:])
        ng_b = gofs[NA]                          # real groups per bucket
        for b in range(NBUCK):
            assert ng_b[b] <= NGBS[b]

        # atom id of each core-local group (bucket-sectioned, then padded)
        atom_of_G = np.zeros(NG, np.int32)
        for b in range(NBUCK):
            rep = np.repeat(np.arange(NA, dtype=np.int32), g_c[:, b])
            atom_of_G[gb_off[b] : gb_off[b] + len(rep)] = rep
            # pad groups -> atom 0 (their value will be exactly 0.0)

        # slot position for each real edge:
        atom_l = si - a0
        core_mask = (atom_l >= 0) & (atom_l < NA)
        e_sel = np.nonzero(core_mask)[0]
        al = atom_l[e_sel]
        eb = sjb[e_sel]
        flat_id = (si[e_sel] * NBUCK + eb)
        rank = (e_sel - edge_off[flat_id])
        grank = rank // GS
        lane = rank % GS
        G = gb_off[eb] + gofs[al, eb] + grank    # core-local group id
        # group cell mapping: G -> (chunk, partition, tg); slots of a group
        # are GS consecutive columns of one partition.
        c_ch = G // GCH
        pp = (G % GCH) // TPG
        tg = G % TPG
        # lane l of group (pp, tg) sits at column tg*GS+l of partition pp
        pos = c_ch * CHSLOTS + (tg * GS + lane) * P + pp

        # per-slot streams (defaults for pad slots)
        r_s = np.full(SLOTS, 1.0e4, np.float32)
        rcj_s = np.ones(SLOTS, np.float32)
        enj_s = np.ones(SLOTS, np.float32)
        rci_s = np.ones(SLOTS, np.float32)
        eni_s = np.ones(SLOTS, np.float32)
        si_s = np.ones(SLOTS, np.float32)
        sj_s = np.ones(SLOTS, np.float32)
        jl_s = np.zeros(SLOTS, np.int32)

        r_s[pos] = sl[e_sel]
        jl_s[pos] = sj[e_sel] - np.array(BBASE, np.int64)[eb]

        # scatter idx per chunk: token t=(p, cc) -> group c_ch*GCH + p*TPG + cc
        sidx = np.zeros((NCH, 128, GCH // 16), np.int16)
        for ch in range(NCH):
            gg = np.arange(GCH, dtype=np.int64)
            # token order: t -> (p = t%128, cc = t//128)
            ptok = gg % 128
            cctok = gg // 128
            Gtok = ch * GCH + ptok * TPG + cctok
            sidx[ch] = _wrap16(atom_of_G[Gtok])

        per_core.append(dict(
            pos=pos, e_sel=e_sel, sj=sj[e_sel], sp_i=spec_pad[si[e_sel]],
            sp_j=spec_pad[sj[e_sel]],
            r_s=r_s, rcj_s=rcj_s, enj_s=enj_s, rci_s=rci_s, eni_s=eni_s,
            si_s=si_s, sj_s=sj_s, jl_s=jl_s, sidx=sidx,
            spec_slice=spec_pad[a0 : a0 + NA], chg_slice=chg_pad[a0 : a0 + NA],
        ))
    return per_core, meta


def build_core_inputs(pc, meta, rcov, en, sqrt_r4r2):
    """Fill species-derived streams + wrapped idx arrays for one core."""
    SLOTS, NCH = meta["SLOTS"], meta["NCH"]
    pos = pc["pos"]
    pc["rcj_s"][pos] = rcov[pc["sp_j"]]
    pc["enj_s"][pos] = en[pc["sp_j"]]
    pc["rci_s"][pos] = rcov[pc["sp_i"]]
    pc["eni_s"][pos] = en[pc["sp_i"]]
    pc["si_s"][pos] = sqrt_r4r2[pc["sp_i"]]
    pc["sj_s"][pos] = sqrt_r4r2[pc["sp_j"]]

    # jidx16: one gather call per chunk, wrapped
    jl = pc["jl_s"]
    jw = np.zeros((NCH, 128, CALL // 16), np.int16)
    for k in range(NCH):
        jw[k] = _wrap16(jl[k * CALL : (k + 1) * CALL])

    # species wrap per atom chunk (2048 atoms): idx position u*128+p ->
    # atom (p, 16k+u)
    spw = np.zeros((NACH, 128, (ACH * P) // 16), np.int16)
    spec = pc["spec_slice"].reshape(P, ACOLS)
    for k in range(NACH):
        lin = spec[:, k * ACH : (k + 1) * ACH].T.reshape(-1)  # [u, p] -> u*128+p
        spw[k] = _wrap16(lin)

    return dict(
        r_s=pc["r_s"], rcj_s=pc["rcj_s"], enj_s=pc["enj_s"],
        rci_s=pc["rci_s"], eni_s=pc["eni_s"], si_s=pc["si_s"],
        sj_s=pc["sj_s"],
        jw=jw.reshape(-1), spw=spw.reshape(-1), sidx=pc["sidx"].reshape(-1),
        chg=pc["chg_slice"].reshape(P, ACOLS).astype(np.float32),
    )


def _bc(ap, n):
    """Broadcast AP: append a step-0 inner dim of size n."""
    return bass.AP(tensor=ap.tensor, offset=ap.offset, ap=[*ap.ap, [0, n]])


def _dma_gather_raw(nc, out_ap, in_ap, idxs_ap, num_idxs, elem_size, elem_step):
    """dma_gather without the elem_size%256 restriction (payload < row pitch).
    Mirrors bass.BassGpSimd.dma_gather (non-transpose, DRAM source)."""
    eng = nc.gpsimd
    assert idxs_ap.dtype == mybir.dt.int16
    assert in_ap.dtype == out_ap.dtype
    stride_bytes = elem_step * mybir.dt.size(in_ap.dtype)
    assert stride_bytes % 256 == 0
    stride_bytes_256 = stride_bytes // 256
    assert in_ap.ap[0][0] == elem_step
    assert in_ap.ap[-1][1] == elem_size
    assert out_ap.ap[-1][1] == elem_size
    _in_ap = eng.lower_ap_dma(in_ap, for_custom_bir_dma=True)
    _idxs_ap = eng.lower_ap(idxs_ap)
    _out_ap = eng.lower_ap(out_ap)
    return eng.add_instruction(
        mybir.InstDMAGatherAnt(
            name=nc.get_next_instruction_name(),
            ins=[*_in_ap, _idxs_ap, eng.lower_val_access(eng.to_reg(num_idxs))],
            outs=[_out_ap],
            transpose=False,
            num_idxs=num_idxs,
            elem_size=elem_size,
            stride_bytes_256=stride_bytes_256,
            gen_mode=0,
            single_packet=True,
            queue_num=0,
            sbuf_tokens_per_rank=0,
            sbuf_free_dim_per_rank=0,
            sbuf_free_dim_pad_per_rank=0,
            sbuf_byte_offset=0,
        )
    )


def build_program(meta):
    SLOTS, NCH = meta["SLOTS"], meta["NCH"]
    ch_bucket = meta["ch_bucket"]
    A = mybir.AluOpType
    AF = mybir.ActivationFunctionType

    nc = bacc.Bacc(None, num_devices=NCORES, dynamic_dma_scratch_size=40960)

    def din(name, shape, dt=F32):
        return nc.dram_tensor(name, shape, dt, kind="ExternalInput")

    # per-slot streams
    r_d = din("r_s", [SLOTS])
    rcj_d = din("rcj_s", [SLOTS])
    enj_d = din("enj_s", [SLOTS])
    rci_d = din("rci_s", [SLOTS])
    eni_d = din("eni_s", [SLOTS])
    si_d = din("si_s", [SLOTS])
    sj_d = din("sj_s", [SLOTS])
    jw_d = din("jw", [NCH, 128, CALL // 16], I16)
    spw_d = din("spw", [NACH, 128, (ACH * P) // 16], I16)
    sidx_d = din("sidx", [NCH, 128, GCH // 16], I16)
    chg_d = din("chg", [P, ACOLS])
    # tables
    zeffr_d = din("zeff_r", [Z, NREF]); sscr_d = din("sscale_r", [Z, NREF])
    gamr_d = din("gam_r", [Z, NREF]); refh_d = din("refh", [Z, NREF])
    asc_d = din("ascale", [Z, NREF]); hcnt_d = din("hcount", [Z, NREF])
    refq_d = din("refq", [Z, NREF])
    secr_d = din("secaiw_r", [Z, NREF * NW]); aiw_d = din("alphaiw", [Z, NREF * NW])
    gam_d = din("gam", [Z]); zeff_d = din("zeff", [Z]); sr4_d = din("sqrt_r4r2", [Z])
    cnw_d = din("ncount_weight", [Z, NREF * NC]); cnd_d = din("cn", [Z, NREF * NC])
    msk_d = din("ncount_mask", [Z, NREF * NC])
    cpw_d = din("cpw", [NW])
    s6_d = din("s6_raw", [1]); s8_d = din("s8_raw", [1])
    a1_d = din("a1_raw", [1]); a2_d = din("a2_raw", [1]); sq_d = din("scale_q_raw", [1])

    srow_d = nc.dram_tensor("srowd", [Z, SROWW], F32)
    nco_d = nc.dram_tensor("ncod", [NA, BTW], F32)
    btab_d = nc.dram_tensor("btabd", [NA, 2 * BTW], BF16)
    t2s_d = nc.dram_tensor("t2s", [NA, BTW], F32)
    t2sb_d = nc.dram_tensor("t2sb", [NA, NW], BF16)
    t2f_d = nc.dram_tensor("t2f", [NPAD, P], BF16, addr_space="Shared")
    e_d = nc.dram_tensor("e_out", [NA], F32, kind="ExternalOutput")

    def brc(dram, parts, width):
        """AP reading a [width] DRAM tensor broadcast across `parts` partitions."""
        return bass.AP(tensor=dram.tensor if hasattr(dram, "tensor") else dram,
                       offset=0, ap=[[0, parts], [1, width]])

    with tile.TileContext(nc) as tc:
        import contextlib
        with contextlib.ExitStack() as ctx:
            const = ctx.enter_context(tc.tile_pool(name="const", bufs=1))
            _srowcm = tc.tile_pool(name="srowp", bufs=1)
            srowp = _srowcm.__enter__()
            _wcm = tc.tile_pool(name="p0", bufs=2)
            work = _wcm.__enter__()

            nc.gpsimd.load_library(mlp_library)

            b3_87 = const.tile([Z, 1], F32)
            nc.vector.memset(b3_87[:], 3.0)
            b3_p = const.tile([P, 1], F32)
            nc.vector.memset(b3_p[:], 3.0)
            bk5_p = const.tile([P, 1], F32)
            nc.vector.memset(bk5_p[:], K5)
            bkk_p = const.tile([P, 1], F32)
            nc.vector.memset(bkk_p[:], KK)

            # zero rows of nco/btab tables (only the columns we touch)
            zcol = const.tile([P, ACOLS, 1], F32)
            nc.vector.memset(zcol[:], 0.0)
            nc.sync.dma_start(
                out=nco_d.rearrange("(p a) f -> p a f", p=P)[:, :, 0:1],
                in_=zcol[:])
            zrow23 = const.tile([P, ACOLS, NW], BF16)
            nc.vector.memset(zrow23[:], 0.0)
            nc.sync.dma_start(
                out=btab_d.rearrange("(p a) f -> p a f", p=P)[:, :, 0:NW],
                in_=zrow23[:])

            # ---------- P0: per-species row table ----------
            def ld87(dram, w):
                t = const.tile([Z, w], F32, tag=f"ld_{dram.name}")
                nc.sync.dma_start(out=t[:], in_=dram[:] if w > 1 else dram[:, None])
                return t

            zeffr = ld87(zeffr_d, NREF); sscr = ld87(sscr_d, NREF)
            gamr = ld87(gamr_d, NREF); refh = ld87(refh_d, NREF)
            asc = ld87(asc_d, NREF); hcnt = ld87(hcnt_d, NREF)
            refq = ld87(refq_d, NREF)
            secr = ld87(secr_d, NREF * NW); aiw = ld87(aiw_d, NREF * NW)
            gam1 = ld87(gam_d, 1); zeff1 = ld87(zeff_d, 1); sr41 = ld87(sr4_d, 1)
            cnw = ld87(cnw_d, NREF * NC); cnt_ = ld87(cnd_d, NREF * NC)
            msk = ld87(msk_d, NREF * NC)

            sq87 = const.tile([Z, 1], F32)
            nc.sync.dma_start(out=sq87[:], in_=brc(sq_d, Z, 1))
            nc.scalar.activation(out=sq87[:], in_=sq87[:], func=AF.Exp)
            nc.vector.tensor_scalar(out=sq87[:], in0=sq87[:], scalar1=1.0,
                                    scalar2=None, op0=A.add)
            nc.scalar.activation(out=sq87[:], in_=sq87[:], func=AF.Ln)

            qmod = work.tile([Z, NREF], F32, tag="p0a")
            nc.vector.tensor_scalar(out=qmod[:], in0=refh[:], scalar1=sq87[:, 0:1],
                                    scalar2=None, op0=A.mult)
            nc.vector.tensor_tensor(out=qmod[:], in0=qmod[:], in1=zeffr[:], op=A.add)
            qmsk = work.tile([Z, NREF], F32, tag="p0b")
            nc.vector.tensor_scalar(out=qmsk[:], in0=qmod[:], scalar1=1e-8,
                                    scalar2=None, op0=A.is_gt)
            qsafe = work.tile([Z, NREF], F32, tag="p0c")
            nc.vector.tensor_scalar(out=qsafe[:], in0=qmod[:], scalar1=1.0,
                                    scalar2=None, op0=A.subtract)
            nc.vector.tensor_tensor(out=qsafe[:], in0=qsafe[:], in1=qmsk[:],
                                    op=A.mult)
            nc.vector.tensor_scalar(out=qsafe[:], in0=qsafe[:], scalar1=1.0,
                                    scalar2=None, op0=A.add)
            rq = work.tile([Z, NREF], F32, tag="p0d")
            nc.vector.reciprocal(out=rq[:], in_=qsafe[:])
            t0 = work.tile([Z, NREF], F32, tag="p0e")
            nc.vector.tensor_tensor(out=t0[:], in0=zeffr[:], in1=rq[:], op=A.mult)
            nc.vector.tensor_tensor(out=t0[:], in0=t0[:], in1=gamr[:], op=A.mult)
            nc.vector.tensor_tensor(out=t0[:], in0=gamr[:], in1=t0[:], op=A.subtract)
            nc.scalar.activation(out=t0[:], in_=t0[:], func=AF.Exp, scale=2.0)
            nc.scalar.activation(out=t0[:], in_=t0[:], func=AF.Exp, scale=-3.0,
                                 bias=b3_87[:, 0:1])
            zfac = work.tile([Z, NREF], F32, tag="p0f")
            nc.vector.tensor_scalar(out=zfac[:], in0=t0[:], scalar1=E3,
                                    scalar2=None, op0=A.subtract)
            nc.vector.tensor_tensor(out=zfac[:], in0=zfac[:], in1=qmsk[:],
                                    op=A.mult)
            nc.vector.tensor_scalar(out=zfac[:], in0=zfac[:], scalar1=E3,
                                    scalar2=None, op0=A.add)
            al = work.tile([Z, NREF, NW], F32, tag="p0g")
            nc.vector.tensor_tensor(
                out=al[:], in0=secr[:].rearrange("z (a w) -> z a w", w=NW),
                in1=_bc(sscr[:], NW), op=A.mult)
            nc.vector.tensor_tensor(out=al[:], in0=al[:], in1=_bc(zfac[:], NW),
                                    op=A.mult)
            nc.vector.tensor_tensor(out=al[:], in0=al[:], in1=_bc(hcnt[:], NW),
                                    op=A.mult)
            nc.vector.tensor_tensor(
                out=al[:], in0=aiw[:].rearrange("z (a w) -> z a w", w=NW),
                in1=al[:], op=A.subtract)
            nc.vector.tensor_tensor(out=al[:], in0=al[:], in1=_bc(asc[:], NW),
                                    op=A.mult)
            nc.vector.tensor_scalar(out=al[:], in0=al[:], scalar1=0.0,
                                    scalar2=None, op0=A.max)
            cpw87 = const.tile([Z, NW], F32)
            nc.sync.dma_start(out=cpw87[:], in_=brc(cpw_d, Z, NW))
            nc.scalar.activation(out=cpw87[:], in_=cpw87[:], func=AF.Sqrt,
                                 scale=CPFAC)
            wb = bass.AP(tensor=cpw87[:].tensor, offset=cpw87[:].offset,
                         ap=[cpw87[:].ap[0], [0, NREF], [1, NW]])
            nc.vector.tensor_tensor(out=al[:], in0=al[:], in1=wb, op=A.mult)

            # assemble SROW
            srow = const.tile([Z, SROWW], F32)
            nc.vector.memset(srow[:], 0.0)
            nc.vector.tensor_copy(out=srow[:, 0:1], in_=gam1[:])
            nc.vector.tensor_copy(out=srow[:, 1:2], in_=zeff1[:])
            nc.vector.tensor_copy(out=srow[:, 2:9], in_=refq[:])
            nc.vector.tensor_copy(out=srow[:, 9:44], in_=cnw[:])
            nc.vector.tensor_copy(out=srow[:, 44:79], in_=cnt_[:])
            nc.vector.tensor_copy(out=srow[:, 79:114], in_=msk[:])
            nc.vector.tensor_copy(
                out=srow[:, 114:275],
                in_=al[:].rearrange("z a w -> z (a w)"))
            nc.vector.tensor_copy(out=srow[:, 275:276], in_=sr41[:])
            nc.sync.dma_start(out=srow_d[:], in_=srow[:])

            # params [128,1]
            params = const.tile([P, 4], F32)
            for ii, dd in enumerate([s6_d, s8_d, a1_d, a2_d]):
                nc.sync.dma_start(out=params[:, ii:ii+1], in_=brc(dd, P, 1))
            nc.scalar.activation(out=params[:], in_=params[:], func=AF.Exp)
            nc.vector.tensor_scalar(out=params[:], in0=params[:], scalar1=1.0,
                                    scalar2=None, op0=A.add)
            nc.scalar.activation(out=params[:], in_=params[:], func=AF.Ln)
            s6p, s8p = params[:, 0:1], params[:, 1:2]
            a1p, a2p = params[:, 2:3], params[:, 3:4]

            spq = const.tile([P, 1], F32)
            nc.sync.dma_start(out=spq[:], in_=brc(sq_d, P, 1))
            nc.scalar.activation(out=spq[:], in_=spq[:], func=AF.Exp)
            nc.vector.tensor_scalar(out=spq[:], in0=spq[:], scalar1=1.0,
                                    scalar2=None, op0=A.add)
            nc.scalar.activation(out=spq[:], in_=spq[:], func=AF.Ln)

            _wcm.__exit__(None, None, None)
            _wcm = tc.tile_pool(name="pA", bufs=3)
            work = _wcm.__enter__()

            # ---------- P1: pass A (coordination numbers) ----------
            def ldslot(dram, c, pool, tag):
                t = pool.tile([P, TCH], F32, tag=tag)
                nc.sync.dma_start(
                    out=t[:],
                    in_=dram[c * CHSLOTS:(c + 1) * CHSLOTS].rearrange(
                        "(t p) -> p t", p=P))
                return t

            def ldsidx(c, pool, tag):
                t = pool.tile([P, GCH // 16], I16, tag=tag)
                nc.sync.dma_start(out=t[:], in_=sidx_d[c])
                return t

            def scatter_add(out_ap, in_ap, idxs_t, num, elem, step):
                return nc.gpsimd.dma_scatter_add(
                    out_ap, in_ap, idxs_t, num, num, elem, elem_step=step)

            srow_t = []
            for c in range(NCH):
                r_t = ldslot(r_d, c, work, "a_r")
                rcj = ldslot(rcj_d, c, work, "a_rcj")
                enj = ldslot(enj_d, c, work, "a_enj")
                rci = ldslot(rci_d, c, work, "a_rci")
                eni = ldslot(eni_d, c, work, "a_eni")
                sidx_t = ldsidx(c, work, "a_sidx")
                # interleave stage-2 srow prefetch gathers (needed only by
                # stage 2) so their DMA doesn't starve pass-A stream loads
                if (c % 2 == 1 or c == NCH - 1) and len(srow_t) < NACH:
                    k = len(srow_t)
                    spw_t = work.tile([P, (ACH * P) // 16], I16, tag="pf_spw")
                    nc.sync.dma_start(out=spw_t[:], in_=spw_d[k])
                    st = srowp.tile([P, ACH, 276], F32, tag=f"pf_srow{k}")
                    _dma_gather_raw(nc, st[:], srow_d[:, 0:276], spw_t[:],
                                    ACH * P, 276, SROWW)
                    srow_t.append(st)
                # rcv = 4/3*(rci+rcj)
                rcv = work.tile([P, TCH], F32, tag="a_rcv")
                nc.vector.tensor_tensor(out=rcv[:], in0=rci[:], in1=rcj[:], op=A.add)
                nc.vector.tensor_scalar(out=rcv[:], in0=rcv[:], scalar1=4.0 / 3.0,
                                        scalar2=None, op0=A.mult)
                # den = K4*exp(-((|eni-enj|+K5)^2)/K6) via sigmoid identity:
                # exp(-v) = 1/sigmoid(v) - 1
                den = work.tile([P, TCH], F32, tag="a_den")
                nc.vector.tensor_tensor(out=den[:], in0=eni[:], in1=enj[:],
                                        op=A.subtract)
                nc.scalar.activation(out=den[:], in_=den[:], func=AF.Abs)
                nc.scalar.activation(out=den[:], in_=den[:], func=AF.Square,
                                     bias=bk5_p[:, 0:1])
                nc.scalar.activation(out=den[:], in_=den[:], func=AF.Sigmoid,
                                     scale=1.0 / K6)
                nc.vector.reciprocal(out=den[:], in_=den[:])
                nc.vector.tensor_scalar(out=den[:], in0=den[:], scalar1=1.0,
                                        scalar2=0.5 * K4, op0=A.subtract,
                                        op1=A.mult)
                # erf(-KK*(rr-rcv)/rcv) = Erf(-KK*u + KK), u = rr/rcv
                cf = work.tile([P, TCH], F32, tag="a_cf")
                nc.vector.reciprocal(out=cf[:], in_=rcv[:])
                nc.vector.tensor_tensor(out=cf[:], in0=cf[:], in1=r_t[:], op=A.mult)
                nc.scalar.activation(out=cf[:], in_=cf[:], func=AF.Erf,
                                     scale=-KK / BOHR, bias=bkk_p[:, 0:1])
                # countf = (erf + 1) * den_scaled
                nc.vector.scalar_tensor_tensor(out=cf[:], in0=cf[:], scalar=1.0,
                                               in1=den[:], op0=A.add, op1=A.mult)
                # tree reduce GS -> 1
                l1 = work.tile([P, TCH // 2], F32, tag="a_l1")
                v = cf[:].rearrange("p (a two) -> p a two", two=2)
                nc.vector.tensor_tensor(out=l1[:], in0=v[:, :, 0], in1=v[:, :, 1],
                                        op=A.add)
                l2 = work.tile([P, TPG, 1], F32, tag="a_l2")
                v = l1[:].rearrange("p (a two) -> p a two", two=2)
                nc.vector.tensor_tensor(out=l2[:, :, 0],
                                        in0=v[:, :, 0], in1=v[:, :, 1],
                                        op=A.add)
                # scatter-add group sums into per-atom ncoord table
                scatter_add(nco_d[:, 0:1], l2[:], sidx_t[:], GCH, 1, BTW)

            _wcm.__exit__(None, None, None)
            _wcm = tc.tile_pool(name="pS2", bufs=2)
            work = _wcm.__enter__()

            # ---------- P2: stage 2 (per-atom A~ rows) ----------
            for k in range(NACH):
                sr = srow_t[k][:]
                nco = work.tile([P, ACH, 1], F32, tag="s2_nco")
                nc.sync.dma_start(
                    out=nco[:],
                    in_=nco_d.rearrange("(p a) f -> p a f", p=P)[
                        :, k * ACH:(k + 1) * ACH, 0:1])

                # gaussian weights gw[P, ACH, NREF]
                gw35 = work.tile([P, ACH, NREF * NC], F32, tag="s2_gw35")
                nc.vector.tensor_tensor(out=gw35[:],
                                        in0=_bc(nco[:, :, 0], NREF * NC),
                                        in1=sr[:, :, 44:79], op=A.subtract)
                nc.vector.tensor_tensor(out=gw35[:], in0=gw35[:], in1=gw35[:],
                                        op=A.mult)
                nc.vector.tensor_tensor(out=gw35[:], in0=gw35[:],
                                        in1=sr[:, :, 9:44], op=A.mult)
                nc.scalar.activation(out=gw35[:], in_=gw35[:], func=AF.Exp,
                                     scale=-6.0)
                nc.vector.tensor_tensor(out=gw35[:], in0=gw35[:],
                                        in1=sr[:, :, 79:114], op=A.mult)
                gw = work.tile([P, ACH, NREF], F32, tag="s2_gw")
                g5 = gw35[:].rearrange("p c (a n) -> p c a n", n=NC)
                nc.vector.tensor_tensor(out=gw[:], in0=g5[:, :, :, 0],
                                        in1=g5[:, :, :, 1], op=A.add)
                for n5 in range(2, NC):
                    nc.vector.tensor_tensor(out=gw[:], in0=gw[:],
                                            in1=g5[:, :, :, n5], op=A.add)
                nrm = work.tile([P, ACH], F32, tag="s2_nrm")
                nc.vector.tensor_reduce(out=nrm[:], in_=gw[:],
                                        axis=mybir.AxisListType.X, op=A.add)
                nc.vector.tensor_scalar(out=nrm[:], in0=nrm[:], scalar1=1e-7,
                                        scalar2=None, op0=A.max)
                nc.vector.reciprocal(out=nrm[:], in_=nrm[:])
                nc.vector.tensor_tensor(out=gw[:], in0=gw[:], in1=_bc(nrm[:], NREF),
                                        op=A.mult)
                # zeta
                chg_t = work.tile([P, ACH], F32, tag="s2_chg")
                nc.sync.dma_start(out=chg_t[:], in_=chg_d[:, k * ACH:(k + 1) * ACH])
                qmod2 = work.tile([P, ACH], F32, tag="s2_qm")
                nc.vector.tensor_tensor(out=qmod2[:], in0=chg_t[:],
                                        in1=sr[:, :, 1], op=A.add)
                msk2 = work.tile([P, ACH], F32, tag="s2_msk")
                nc.vector.tensor_scalar(out=msk2[:], in0=qmod2[:], scalar1=1e-8,
                                        scalar2=None, op0=A.is_gt)
                qs2 = work.tile([P, ACH], F32, tag="s2_qs")
                nc.vector.tensor_scalar(out=qs2[:], in0=qmod2[:], scalar1=1.0,
                                        scalar2=None, op0=A.subtract)
                nc.vector.tensor_tensor(out=qs2[:], in0=qs2[:], in1=msk2[:],
                                        op=A.mult)
                nc.vector.tensor_scalar(out=qs2[:], in0=qs2[:], scalar1=1.0,
                                        scalar2=None, op0=A.add)
                nc.vector.reciprocal(out=qs2[:], in_=qs2[:])
                zt = work.tile([P, ACH, NREF], F32, tag="s2_zt")
                nc.vector.tensor_scalar(out=zt[:], in0=sr[:, :, 2:9],
                                        scalar1=spq[:, 0:1], scalar2=None,
                                        op0=A.mult)
                nc.vector.tensor_tensor(out=zt[:], in0=zt[:],
                                        in1=_bc(sr[:, :, 1], NREF), op=A.add)
                nc.vector.tensor_tensor(out=zt[:], in0=zt[:],
                                        in1=_bc(qs2[:], NREF), op=A.mult)
                nc.vector.tensor_tensor(out=zt[:], in0=zt[:],
                                        in1=_bc(sr[:, :, 0], NREF), op=A.mult)
                nc.vector.tensor_tensor(out=zt[:], in0=_bc(sr[:, :, 0], NREF),
                                        in1=zt[:], op=A.subtract)
                nc.scalar.activation(out=zt[:], in_=zt[:], func=AF.Exp, scale=2.0)
                nc.scalar.activation(out=zt[:], in_=zt[:], func=AF.Exp,
                                     scale=-3.0, bias=b3_p[:, 0:1])
                zeta = work.tile([P, ACH, NREF], F32, tag="s2_zeta")
                mb = bass.AP(tensor=msk2[:].tensor, offset=msk2[:].offset,
                             ap=[*msk2[:].ap, [0, NREF]])
                nc.vector.tensor_scalar(out=zeta[:], in0=zt[:], scalar1=E3,
                                        scalar2=None, op0=A.subtract)
                nc.vector.tensor_tensor(out=zeta[:], in0=zeta[:], in1=mb,
                                        op=A.mult)
                nc.vector.tensor_scalar(out=zeta[:], in0=zeta[:], scalar1=E3,
                                        scalar2=None, op0=A.add)
                nc.vector.tensor_tensor(out=zeta[:], in0=zeta[:], in1=gw[:],
                                        op=A.mult)
                # A~_i[w] = sum_a zeta[a]*atil[a,w]
                t2row = work.tile([P, ACH, NW], F32, tag="s2_t2row")
                for a_ in range(NREF):
                    col = 114 + a_ * NW
                    if a_ == 0:
                        nc.vector.tensor_tensor(
                            out=t2row[:], in0=sr[:, :, col:col + NW],
                            in1=_bc(zeta[:, :, a_], NW), op=A.mult)
                    else:
                        tmp_ = work.tile([P, ACH, NW], F32, tag="s2_tmp")
                        nc.vector.tensor_tensor(
                            out=tmp_[:], in0=sr[:, :, col:col + NW],
                            in1=_bc(zeta[:, :, a_], NW), op=A.mult)
                        nc.vector.tensor_tensor(out=t2row[:], in0=t2row[:],
                                                in1=tmp_[:], op=A.add)
                # f32 copy for P5 + bf16 packed copy for AllGather
                nc.sync.dma_start(
                    out=t2s_d.rearrange("(p a) f -> p a f", p=P)[
                        :, k * ACH:(k + 1) * ACH, 0:NW],
                    in_=t2row[:])
                t2b = work.tile([P, ACH, NW], BF16, tag="s2_t2b")
                nc.vector.tensor_copy(out=t2b[:], in_=t2row[:])
                nc.sync.dma_start(
                    out=t2sb_d.rearrange("(p a) w -> p a w", p=P)[
                        :, k * ACH:(k + 1) * ACH, :],
                    in_=t2b[:])

            _wcm.__exit__(None, None, None)
            _srowcm.__exit__(None, None, None)

            # ---------- P3: AllGather packed bf16 A~ rows into strided table --
            # (emitted before pass B1 in program order so the Pool engine
            # starts the collective while the DVE computes damping factors)
            nc.gpsimd.collective_compute(
                "AllGather", A.bypass,
                replica_groups=[list(range(NCORES))],
                ins=[t2sb_d[:]], outs=[t2f_d[:, 0:NW]])

            _wcm = tc.tile_pool(name="pB", bufs=2)
            work = _wcm.__enter__()

            # ---------- P4a: pass B1 — damping factors (overlaps AllGather) --
            dbts = []
            for c in range(NCH):
                r_t = ldslot(r_d, c, work, "b_r")
                si_t = ldslot(si_d, c, work, "b_si")
                sj_t = ldslot(sj_d, c, work, "b_sj")
                r2 = work.tile([P, TCH], F32, tag="b_r2")
                nc.scalar.activation(out=r2[:], in_=r_t[:], func=AF.Square,
                                     scale=1.0 / BOHR)
                r6 = work.tile([P, TCH], F32, tag="b_r6")
                nc.vector.tensor_tensor(out=r6[:], in0=r2[:], in1=r2[:], op=A.mult)
                nc.vector.tensor_tensor(out=r6[:], in0=r6[:], in1=r2[:], op=A.mult)
                r8 = work.tile([P, TCH], F32, tag="b_r8")
                nc.vector.tensor_tensor(out=r8[:], in0=r6[:], in1=r2[:], op=A.mult)
                R3 = work.tile([P, TCH], F32, tag="b_R3")
                nc.vector.scalar_tensor_tensor(out=R3[:], in0=si_t[:], scalar=3.0,
                                               in1=sj_t[:], op0=A.mult,
                                               op1=A.mult)
                r0 = work.tile([P, TCH], F32, tag="b_r0")
                nc.scalar.activation(out=r0[:], in_=R3[:], func=AF.Sqrt)
                nc.vector.tensor_scalar(out=r0[:], in0=r0[:], scalar1=a1p,
                                        scalar2=a2p, op0=A.mult, op1=A.add)
                q2 = work.tile([P, TCH], F32, tag="b_q2")
                nc.vector.tensor_tensor(out=q2[:], in0=r0[:], in1=r0[:], op=A.mult)
                c4 = work.tile([P, TCH], F32, tag="b_c4")
                nc.vector.tensor_tensor(out=c4[:], in0=q2[:], in1=q2[:], op=A.mult)
                c3 = work.tile([P, TCH], F32, tag="b_c3")
                nc.vector.tensor_tensor(out=c3[:], in0=c4[:], in1=q2[:], op=A.mult)
                c8 = work.tile([P, TCH], F32, tag="b_c8")
                nc.vector.tensor_tensor(out=c8[:], in0=c4[:], in1=c4[:], op=A.mult)
                d6 = work.tile([P, TCH], F32, tag="b_d6")
                nc.vector.tensor_tensor(out=d6[:], in0=r6[:], in1=c3[:], op=A.add)
                nc.vector.reciprocal(out=d6[:], in_=d6[:])
                d8 = work.tile([P, TCH], F32, tag="b_d8")
                nc.vector.tensor_tensor(out=d8[:], in0=r8[:], in1=c8[:], op=A.add)
                nc.vector.reciprocal(out=d8[:], in_=d8[:])
                nc.vector.tensor_tensor(out=d8[:], in0=d8[:], in1=R3[:], op=A.mult)
                nc.vector.tensor_scalar(out=d8[:], in0=d8[:], scalar1=s8p,
                                        scalar2=None, op0=A.mult)
                d6c = const.tile([P, TCH], F32, tag=f"b_d6_{c}")
                nc.vector.scalar_tensor_tensor(out=d6c[:], in0=d6[:], scalar=s6p,
                                               in1=d8[:], op0=A.mult, op1=A.add)
                dbts.append(d6c)

            # ---------- P4b: pass B2 (gather + scale + reduce + scatter) -----
            for c in range(NCH):
                b = ch_bucket[c]
                sidx_t = ldsidx(c, work, "b_sidx")
                jw_t = work.tile([P, CALL // 16], I16, tag="b_jw")
                nc.sync.dma_start(out=jw_t[:], in_=jw_d[c])
                gt = work.tile([P, TCH, NW], BF16, tag="b_g")
                _dma_gather_raw(
                    nc, gt[:],
                    t2f_d[BBASE[b]:BBASE[b] + BSIZE[b], 0:NW],
                    jw_t[:], CALL, NW, P)
                d6c = dbts[c]
                # replicate D over the 23 w-columns on the Activation engine
                # (bf16 cast + broadcast), keeping the DVE mult in 2x mode
                db = bass.AP(tensor=d6c[:].tensor, offset=d6c[:].offset,
                             ap=[*d6c[:].ap, [0, NW]])
                drep = work.tile([P, TCH, NW], BF16, tag="b_drep")
                nc.scalar.activation(out=drep[:], in_=db, func=AF.Copy)
                nc.vector.tensor_tensor(out=gt[:], in0=gt[:], in1=drep[:],
                                        op=A.mult)
                # tree reduce over GS slots
                m1 = work.tile([P, TCH // 2, NW], BF16, tag="b_m1")
                v = gt[:].rearrange("p (a two) f -> p a two f", two=2)
                nc.vector.tensor_tensor(out=m1[:], in0=v[:, :, 0, :],
                                        in1=v[:, :, 1, :], op=A.add)
                m2 = work.tile([P, TPG, NW], BF16, tag="b_m2")
                v = m1[:].rearrange("p (a two) f -> p a two f", two=2)
                nc.vector.tensor_tensor(out=m2[:], in0=v[:, :, 0, :],
                                        in1=v[:, :, 1, :], op=A.add)
                # scatter-add group rows into per-atom B table
                scatter_add(btab_d[:, 0:NW], m2[:], sidx_t[:], GCH, NW, 2 * BTW)

            _wcm.__exit__(None, None, None)
            _wcm = tc.tile_pool(name="pE", bufs=2)
            work = _wcm.__enter__()

            # ---------- P5: assemble E ----------
            for k in range(NACH):
                bsum = work.tile([P, ACH, NW], BF16, tag="e_bsum")
                nc.sync.dma_start(
                    out=bsum[:],
                    in_=btab_d.rearrange("(p a) f -> p a f", p=P)[
                        :, k * ACH:(k + 1) * ACH, 0:NW])
                ai = work.tile([P, ACH, NW], F32, tag="e_ai")
                nc.sync.dma_start(
                    out=ai[:],
                    in_=t2s_d.rearrange("(p a) f -> p a f", p=P)[
                        :, k * ACH:(k + 1) * ACH, 0:NW])
                bsum32 = work.tile([P, ACH, NW], F32, tag="e_bsum32")
                nc.vector.tensor_copy(out=bsum32[:], in_=bsum[:])
                prod = work.tile([P, ACH, NW], F32, tag="e_prod")
                nc.vector.tensor_tensor(out=prod[:], in0=ai[:],
                                        in1=bsum32[:], op=A.mult)
                ev = work.tile([P, ACH], F32, tag="e_ev")
                nc.vector.tensor_reduce(out=ev[:], in_=prod[:],
                                        axis=mybir.AxisListType.X, op=A.add)
                nc.vector.tensor_scalar(out=ev[:], in0=ev[:],
                                        scalar1=-0.5 * HARTREE, scalar2=None,
                                        op0=A.mult)
                nc.sync.dma_start(
                    out=e_d.rearrange("(p a) -> p a", p=P)[:, k * ACH:(k + 1) * ACH],
                    in_=ev[:])
            _wcm.__exit__(None, None, None)
    return nc


_PROG_CACHE = {}


def kernel(**inputs):
    species = np.asarray(inputs["species"])
    per_core, meta = preprocess(species, inputs["edge_index"],
                                inputs["lengths"], inputs["partial_charges"])
    rcov = np.asarray(inputs["rcov"], np.float32)
    en = np.asarray(inputs["en"], np.float32)
    sr4 = np.asarray(inputs["sqrt_r4r2"], np.float32)
    refsys = np.asarray(inputs["refsys"]).astype(np.int64)

    # refsys-expanded tables (pure host-side permutation of inputs)
    zeff = np.asarray(inputs["zeff"], np.float32)
    sscale = np.asarray(inputs["sscale"], np.float32)
    gam = np.asarray(inputs["gam"], np.float32)
    secaiw = np.asarray(inputs["secaiw"], np.float32)
    zeff_r = zeff[refsys]
    sscale_r = sscale[refsys]
    gam_r = gam[refsys]
    secaiw_r = secaiw[refsys].reshape(Z, NREF * NW)

    import os as _os
    _bedrock = _os.environ.get("BEDROCK") == "1"
    if not _bedrock:
        key = tuple(meta["NGBS"])
        if key not in _PROG_CACHE:
            nc = build_program(meta)
            nc.finalize()
            _PROG_CACHE[key] = nc
        nc = _PROG_CACHE[key]

    shared = dict(
        zeff_r=zeff_r, sscale_r=sscale_r, gam_r=gam_r, secaiw_r=secaiw_r,
        refh=np.asarray(inputs["refh"], np.float32),
        ascale=np.asarray(inputs["ascale"], np.float32),
        hcount=np.asarray(inputs["hcount"], np.float32),
        refq=np.asarray(inputs["refq"], np.float32),
        alphaiw=np.asarray(inputs["alphaiw"], np.float32).reshape(Z, NREF * NW),
        gam=gam, zeff=zeff, sqrt_r4r2=sr4,
        ncount_weight=np.asarray(inputs["ncount_weight"], np.float32).reshape(Z, -1),
        cn=np.asarray(inputs["cn"], np.float32).reshape(Z, -1),
        ncount_mask=np.asarray(inputs["ncount_mask"], np.float32).reshape(Z, -1),
        cpw=np.asarray(inputs["cpw"], np.float32),
        s6_raw=np.asarray(inputs["s6_raw"], np.float32),
        s8_raw=np.asarray(inputs["s8_raw"], np.float32),
        a1_raw=np.asarray(inputs["a1_raw"], np.float32),
        a2_raw=np.asarray(inputs["a2_raw"], np.float32),
        scale_q_raw=np.asarray(inputs["scale_q_raw"], np.float32),
    )
    in_maps = []
    for c in range(NCORES):
        ci = build_core_inputs(per_core[c], meta, rcov, en, sr4)
        m = dict(shared)
        m.update(
            r_s=ci["r_s"], rcj_s=ci["rcj_s"], enj_s=ci["enj_s"],
            rci_s=ci["rci_s"], eni_s=ci["eni_s"], si_s=ci["si_s"],
            sj_s=ci["sj_s"],
            jw=ci["jw"].reshape(meta["NCH"], 128, CALL // 16),
            spw=ci["spw"].reshape(NACH, 128, (ACH * P) // 16),
            sidx=ci["sidx"].reshape(meta["NCH"], 128, GCH // 16),
            chg=ci["chg"],
        )
        in_maps.append(m)

    if _bedrock:
        outs = _sim_fallback(build_program(meta), in_maps)
    else:
        try:
            from concourse.bass_utils import run_bass_kernel_spmd
            res = run_bass_kernel_spmd(nc, in_maps, list(range(NCORES)))
            outs = [res.results[c]["e_out"] for c in range(NCORES)]
        except Exception:
            outs = _sim_fallback(build_program(meta), in_maps)
    e = np.concatenate(outs)
    return e[: species.shape[0]].astype(np.float32)


def _sim_fallback(nc, in_maps):
    import inspect
    import textwrap
    from scipy.special import erf as _scipy_erf
    from concourse import bass_interp
    src = textwrap.dedent(inspect.getsource(
        bass_interp.InstructionExecutor.visit_InstActivation))
    if "_scipy_erf" not in src:
        pat = ("    else:\n"
               "        # NOTE: If you are adding a new activation instruction")
        rep = ("    elif instruction.func == mb.ActivationFunctionType.Erf:\n"
               "        acted = _scipy_erf(scaled_and_biased)\n"
               "    else:\n"
               "        # NOTE: If you are adding a new activation instruction")
        assert pat in src
        src = src.replace(pat, rep)
        ns = dict(bass_interp.__dict__)
        ns["_scipy_erf"] = _scipy_erf
        exec(compile(src, "<erfpatch>", "exec"), ns)
        bass_interp.InstructionExecutor.visit_InstActivation = ns[
            "visit_InstActivation"]
    sim = bass_interp.MultiCoreSim(nc, NCORES, num_workers=1)
    for c in range(NCORES):
        for k, v in in_maps[c].items():
            sim.cores[c].tensor(k)[:] = v
    sim.simulate()
    global LAST_EXEC_TIME_NS
    LAST_EXEC_TIME_NS = int(getattr(sim, "global_time", 0))
    return [np.array(sim.cores[c].tensor("e_out")) for c in range(NCORES)]


LAST_EXEC_TIME_NS = None


# revision 43
# speedup vs baseline: 1.1094x; 1.1094x over previous
"""D4 dispersion energy kernel for 8 Trainium2 NeuronCores.

Strategy (v2):
- Host (numpy, integer/permutation work only): sort the edge list by (dst
  atom, j-range bucket), pad each (atom,bucket) edge run to a multiple of 8
  ("groups"), lay slots out in a fixed chunk/call/partition grid, and
  pre-permute all per-edge input data into that slot order.
- Device (all float math):
  * pass A computes per-edge coordination-number contributions and
    tree-reduces them into group sums, then dma_scatter_add's the group sums
    directly into a dense per-atom ncoord table;
  * stage 2 computes per-atom Gaussian weights / zeta / effective alpha
    table A~ from this core's atom slice (bf16, packed 23 floats/atom);
  * one AllGather shares the packed bf16 A~ rows into a 256B-stride table;
  * pass B gathers A~ rows for edge sources via dma_gather (bf16, 46B
    payload), applies Becke-Johnson damping, tree-reduces into group rows and
    dma_scatter_add's them into a dense per-atom B table;
  * E_i = -0.5*HARTREE * <A~_i, B_i>.
"""
import math
import numpy as np

import concourse.bass as bass
import concourse.bacc as bacc
import concourse.tile as tile
from concourse import mybir
from concourse.library_config import mlp as mlp_library

F32 = mybir.dt.float32
BF16 = mybir.dt.bfloat16
I16 = mybir.dt.int16

Z = 87
NREF = 7
NC = 5
NW = 23
BOHR = 0.5291772105638411
HARTREE = 27.211386024367243
K4, K5, K6, KK = 4.10451, 19.08857, 254.5553148552, 7.5
E3 = float(np.exp(3.0))
CPFAC = 3.0 / (2.0 * np.pi)

NCORES = 8
P = 128
ACOLS = 80              # atom columns per partition -> NA = 128*80
NA = P * ACOLS          # atoms per core (10240)
NPAD = NCORES * NA      # padded atom count (81920)
ACH = 16                # atom columns per stage-2 chunk (2048 atoms)
NACH = ACOLS // ACH     # atom chunks per core (5)
CALL = 32768            # idxs per dma_gather call (one per chunk)
TCH = 256               # slots per partition per compute chunk
GS = 4                  # slots per group
CHSLOTS = P * TCH       # slots per compute chunk (32768)
GCH = CHSLOTS // GS     # groups per chunk (4096)
TPG = TCH // GS         # group cells per partition per chunk (32)

# j-range buckets (dma_gather idx is int16)
NBUCK = 3
BBASE = [0, 27307, 54614]
BSIZE = [27307, 27307, NPAD - 54614]

SECB = 6144             # ncoord lo/hi section boundary (atoms, 3 s2 chunks)
SROWW = 320             # per-species row width (f32); 1280 B, 256-aligned
BTW = 64                # per-atom table row width (f32); 256 B stride


def _wrap16(idx_lin):
    """int linear idx list -> [16, ceil(n/16)] int16 wrapped tile.

    The gather/scatter ucode reads indices from the first 16 partitions
    only, so the upload carries just those rows (the SBUF tile is still
    128 partitions tall; rows 16-127 are never read)."""
    n = len(idx_lin)
    m = (n + 15) // 16
    pad = np.zeros(m * 16, np.int16)
    pad[:n] = idx_lin.astype(np.int16)
    return np.ascontiguousarray(pad.reshape(m, 16).T)  # [16, m]


def preprocess(species, edge_index, lengths, partial_charges):
    """Build per-core host-side data. Returns (per_core list of dicts, meta)."""
    n_at = species.shape[0]
    species = np.asarray(species).astype(np.int32)
    idx_i = np.asarray(edge_index[0]).astype(np.int64)
    idx_j = np.asarray(edge_index[1]).astype(np.int64)
    lengths = np.asarray(lengths).astype(np.float32)
    charges = np.asarray(partial_charges).astype(np.float32)

    spec_pad = np.zeros(NPAD, np.int32)
    spec_pad[:n_at] = species
    chg_pad = np.zeros(NPAD, np.float32)
    chg_pad[:n_at] = charges

    # bucket of each edge by j range
    jb = np.searchsorted(np.array(BBASE[1:]), idx_j, side="right")  # 0..2
    key = idx_i * NBUCK + jb
    order = np.argsort(key, kind="stable")
    si = idx_i[order]
    sj = idx_j[order]
    sl = lengths[order]
    sjb = jb[order]

    # count edges per (atom, bucket)
    cnt = np.bincount(idx_i * NBUCK + jb, minlength=NPAD * NBUCK).reshape(NPAD, NBUCK)
    grp = (cnt + GS - 1) // GS  # groups per (atom,bucket)
    # CSR offsets into sorted edge array for (atom,bucket)
    flat_cnt = cnt.reshape(-1)
    edge_off = np.zeros(NPAD * NBUCK + 1, np.int64)
    np.cumsum(flat_cnt, out=edge_off[1:])

    # group quota per bucket (max over cores, rounded to chunk multiple)
    grp_cb = grp.reshape(NCORES, NA, NBUCK).sum(axis=1)  # [core, bucket]
    NGBS = []
    for b in range(NBUCK):
        m = int(grp_cb[:, b].max())
        NGBS.append(((m + GCH - 1) // GCH) * GCH)
    NG = sum(NGBS)                       # groups per core
    SLOTS = NG * GS                      # slots per core
    NCH = SLOTS // CHSLOTS               # compute chunks
    assert SLOTS % CHSLOTS == 0
    # chunk -> bucket map (buckets are whole chunks)
    ch_bucket = []
    for b in range(NBUCK):
        ch_bucket += [b] * (NGBS[b] * GS // CHSLOTS)
    gb_off = np.concatenate([[0], np.cumsum(NGBS)])  # group offset per bucket

    meta = dict(NGBS=NGBS, NG=NG, SLOTS=SLOTS, NCH=NCH, ch_bucket=ch_bucket)

    per_core = []
    for c in range(NCORES):
        a0 = c * NA
        g_c = grp[a0 : a0 + NA]                 # [NA, NBUCK]
        gofs = np.zeros((NA + 1, NBUCK), np.int64)
        np.cumsum(g_c, axis=0, out=gofs[1:])
        ng_b = gofs[NA]                          # real groups per bucket
        for b in range(NBUCK):
            assert ng_b[b] <= NGBS[b]

        # atom id of each core-local group (bucket-sectioned, then padded)
        atom_of_G = np.full(NG, -1, np.int32)   # pads -> -1 (trash rows)
        for b in range(NBUCK):
            rep = np.repeat(np.arange(NA, dtype=np.int32), g_c[:, b])
            atom_of_G[gb_off[b] : gb_off[b] + len(rep)] = rep

        # slot position for each real edge:
        atom_l = si - a0
        core_mask = (atom_l >= 0) & (atom_l < NA)
        e_sel = np.nonzero(core_mask)[0]
        al = atom_l[e_sel]
        eb = sjb[e_sel]
        flat_id = (si[e_sel] * NBUCK + eb)
        rank = (e_sel - edge_off[flat_id])
        grank = rank // GS
        lane = rank % GS
        G = gb_off[eb] + gofs[al, eb] + grank    # core-local group id
        # group cell mapping: scatter token index == group rank within the
        # chunk (atom-monotone), so token prefixes map to atom ranges.
        # token t -> (partition t%128, cell t//128); slots of a group are GS
        # consecutive columns of one partition.
        c_ch = G // GCH
        pp = G % 128
        tg = (G % GCH) // 128
        pos = c_ch * CHSLOTS + (tg * GS + lane) * P + pp

        # per-slot streams (defaults for pad slots)
        r_s = np.full(SLOTS, 1.0e4, np.float32)
        rcj_s = np.ones(SLOTS, np.float32)
        enj_s = np.ones(SLOTS, np.float32)
        rci_s = np.ones(SLOTS, np.float32)
        eni_s = np.ones(SLOTS, np.float32)
        si_s = np.ones(SLOTS, np.float32)
        sj_s = np.ones(SLOTS, np.float32)
        jl_s = np.zeros(SLOTS, np.int32)

        r_s[pos] = sl[e_sel]
        jl_s[pos] = sj[e_sel] - np.array(BBASE, np.int64)[eb]

        # pass-B scatter idx per chunk: token t == group rank in chunk
        sidx = np.zeros((NCH, 16, GCH // 16), np.int16)
        for ch in range(NCH):
            av = atom_of_G[ch * GCH : (ch + 1) * GCH].copy()
            av[av < 0] = 0   # pad groups sum to ~0; row 0 is harmless
            sidx[ch] = _wrap16(av)



        per_core.append(dict(
            pos=pos, e_sel=e_sel, sj=sj[e_sel], sp_i=spec_pad[si[e_sel]],
            sp_j=spec_pad[sj[e_sel]], atom_of_G=atom_of_G,
            r_s=r_s, rcj_s=rcj_s, enj_s=enj_s, rci_s=rci_s, eni_s=eni_s,
            si_s=si_s, sj_s=sj_s, jl_s=jl_s, sidx=sidx,
            spec_slice=spec_pad[a0 : a0 + NA], chg_slice=chg_pad[a0 : a0 + NA],
        ))
    # pass-A sub-scatter structure: split ncoord into lo [0,SECB) / hi
    # [SECB,NA) tables so stage 2 can start before pass A finishes. Token
    # ranges are uniform across cores (SPMD); out-of-section tokens in the
    # overlap zone hit a trash row.
    asub = []      # per slot-chunk: list of (section, tok0, ntok, rowid)
    nrow = 0
    for ch in range(NCH):
        t1s, t0s = [], []
        for pc in per_core:
            av = pc["atom_of_G"][ch * GCH : (ch + 1) * GCH]
            lo = (av >= 0) & (av < SECB)
            hi = av >= SECB
            if lo.any():
                t1s.append(int(np.nonzero(lo)[0][-1]) + 1)
            if hi.any():
                t0s.append(int(np.nonzero(hi)[0][0]))
        subs = []
        if t1s:
            t1 = min((max(t1s) + 127) // 128 * 128, GCH)
            subs.append((0, 0, t1, nrow)); nrow += 1
        if t0s:
            t0 = min(t0s) // 128 * 128
            subs.append((1, t0, GCH - t0, nrow)); nrow += 1
        asub.append(subs)
    meta["asub"] = asub
    meta["NASC"] = nrow
    for pc in per_core:
        aidx = np.zeros((nrow, 16, GCH // 16), np.int16)
        for ch in range(NCH):
            av = pc["atom_of_G"][ch * GCH : (ch + 1) * GCH]
            for (sec, t0, ntok, row) in asub[ch]:
                iv = av[t0:t0 + ntok].copy()
                if sec == 0:
                    bad = ~((iv >= 0) & (iv < SECB))
                    iv[bad] = SECB
                else:
                    sel = iv >= SECB
                    iv = np.where(sel, iv - SECB, NA - SECB)
                aidx[row, :, :ntok // 16] = _wrap16(iv)
        pc["aidx"] = aidx
    return per_core, meta


def build_core_inputs(pc, meta, rcov, en, sqrt_r4r2):
    """Fill species-derived streams + wrapped idx arrays for one core."""
    SLOTS, NCH = meta["SLOTS"], meta["NCH"]
    pos = pc["pos"]
    pc["rcj_s"][pos] = rcov[pc["sp_j"]]
    pc["enj_s"][pos] = en[pc["sp_j"]]
    pc["rci_s"][pos] = rcov[pc["sp_i"]]
    pc["eni_s"][pos] = en[pc["sp_i"]]
    pc["si_s"][pos] = sqrt_r4r2[pc["sp_i"]]
    pc["sj_s"][pos] = sqrt_r4r2[pc["sp_j"]]

    # jidx16: one gather call per chunk, wrapped
    jl = pc["jl_s"]
    jw = np.zeros((NCH, 16, CALL // 16), np.int16)
    for k in range(NCH):
        jw[k] = _wrap16(jl[k * CALL : (k + 1) * CALL])

    # species wrap per atom chunk (2048 atoms): idx position u*128+p ->
    # local atom (16k+u)*128+p  (atom id = col*128 + p)
    spw = np.zeros((NACH, 16, (ACH * P) // 16), np.int16)
    spec = pc["spec_slice"].reshape(ACOLS, P)
    for k in range(NACH):
        lin = spec[k * ACH : (k + 1) * ACH, :].reshape(-1)  # [u, p] -> u*128+p
        spw[k] = _wrap16(lin)

    pa_pack = np.stack([
        pc["r_s"].reshape(NCH, CHSLOTS), pc["rcj_s"].reshape(NCH, CHSLOTS),
        pc["enj_s"].reshape(NCH, CHSLOTS), pc["rci_s"].reshape(NCH, CHSLOTS),
        pc["eni_s"].reshape(NCH, CHSLOTS)], axis=1)
    pb_pack = np.stack([
        pc["r_s"].reshape(NCH, CHSLOTS), pc["si_s"].reshape(NCH, CHSLOTS),
        pc["sj_s"].reshape(NCH, CHSLOTS)], axis=1)
    return dict(
        pa_pack=pa_pack, pb_pack=pb_pack,
        jw=jw.reshape(-1), spw=spw.reshape(-1), sidx=pc["sidx"].reshape(-1),
        aidx=pc["aidx"].reshape(-1),
        chg=np.ascontiguousarray(
            pc["chg_slice"].reshape(ACOLS, P).T).astype(np.float32),
    )


def _bc(ap, n):
    """Broadcast AP: append a step-0 inner dim of size n."""
    return bass.AP(tensor=ap.tensor, offset=ap.offset, ap=[*ap.ap, [0, n]])


def _dma_gather_raw(nc, out_ap, in_ap, idxs_ap, num_idxs, elem_size, elem_step):
    """dma_gather without the elem_size%256 restriction (payload < row pitch).
    Mirrors bass.BassGpSimd.dma_gather (non-transpose, DRAM source)."""
    eng = nc.gpsimd
    assert idxs_ap.dtype == mybir.dt.int16
    assert in_ap.dtype == out_ap.dtype
    stride_bytes = elem_step * mybir.dt.size(in_ap.dtype)
    assert stride_bytes % 256 == 0
    stride_bytes_256 = stride_bytes // 256
    assert in_ap.ap[0][0] == elem_step
    assert in_ap.ap[-1][1] == elem_size
    assert out_ap.ap[-1][1] == elem_size
    _in_ap = eng.lower_ap_dma(in_ap, for_custom_bir_dma=True)
    _idxs_ap = eng.lower_ap(idxs_ap)
    _out_ap = eng.lower_ap(out_ap)
    return eng.add_instruction(
        mybir.InstDMAGatherAnt(
            name=nc.get_next_instruction_name(),
            ins=[*_in_ap, _idxs_ap, eng.lower_val_access(eng.to_reg(num_idxs))],
            outs=[_out_ap],
            transpose=False,
            num_idxs=num_idxs,
            elem_size=elem_size,
            stride_bytes_256=stride_bytes_256,
            gen_mode=0,
            single_packet=True,
            queue_num=0,
            sbuf_tokens_per_rank=0,
            sbuf_free_dim_per_rank=0,
            sbuf_free_dim_pad_per_rank=0,
            sbuf_byte_offset=0,
        )
    )


def build_program(meta):
    SLOTS, NCH = meta["SLOTS"], meta["NCH"]
    ch_bucket = meta["ch_bucket"]
    A = mybir.AluOpType
    AF = mybir.ActivationFunctionType

    nc = bacc.Bacc(None, num_devices=NCORES, dynamic_dma_scratch_size=40960)

    def din(name, shape, dt=F32):
        return nc.dram_tensor(name, shape, dt, kind="ExternalInput")

    # per-slot streams, packed stream-major per chunk (one DMA per chunk)
    pa_d = din("pa_pack", [NCH, 5, CHSLOTS])
    pb_d = din("pb_pack", [NCH, 3, CHSLOTS])
    jw_d = din("jw", [NCH, 16, CALL // 16], I16)
    spw_d = din("spw", [NACH, 16, (ACH * P) // 16], I16)
    sidx_d = din("sidx", [NCH, 16, GCH // 16], I16)
    aidx_d = din("aidx", [meta["NASC"], 16, GCH // 16], I16)
    chg_d = din("chg", [P, ACOLS])
    # tables
    zeffr_d = din("zeff_r", [Z, NREF]); sscr_d = din("sscale_r", [Z, NREF])
    gamr_d = din("gam_r", [Z, NREF]); refh_d = din("refh", [Z, NREF])
    asc_d = din("ascale", [Z, NREF]); hcnt_d = din("hcount", [Z, NREF])
    refq_d = din("refq", [Z, NREF])
    secr_d = din("secaiw_r", [Z, NREF * NW]); aiw_d = din("alphaiw", [Z, NREF * NW])
    gam_d = din("gam", [Z]); zeff_d = din("zeff", [Z]); sr4_d = din("sqrt_r4r2", [Z])
    cnw_d = din("ncount_weight", [Z, NREF * NC]); cnd_d = din("cn", [Z, NREF * NC])
    msk_d = din("ncount_mask", [Z, NREF * NC])
    cpw_d = din("cpw", [NW])
    s6_d = din("s6_raw", [1]); s8_d = din("s8_raw", [1])
    a1_d = din("a1_raw", [1]); a2_d = din("a2_raw", [1]); sq_d = din("scale_q_raw", [1])

    srowA_d = nc.dram_tensor("srowad", [Z, 128], F32)
    srowB_d = nc.dram_tensor("srowbd", [Z, 256], BF16)
    ncoL_d = nc.dram_tensor("ncold", [SECB + 16, BTW], F32)
    ncoH_d = nc.dram_tensor("ncohd", [NA - SECB + 16, BTW], F32)
    btab_d = nc.dram_tensor("btabd", [NA, 2 * BTW], BF16)
    t2sb_d = nc.dram_tensor("t2sb", [NA, NW], BF16)
    t2f_d = nc.dram_tensor("t2f", [NPAD, P], BF16, addr_space="Shared")
    e_d = nc.dram_tensor("e_out", [NA], F32, kind="ExternalOutput")

    def brc(dram, parts, width):
        """AP reading a [width] DRAM tensor broadcast across `parts` partitions."""
        return bass.AP(tensor=dram.tensor if hasattr(dram, "tensor") else dram,
                       offset=0, ap=[[0, parts], [1, width]])

    with tile.TileContext(nc) as tc:
        import contextlib
        with contextlib.ExitStack() as ctx:
            const = ctx.enter_context(tc.tile_pool(name="const", bufs=1))
            _srowcm = tc.tile_pool(name="srowp", bufs=1)
            srowp = _srowcm.__enter__()
            _wcm = tc.tile_pool(name="p0", bufs=2)
            work = _wcm.__enter__()

            nc.gpsimd.load_library(mlp_library)

            # dedicated index tiles: ucode reads only rows 0:16, so uploads
            # write just those rows; memset once here to satisfy init checks
            jwt_a = const.tile([P, CALL // 16], I16, tag="jwt0")
            jwt_b = const.tile([P, CALL // 16], I16, tag="jwt1")
            sxt_a = const.tile([P, GCH // 16], I16, tag="sxt0")
            sxt_b = const.tile([P, GCH // 16], I16, tag="sxt1")
            spwt = const.tile([P, (ACH * P) // 16], I16, tag="spwt")
            axt_a = const.tile([P, GCH // 16], I16, tag="axt0")
            axt_b = const.tile([P, GCH // 16], I16, tag="axt1")
            axt_c = const.tile([P, GCH // 16], I16, tag="axt2")
            axt_d = const.tile([P, GCH // 16], I16, tag="axt3")
            jwt2 = [jwt_a, jwt_b]
            sxt2 = [sxt_a, sxt_b]
            axt4 = [axt_a, axt_b, axt_c, axt_d]
            for t_ in (*jwt2, *sxt2, *axt4, spwt):
                nc.gpsimd.memset(t_[:], 0)

            b3_87 = const.tile([Z, 1], F32)
            nc.vector.memset(b3_87[:], 3.0)
            b3_p = const.tile([P, 1], F32)
            nc.vector.memset(b3_p[:], 3.0)
            bk5_p = const.tile([P, 1], F32)
            nc.vector.memset(bk5_p[:], K5)
            bkk_p = const.tile([P, 1], F32)
            nc.vector.memset(bkk_p[:], KK)

            # zero rows of nco/btab tables (only the columns we touch)
            zcol = const.tile([P, ACOLS, 1], F32)
            nc.vector.memset(zcol[:], 0.0)
            nc.sync.dma_start(
                out=ncoL_d[0:SECB].rearrange("(a p) f -> p a f", p=P)[:, :, 0:1],
                in_=zcol[:, 0:SECB // P, :])
            nc.sync.dma_start(
                out=ncoH_d[0:NA - SECB].rearrange("(a p) f -> p a f", p=P)[:, :, 0:1],
                in_=zcol[:, 0:(NA - SECB) // P, :])
            zrow23 = const.tile([P, ACOLS, NW], BF16)
            nc.vector.memset(zrow23[:], 0.0)
            nc.sync.dma_start(
                out=btab_d[0:NA].rearrange("(a p) f -> p a f", p=P)[:, :, 0:NW],
                in_=zrow23[:])

            # ---------- P0: per-species row table ----------
            def ld87(dram, w):
                t = const.tile([Z, w], F32, tag=f"ld_{dram.name}")
                nc.sync.dma_start(out=t[:], in_=dram[:] if w > 1 else dram[:, None])
                return t

            zeffr = ld87(zeffr_d, NREF); sscr = ld87(sscr_d, NREF)
            gamr = ld87(gamr_d, NREF); refh = ld87(refh_d, NREF)
            asc = ld87(asc_d, NREF); hcnt = ld87(hcnt_d, NREF)
            refq = ld87(refq_d, NREF)
            secr = ld87(secr_d, NREF * NW); aiw = ld87(aiw_d, NREF * NW)
            gam1 = ld87(gam_d, 1); zeff1 = ld87(zeff_d, 1); sr41 = ld87(sr4_d, 1)
            cnw = ld87(cnw_d, NREF * NC); cnt_ = ld87(cnd_d, NREF * NC)
            msk = ld87(msk_d, NREF * NC)

            # softplus of all 5 scalar params in one Exp->Ln block
            params = const.tile([P, 5], F32)
            for ii, dd in enumerate([s6_d, s8_d, a1_d, a2_d, sq_d]):
                nc.sync.dma_start(out=params[:, ii:ii+1], in_=brc(dd, P, 1))
            nc.scalar.activation(out=params[:], in_=params[:], func=AF.Exp)
            nc.vector.tensor_scalar(out=params[:], in0=params[:], scalar1=1.0,
                                    scalar2=None, op0=A.add)
            nc.scalar.activation(out=params[:], in_=params[:], func=AF.Ln)
            s6p, s8p = params[:, 0:1], params[:, 1:2]
            a1p, a2p = params[:, 2:3], params[:, 3:4]
            spq = params[:, 4:5]
            sq87 = params[0:Z, 4:5]

            qmod = work.tile([Z, NREF], F32, tag="p0a")
            nc.vector.tensor_scalar(out=qmod[:], in0=refh[:], scalar1=sq87,
                                    scalar2=None, op0=A.mult)
            nc.vector.tensor_tensor(out=qmod[:], in0=qmod[:], in1=zeffr[:], op=A.add)
            qmsk = work.tile([Z, NREF], F32, tag="p0b")
            nc.vector.tensor_scalar(out=qmsk[:], in0=qmod[:], scalar1=1e-8,
                                    scalar2=None, op0=A.is_gt)
            qsafe = work.tile([Z, NREF], F32, tag="p0c")
            nc.vector.tensor_scalar(out=qsafe[:], in0=qmod[:], scalar1=1.0,
                                    scalar2=None, op0=A.subtract)
            nc.vector.tensor_tensor(out=qsafe[:], in0=qsafe[:], in1=qmsk[:],
                                    op=A.mult)
            nc.vector.tensor_scalar(out=qsafe[:], in0=qsafe[:], scalar1=1.0,
                                    scalar2=None, op0=A.add)
            rq = work.tile([Z, NREF], F32, tag="p0d")
            nc.vector.reciprocal(out=rq[:], in_=qsafe[:])
            t0 = work.tile([Z, NREF], F32, tag="p0e")
            nc.vector.tensor_tensor(out=t0[:], in0=zeffr[:], in1=rq[:], op=A.mult)
            nc.vector.tensor_tensor(out=t0[:], in0=t0[:], in1=gamr[:], op=A.mult)
            nc.vector.tensor_tensor(out=t0[:], in0=gamr[:], in1=t0[:], op=A.subtract)
            nc.scalar.activation(out=t0[:], in_=t0[:], func=AF.Exp, scale=2.0)
            nc.scalar.activation(out=t0[:], in_=t0[:], func=AF.Exp, scale=-3.0,
                                 bias=b3_87[:, 0:1])
            zfac = work.tile([Z, NREF], F32, tag="p0f")
            nc.vector.tensor_scalar(out=zfac[:], in0=t0[:], scalar1=E3,
                                    scalar2=None, op0=A.subtract)
            nc.vector.tensor_tensor(out=zfac[:], in0=zfac[:], in1=qmsk[:],
                                    op=A.mult)
            nc.vector.tensor_scalar(out=zfac[:], in0=zfac[:], scalar1=E3,
                                    scalar2=None, op0=A.add)
            al = work.tile([Z, NREF, NW], F32, tag="p0g")
            nc.vector.tensor_tensor(
                out=al[:], in0=secr[:].rearrange("z (a w) -> z a w", w=NW),
                in1=_bc(sscr[:], NW), op=A.mult)
            nc.vector.tensor_tensor(out=al[:], in0=al[:], in1=_bc(zfac[:], NW),
                                    op=A.mult)
            nc.vector.tensor_tensor(out=al[:], in0=al[:], in1=_bc(hcnt[:], NW),
                                    op=A.mult)
            nc.vector.tensor_tensor(
                out=al[:], in0=aiw[:].rearrange("z (a w) -> z a w", w=NW),
                in1=al[:], op=A.subtract)
            nc.vector.tensor_tensor(out=al[:], in0=al[:], in1=_bc(asc[:], NW),
                                    op=A.mult)
            nc.vector.tensor_scalar(out=al[:], in0=al[:], scalar1=0.0,
                                    scalar2=None, op0=A.max)
            cpw87 = const.tile([Z, NW], F32)
            nc.sync.dma_start(out=cpw87[:], in_=brc(cpw_d, Z, NW))
            nc.scalar.activation(out=cpw87[:], in_=cpw87[:], func=AF.Sqrt,
                                 scale=CPFAC)
            wb = bass.AP(tensor=cpw87[:].tensor, offset=cpw87[:].offset,
                         ap=[cpw87[:].ap[0], [0, NREF], [1, NW]])
            nc.vector.tensor_tensor(out=al[:], in0=al[:], in1=wb, op=A.mult)

            # assemble split species rows: f32 part + bf16 atil part
            srow = const.tile([Z, 128], F32)
            nc.vector.memset(srow[:], 0.0)
            nc.vector.tensor_copy(out=srow[:, 0:1], in_=gam1[:])
            nc.vector.tensor_copy(out=srow[:, 1:2], in_=zeff1[:])
            nc.vector.tensor_copy(out=srow[:, 2:9], in_=refq[:])
            nc.vector.tensor_copy(out=srow[:, 9:44], in_=cnw[:])
            nc.vector.tensor_copy(out=srow[:, 44:79], in_=cnt_[:])
            nc.vector.tensor_copy(out=srow[:, 79:114], in_=msk[:])
            nc.sync.dma_start(out=srowA_d[:], in_=srow[:])
            srowb = const.tile([Z, 256], BF16)
            nc.gpsimd.memset(srowb[:], 0)
            nc.scalar.activation(out=srowb[:, 0:161],
                                 in_=al[:].rearrange("z a w -> z (a w)"),
                                 func=AF.Copy)
            nc.sync.dma_start(out=srowB_d[:], in_=srowb[:])


            _wcm.__exit__(None, None, None)
            _wcm = tc.tile_pool(name="pA", bufs=3)
            work = _wcm.__enter__()

            # ---------- P1: pass A (coordination numbers) ----------
            def ldsidx(c, pool, tag):
                t = sxt2[c % 2]
                nc.sync.dma_start(out=t[0:16, :], in_=sidx_d[c])
                return t

            def scatter_add(out_ap, in_ap, idxs_t, num, elem, step):
                return nc.gpsimd.dma_scatter_add(
                    out_ap, in_ap, idxs_t, num, num, elem, elem_step=step)

            srow_t = []
            asub = meta["asub"]

            def emit_passA(c, pos_):
                st = work.tile([P, 5, TCH], F32, tag="a_st")
                nc.sync.dma_start(
                    out=st[:], in_=pa_d[c].rearrange("v (t p) -> p v t", p=P))
                r_t, rcj, enj = st[:, 0, :], st[:, 1, :], st[:, 2, :]
                rci, eni = st[:, 3, :], st[:, 4, :]
                # interleave stage-2 srow prefetch gathers (needed only by
                # stage 2) so their DMA doesn't starve pass-A stream loads
                if (pos_ % 2 == 1 or pos_ == NCH - 1) and len(srow_t) < NACH:
                    k = len(srow_t)
                    spw_t = spwt
                    nc.sync.dma_start(out=spw_t[0:16, :], in_=spw_d[k])
                    sg = srowp.tile([P, ACH, 115], F32, tag=f"pf_srow{k}")
                    _dma_gather_raw(nc, sg[:], srowA_d[:, 0:115], spw_t[:],
                                    ACH * P, 115, 128)
                    sgb = srowp.tile([P, ACH, 161], BF16, tag=f"pf_srowb{k}")
                    _dma_gather_raw(nc, sgb[:], srowB_d[:, 0:161], spw_t[:],
                                    ACH * P, 161, 256)
                    srow_t.append((sg, sgb))
                # rcv = 4/3*(rci+rcj)
                rcv = work.tile([P, TCH], F32, tag="a_rcv")
                nc.vector.tensor_tensor(out=rcv[:], in0=rci, in1=rcj, op=A.add)
                nc.vector.tensor_scalar(out=rcv[:], in0=rcv[:], scalar1=4.0 / 3.0,
                                        scalar2=None, op0=A.mult)
                # den = K4*exp(-((|eni-enj|+K5)^2)/K6) via sigmoid identity:
                # exp(-v) = 1/sigmoid(v) - 1
                den = work.tile([P, TCH], F32, tag="a_den")
                nc.vector.tensor_tensor(out=den[:], in0=eni, in1=enj,
                                        op=A.subtract)
                nc.scalar.activation(out=den[:], in_=den[:], func=AF.Abs)
                nc.scalar.activation(out=den[:], in_=den[:], func=AF.Square,
                                     bias=bk5_p[:, 0:1])
                nc.scalar.activation(out=den[:], in_=den[:], func=AF.Sigmoid,
                                     scale=1.0 / K6)
                nc.vector.reciprocal(out=den[:], in_=den[:])
                nc.vector.tensor_scalar(out=den[:], in0=den[:], scalar1=1.0,
                                        scalar2=0.5 * K4, op0=A.subtract,
                                        op1=A.mult)
                # erf(-KK*(rr-rcv)/rcv) = Erf(-KK*u + KK), u = rr/rcv
                cf = work.tile([P, TCH], F32, tag="a_cf")
                nc.vector.reciprocal(out=cf[:], in_=rcv[:])
                nc.vector.tensor_tensor(out=cf[:], in0=cf[:], in1=r_t, op=A.mult)
                nc.scalar.activation(out=cf[:], in_=cf[:], func=AF.Erf,
                                     scale=-KK / BOHR, bias=bkk_p[:, 0:1])
                # countf = (erf + 1) * den_scaled
                nc.vector.scalar_tensor_tensor(out=cf[:], in0=cf[:], scalar=1.0,
                                               in1=den[:], op0=A.add, op1=A.mult)
                # tree reduce GS -> 1
                l1 = work.tile([P, TCH // 2], F32, tag="a_l1")
                v = cf[:].rearrange("p (a two) -> p a two", two=2)
                nc.vector.tensor_tensor(out=l1[:], in0=v[:, :, 0], in1=v[:, :, 1],
                                        op=A.add)
                l2 = work.tile([P, TPG, 1], F32, tag="a_l2")
                v = l1[:].rearrange("p (a two) -> p a two", two=2)
                nc.vector.tensor_tensor(out=l2[:, :, 0],
                                        in0=v[:, :, 0], in1=v[:, :, 1],
                                        op=A.add)
                # scatter-add group sums into lo/hi per-atom ncoord tables
                for (sec, t0, ntok, row) in asub[c]:
                    it = axt4[(pos_ * 2 + sec) % 4]
                    nc.sync.dma_start(out=it[0:16, 0:ntok // 16],
                                      in_=aidx_d[row][:, 0:ntok // 16])
                    tab = ncoL_d if sec == 0 else ncoH_d
                    scatter_add(tab[:, 0:1],
                                l2[:, t0 // 128:(t0 + ntok) // 128, :],
                                it[:, 0:ntok // 16], ntok, 1, BTW)

            _s2cm = tc.tile_pool(name="pS2", bufs=2)
            works2 = _s2cm.__enter__()

            # ---------- P2: stage 2 (per-atom A~ rows) ----------
            def emit_s2(k):
                work = works2
                sr = srow_t[k][0][:]
                srb = srow_t[k][1][:]
                nco = work.tile([P, ACH, 1], F32, tag="s2_nco")
                if k * ACH < SECB // P:
                    nsrc = ncoL_d[0:SECB].rearrange("(a p) f -> p a f", p=P)[
                        :, k * ACH:(k + 1) * ACH, 0:1]
                else:
                    k2 = k - SECB // P // ACH
                    nsrc = ncoH_d[0:NA - SECB].rearrange(
                        "(a p) f -> p a f", p=P)[
                        :, k2 * ACH:(k2 + 1) * ACH, 0:1]
                nc.sync.dma_start(out=nco[:], in_=nsrc)

                # gaussian weights gw[P, ACH, NREF]
                gw35 = work.tile([P, ACH, NREF * NC], F32, tag="s2_gw35")
                nc.vector.tensor_tensor(out=gw35[:],
                                        in0=_bc(nco[:, :, 0], NREF * NC),
                                        in1=sr[:, :, 44:79], op=A.subtract)
                nc.vector.tensor_tensor(out=gw35[:], in0=gw35[:], in1=gw35[:],
                                        op=A.mult)
                nc.vector.tensor_tensor(out=gw35[:], in0=gw35[:],
                                        in1=sr[:, :, 9:44], op=A.mult)
                nc.scalar.activation(out=gw35[:], in_=gw35[:], func=AF.Exp,
                                     scale=-6.0)
                nc.vector.tensor_tensor(out=gw35[:], in0=gw35[:],
                                        in1=sr[:, :, 79:114], op=A.mult)
                gw = work.tile([P, ACH, NREF], F32, tag="s2_gw")
                g5 = gw35[:].rearrange("p c (a n) -> p c a n", n=NC)
                nc.vector.tensor_tensor(out=gw[:], in0=g5[:, :, :, 0],
                                        in1=g5[:, :, :, 1], op=A.add)
                for n5 in range(2, NC):
                    nc.vector.tensor_tensor(out=gw[:], in0=gw[:],
                                            in1=g5[:, :, :, n5], op=A.add)
                nrm = work.tile([P, ACH], F32, tag="s2_nrm")
                nc.vector.tensor_reduce(out=nrm[:], in_=gw[:],
                                        axis=mybir.AxisListType.X, op=A.add)
                nc.vector.tensor_scalar(out=nrm[:], in0=nrm[:], scalar1=1e-7,
                                        scalar2=None, op0=A.max)
                nc.vector.reciprocal(out=nrm[:], in_=nrm[:])
                nc.vector.tensor_tensor(out=gw[:], in0=gw[:], in1=_bc(nrm[:], NREF),
                                        op=A.mult)
                # zeta
                chg_t = work.tile([P, ACH], F32, tag="s2_chg")
                nc.sync.dma_start(out=chg_t[:], in_=chg_d[:, k * ACH:(k + 1) * ACH])
                qmod2 = work.tile([P, ACH], F32, tag="s2_qm")
                nc.vector.tensor_tensor(out=qmod2[:], in0=chg_t[:],
                                        in1=sr[:, :, 1], op=A.add)
                msk2 = work.tile([P, ACH], F32, tag="s2_msk")
                nc.vector.tensor_scalar(out=msk2[:], in0=qmod2[:], scalar1=1e-8,
                                        scalar2=None, op0=A.is_gt)
                qs2 = work.tile([P, ACH], F32, tag="s2_qs")
                nc.vector.tensor_scalar(out=qs2[:], in0=qmod2[:], scalar1=1.0,
                                        scalar2=None, op0=A.subtract)
                nc.vector.tensor_tensor(out=qs2[:], in0=qs2[:], in1=msk2[:],
                                        op=A.mult)
                nc.vector.tensor_scalar(out=qs2[:], in0=qs2[:], scalar1=1.0,
                                        scalar2=None, op0=A.add)
                nc.vector.reciprocal(out=qs2[:], in_=qs2[:])
                zt = work.tile([P, ACH, NREF], F32, tag="s2_zt")
                nc.vector.tensor_scalar(out=zt[:], in0=sr[:, :, 2:9],
                                        scalar1=spq, scalar2=None,
                                        op0=A.mult)
                nc.vector.tensor_tensor(out=zt[:], in0=zt[:],
                                        in1=_bc(sr[:, :, 1], NREF), op=A.add)
                nc.vector.tensor_tensor(out=zt[:], in0=zt[:],
                                        in1=_bc(qs2[:], NREF), op=A.mult)
                nc.vector.tensor_tensor(out=zt[:], in0=zt[:],
                                        in1=_bc(sr[:, :, 0], NREF), op=A.mult)
                nc.vector.tensor_tensor(out=zt[:], in0=_bc(sr[:, :, 0], NREF),
                                        in1=zt[:], op=A.subtract)
                nc.scalar.activation(out=zt[:], in_=zt[:], func=AF.Exp, scale=2.0)
                nc.scalar.activation(out=zt[:], in_=zt[:], func=AF.Exp,
                                     scale=-3.0, bias=b3_p[:, 0:1])
                zeta = work.tile([P, ACH, NREF], F32, tag="s2_zeta")
                mb = bass.AP(tensor=msk2[:].tensor, offset=msk2[:].offset,
                             ap=[*msk2[:].ap, [0, NREF]])
                nc.vector.tensor_scalar(out=zeta[:], in0=zt[:], scalar1=E3,
                                        scalar2=None, op0=A.subtract)
                nc.vector.tensor_tensor(out=zeta[:], in0=zeta[:], in1=mb,
                                        op=A.mult)
                nc.vector.tensor_scalar(out=zeta[:], in0=zeta[:], scalar1=E3,
                                        scalar2=None, op0=A.add)
                nc.vector.tensor_tensor(out=zeta[:], in0=zeta[:], in1=gw[:],
                                        op=A.mult)
                # A~_i[w] = sum_a zeta[a]*atil[a,w]; replicate zeta over w
                # and cast atil to bf16 on the Activation engine so the DVE
                # mult/add chain runs in the packed-bf16 2x mode
                zrep = work.tile([P, ACH, NREF, NW], BF16, tag="s2_zrep")
                nc.scalar.activation(out=zrep[:], in_=_bc(zeta[:], NW),
                                     func=AF.Copy)
                av = srb[:, :, 0:161].rearrange("p c (a w) -> p c a w", w=NW)
                t2row = work.tile([P, ACH, NW], BF16, tag="s2_t2row")
                for a_ in range(NREF):
                    if a_ == 0:
                        nc.vector.tensor_tensor(
                            out=t2row[:], in0=av[:, :, 0, :],
                            in1=zrep[:, :, 0, :], op=A.mult)
                    else:
                        tmp_ = work.tile([P, ACH, NW], BF16, tag="s2_tmp")
                        nc.vector.tensor_tensor(
                            out=tmp_[:], in0=av[:, :, a_, :],
                            in1=zrep[:, :, a_, :], op=A.mult)
                        nc.vector.tensor_tensor(out=t2row[:], in0=t2row[:],
                                                in1=tmp_[:], op=A.add)
                nc.sync.dma_start(
                    out=t2sb_d.rearrange("(a p) w -> p a w", p=P)[
                        :, k * ACH:(k + 1) * ACH, :],
                    in_=t2row[:])

            # ---------- driver: pass A round-robin with interleaved s2 ----
            cpb = [meta["NGBS"][b] * GS // CHSLOTS for b in range(NBUCK)]
            coff = [0]
            for b in range(NBUCK):
                coff.append(coff[-1] + cpb[b])
            order = []
            for cc in range(max(cpb)):
                for b in range(NBUCK):
                    if cc < cpb[b]:
                        order.append(coff[b] + cc)
            lo_done_pos = max(
                pos_ for pos_, c in enumerate(order)
                if any(s[0] == 0 for s in meta["asub"][c]))
            for pos_, c in enumerate(order):
                emit_passA(c, pos_)
                if pos_ == lo_done_pos:
                    for k_ in range(SECB // P // ACH):
                        emit_s2(k_)
            for k_ in range(SECB // P // ACH, NACH):
                emit_s2(k_)

            _s2cm.__exit__(None, None, None)
            _wcm.__exit__(None, None, None)
            _srowcm.__exit__(None, None, None)

            # ---------- P3: AllGather packed bf16 A~ rows into strided table --
            # (emitted before pass B1 in program order so the Pool engine
            # starts the collective while the DVE computes damping factors)
            nc.gpsimd.collective_compute(
                "AllGather", A.bypass,
                replica_groups=[list(range(NCORES))],
                ins=[t2sb_d[:]], outs=[t2f_d[:, 0:NW]])

            _wcm = tc.tile_pool(name="pB", bufs=2)
            work = _wcm.__enter__()

            # ---------- P4a: pass B1 — damping factors (overlaps AllGather) --
            dbts = []
            for c in range(NCH):
                sb = work.tile([P, 3, TCH], F32, tag="b_sb")
                nc.sync.dma_start(
                    out=sb[:], in_=pb_d[c].rearrange("v (t p) -> p v t", p=P))
                r_t, si_t, sj_t = sb[:, 0, :], sb[:, 1, :], sb[:, 2, :]
                r2 = work.tile([P, TCH], F32, tag="b_r2")
                nc.scalar.activation(out=r2[:], in_=r_t, func=AF.Square,
                                     scale=1.0 / BOHR)
                r4 = work.tile([P, TCH], F32, tag="b_r4")
                nc.scalar.activation(out=r4[:], in_=r2[:], func=AF.Square)
                r8 = work.tile([P, TCH], F32, tag="b_r8")
                nc.scalar.activation(out=r8[:], in_=r4[:], func=AF.Square)
                r6 = work.tile([P, TCH], F32, tag="b_r6")
                nc.vector.tensor_tensor(out=r6[:], in0=r4[:], in1=r2[:], op=A.mult)
                R3 = work.tile([P, TCH], F32, tag="b_R3")
                nc.vector.scalar_tensor_tensor(out=R3[:], in0=si_t, scalar=3.0,
                                               in1=sj_t, op0=A.mult,
                                               op1=A.mult)
                r0 = work.tile([P, TCH], F32, tag="b_r0")
                nc.scalar.activation(out=r0[:], in_=R3[:], func=AF.Sqrt)
                nc.vector.tensor_scalar(out=r0[:], in0=r0[:], scalar1=a1p,
                                        scalar2=a2p, op0=A.mult, op1=A.add)
                q2 = work.tile([P, TCH], F32, tag="b_q2")
                nc.scalar.activation(out=q2[:], in_=r0[:], func=AF.Square)
                c4 = work.tile([P, TCH], F32, tag="b_c4")
                nc.scalar.activation(out=c4[:], in_=q2[:], func=AF.Square)
                c3 = work.tile([P, TCH], F32, tag="b_c3")
                nc.vector.tensor_tensor(out=c3[:], in0=c4[:], in1=q2[:], op=A.mult)
                c8 = work.tile([P, TCH], F32, tag="b_c8")
                nc.scalar.activation(out=c8[:], in_=c4[:], func=AF.Square)
                d6 = work.tile([P, TCH], F32, tag="b_d6")
                nc.vector.tensor_tensor(out=d6[:], in0=r6[:], in1=c3[:], op=A.add)
                nc.vector.reciprocal(out=d6[:], in_=d6[:])
                d8 = work.tile([P, TCH], F32, tag="b_d8")
                nc.vector.tensor_tensor(out=d8[:], in0=r8[:], in1=c8[:], op=A.add)
                nc.vector.reciprocal(out=d8[:], in_=d8[:])
                nc.vector.tensor_tensor(out=d8[:], in0=d8[:], in1=R3[:], op=A.mult)
                nc.vector.tensor_scalar(out=d8[:], in0=d8[:], scalar1=s8p,
                                        scalar2=None, op0=A.mult)
                d6c = const.tile([P, TCH], F32, tag=f"b_d6_{c}")
                nc.vector.scalar_tensor_tensor(out=d6c[:], in0=d6[:], scalar=s6p,
                                               in1=d8[:], op0=A.mult, op1=A.add)
                dbts.append(d6c)

            # ---------- P4b: pass B2 (gather + scale + reduce + scatter) -----
            for c in range(NCH):
                b = ch_bucket[c]
                sidx_t = ldsidx(c, work, "b_sidx")
                jw_t = jwt2[c % 2]
                nc.sync.dma_start(out=jw_t[0:16, :], in_=jw_d[c])
                gt = work.tile([P, TCH, NW], BF16, tag="b_g")
                _dma_gather_raw(
                    nc, gt[:],
                    t2f_d[BBASE[b]:BBASE[b] + BSIZE[b], 0:NW],
                    jw_t[:], CALL, NW, P)
                d6c = dbts[c]
                # replicate D over the 23 w-columns on the Activation engine
                # (bf16 cast + broadcast), keeping the DVE mult in 2x mode
                db = bass.AP(tensor=d6c[:].tensor, offset=d6c[:].offset,
                             ap=[*d6c[:].ap, [0, NW]])
                drep = work.tile([P, TCH, NW], BF16, tag="b_drep")
                nc.scalar.activation(out=drep[:], in_=db, func=AF.Copy)
                nc.vector.tensor_tensor(out=gt[:], in0=gt[:], in1=drep[:],
                                        op=A.mult)
                # tree reduce over GS slots
                m1 = work.tile([P, TCH // 2, NW], BF16, tag="b_m1")
                v = gt[:].rearrange("p (a two) f -> p a two f", two=2)
                nc.vector.tensor_tensor(out=m1[:], in0=v[:, :, 0, :],
                                        in1=v[:, :, 1, :], op=A.add)
                m2 = work.tile([P, TPG, NW], BF16, tag="b_m2")
                v = m1[:].rearrange("p (a two) f -> p a two f", two=2)
                nc.vector.tensor_tensor(out=m2[:], in0=v[:, :, 0, :],
                                        in1=v[:, :, 1, :], op=A.add)
                # scatter-add group rows into per-atom B table
                scatter_add(btab_d[:, 0:NW], m2[:], sidx_t[:], GCH, NW, 2 * BTW)

            _wcm.__exit__(None, None, None)
            _wcm = tc.tile_pool(name="pE", bufs=2)
            work = _wcm.__enter__()

            # ---------- P5: assemble E ----------
            for k in range(NACH):
                bsum = work.tile([P, ACH, NW], BF16, tag="e_bsum")
                nc.sync.dma_start(
                    out=bsum[:],
                    in_=btab_d[0:NA].rearrange("(a p) f -> p a f", p=P)[
                        :, k * ACH:(k + 1) * ACH, 0:NW])
                ai = work.tile([P, ACH, NW], BF16, tag="e_ai")
                nc.sync.dma_start(
                    out=ai[:],
                    in_=t2sb_d.rearrange("(a p) w -> p a w", p=P)[
                        :, k * ACH:(k + 1) * ACH, :])
                prod = work.tile([P, ACH, NW], F32, tag="e_prod")
                nc.vector.tensor_tensor(out=prod[:], in0=ai[:],
                                        in1=bsum[:], op=A.mult)
                ev = work.tile([P, ACH], F32, tag="e_ev")
                nc.vector.tensor_reduce(out=ev[:], in_=prod[:],
                                        axis=mybir.AxisListType.X, op=A.add)
                nc.vector.tensor_scalar(out=ev[:], in0=ev[:],
                                        scalar1=-0.5 * HARTREE, scalar2=None,
                                        op0=A.mult)
                nc.sync.dma_start(
                    out=e_d.rearrange("(a p) -> p a", p=P)[:, k * ACH:(k + 1) * ACH],
                    in_=ev[:])
            _wcm.__exit__(None, None, None)
    return nc


_PROG_CACHE = {}


def kernel(**inputs):
    species = np.asarray(inputs["species"])
    per_core, meta = preprocess(species, inputs["edge_index"],
                                inputs["lengths"], inputs["partial_charges"])
    rcov = np.asarray(inputs["rcov"], np.float32)
    en = np.asarray(inputs["en"], np.float32)
    sr4 = np.asarray(inputs["sqrt_r4r2"], np.float32)
    refsys = np.asarray(inputs["refsys"]).astype(np.int64)

    # refsys-expanded tables (pure host-side permutation of inputs)
    zeff = np.asarray(inputs["zeff"], np.float32)
    sscale = np.asarray(inputs["sscale"], np.float32)
    gam = np.asarray(inputs["gam"], np.float32)
    secaiw = np.asarray(inputs["secaiw"], np.float32)
    zeff_r = zeff[refsys]
    sscale_r = sscale[refsys]
    gam_r = gam[refsys]
    secaiw_r = secaiw[refsys].reshape(Z, NREF * NW)

    import os as _os
    _bedrock = _os.environ.get("BEDROCK") == "1"
    if not _bedrock:
        key = (tuple(meta["NGBS"]),
               tuple(tuple(s) for ss in meta["asub"] for s in ss))
        if key not in _PROG_CACHE:
            nc = build_program(meta)
            nc.finalize()
            _PROG_CACHE[key] = nc
        nc = _PROG_CACHE[key]

    shared = dict(
        zeff_r=zeff_r, sscale_r=sscale_r, gam_r=gam_r, secaiw_r=secaiw_r,
        refh=np.asarray(inputs["refh"], np.float32),
        ascale=np.asarray(inputs["ascale"], np.float32),
        hcount=np.asarray(inputs["hcount"], np.float32),
        refq=np.asarray(inputs["refq"], np.float32),
        alphaiw=np.asarray(inputs["alphaiw"], np.float32).reshape(Z, NREF * NW),
        gam=gam, zeff=zeff, sqrt_r4r2=sr4,
        ncount_weight=np.asarray(inputs["ncount_weight"], np.float32).reshape(Z, -1),
        cn=np.asarray(inputs["cn"], np.float32).reshape(Z, -1),
        ncount_mask=np.asarray(inputs["ncount_mask"], np.float32).reshape(Z, -1),
        cpw=np.asarray(inputs["cpw"], np.float32),
        s6_raw=np.asarray(inputs["s6_raw"], np.float32),
        s8_raw=np.asarray(inputs["s8_raw"], np.float32),
        a1_raw=np.asarray(inputs["a1_raw"], np.float32),
        a2_raw=np.asarray(inputs["a2_raw"], np.float32),
        scale_q_raw=np.asarray(inputs["scale_q_raw"], np.float32),
    )
    in_maps = []
    for c in range(NCORES):
        ci = build_core_inputs(per_core[c], meta, rcov, en, sr4)
        m = dict(shared)
        m.update(
            pa_pack=ci["pa_pack"], pb_pack=ci["pb_pack"],
            jw=ci["jw"].reshape(meta["NCH"], 16, CALL // 16),
            spw=ci["spw"].reshape(NACH, 16, (ACH * P) // 16),
            sidx=ci["sidx"].reshape(meta["NCH"], 16, GCH // 16),
            aidx=ci["aidx"].reshape(meta["NASC"], 16, GCH // 16),
            chg=ci["chg"],
        )
        in_maps.append(m)

    if _bedrock:
        outs = _sim_fallback(build_program(meta), in_maps)
    else:
        try:
            from concourse.bass_utils import run_bass_kernel_spmd
            res = run_bass_kernel_spmd(nc, in_maps, list(range(NCORES)))
            outs = [res.results[c]["e_out"] for c in range(NCORES)]
        except Exception:
            outs = _sim_fallback(build_program(meta), in_maps)
    e = np.concatenate(outs)
    return e[: species.shape[0]].astype(np.float32)


def _sim_fallback(nc, in_maps):
    import inspect
    import textwrap
    from scipy.special import erf as _scipy_erf
    from concourse import bass_interp
    src = textwrap.dedent(inspect.getsource(
        bass_interp.InstructionExecutor.visit_InstActivation))
    if "_scipy_erf" not in src:
        pat = ("    else:\n"
               "        # NOTE: If you are adding a new activation instruction")
        rep = ("    elif instruction.func == mb.ActivationFunctionType.Erf:\n"
               "        acted = _scipy_erf(scaled_and_biased)\n"
               "    else:\n"
               "        # NOTE: If you are adding a new activation instruction")
        assert pat in src
        src = src.replace(pat, rep)
        ns = dict(bass_interp.__dict__)
        ns["_scipy_erf"] = _scipy_erf
        exec(compile(src, "<erfpatch>", "exec"), ns)
        bass_interp.InstructionExecutor.visit_InstActivation = ns[
            "visit_InstActivation"]
    sim = bass_interp.MultiCoreSim(nc, NCORES, num_workers=1)
    for c in range(NCORES):
        for k, v in in_maps[c].items():
            sim.cores[c].tensor(k)[:] = v
    sim.simulate()
    global LAST_EXEC_TIME_NS
    LAST_EXEC_TIME_NS = int(getattr(sim, "global_time", 0))
    return [np.array(sim.cores[c].tensor("e_out")) for c in range(NCORES)]


LAST_EXEC_TIME_NS = None


# revision 51
# speedup vs baseline: 1.1501x; 1.0367x over previous
"""D4 dispersion energy kernel for 8 Trainium2 NeuronCores.

Strategy (v2):
- Host (numpy, integer/permutation work only): sort the edge list by (dst
  atom, j-range bucket), pad each (atom,bucket) edge run to a multiple of 8
  ("groups"), lay slots out in a fixed chunk/call/partition grid, and
  pre-permute all per-edge input data into that slot order.
- Device (all float math):
  * pass A computes per-edge coordination-number contributions and
    tree-reduces them into group sums, then dma_scatter_add's the group sums
    directly into a dense per-atom ncoord table;
  * stage 2 computes per-atom Gaussian weights / zeta / effective alpha
    table A~ from this core's atom slice (bf16, packed 23 floats/atom);
  * one AllGather shares the packed bf16 A~ rows into a 256B-stride table;
  * pass B gathers A~ rows for edge sources via dma_gather (bf16, 46B
    payload), applies Becke-Johnson damping, tree-reduces into group rows and
    dma_scatter_add's them into a dense per-atom B table;
  * E_i = -0.5*HARTREE * <A~_i, B_i>.
"""
import math
import numpy as np

import concourse.bass as bass
import concourse.bacc as bacc
import concourse.tile as tile
from concourse import mybir
from concourse.library_config import mlp as mlp_library

F32 = mybir.dt.float32
BF16 = mybir.dt.bfloat16
I16 = mybir.dt.int16

Z = 87
NREF = 7
NC = 5
NW = 23
BOHR = 0.5291772105638411
HARTREE = 27.211386024367243
K4, K5, K6, KK = 4.10451, 19.08857, 254.5553148552, 7.5
E3 = float(np.exp(3.0))
CPFAC = 3.0 / (2.0 * np.pi)

NCORES = 8
P = 128
ACOLS = 80              # atom columns per partition -> NA = 128*80
NA = P * ACOLS          # atoms per core (10240)
NPAD = NCORES * NA      # padded atom count (81920)
ACH = 16                # atom columns per stage-2 chunk (2048 atoms)
NACH = ACOLS // ACH     # atom chunks per core (5)
CALL = 32768            # idxs per dma_gather call (one per chunk)
TCH = 256               # slots per partition per compute chunk
GS = 4                  # slots per group
CHSLOTS = P * TCH       # slots per compute chunk (32768)
GCH = CHSLOTS // GS     # groups per chunk (4096)
TPG = TCH // GS         # group cells per partition per chunk (32)

# j-range buckets (dma_gather idx is int16)
NBUCK = 3
BBASE = [0, 27307, 54614]
BSIZE = [27307, 27307, NPAD - 54614]

SECB = 6144             # ncoord lo/hi section boundary (atoms, 3 s2 chunks)
SROWW = 320             # per-species row width (f32); 1280 B, 256-aligned
BTW = 64                # per-atom table row width (f32); 256 B stride


def _wrap16(idx_lin):
    """int linear idx list -> [16, ceil(n/16)] int16 wrapped tile.

    The gather/scatter ucode reads indices from the first 16 partitions
    only, so the upload carries just those rows (the SBUF tile is still
    128 partitions tall; rows 16-127 are never read)."""
    n = len(idx_lin)
    m = (n + 15) // 16
    pad = np.zeros(m * 16, np.int16)
    pad[:n] = idx_lin.astype(np.int16)
    return np.ascontiguousarray(pad.reshape(m, 16).T)  # [16, m]


def preprocess(species, edge_index, lengths, partial_charges):
    """Build per-core host-side data. Returns (per_core list of dicts, meta)."""
    n_at = species.shape[0]
    species = np.asarray(species).astype(np.int32)
    idx_i = np.asarray(edge_index[0]).astype(np.int64)
    idx_j = np.asarray(edge_index[1]).astype(np.int64)
    lengths = np.asarray(lengths).astype(np.float32)
    charges = np.asarray(partial_charges).astype(np.float32)

    spec_pad = np.zeros(NPAD, np.int32)
    spec_pad[:n_at] = species
    chg_pad = np.zeros(NPAD, np.float32)
    chg_pad[:n_at] = charges

    # bucket of each edge by j range
    jb = np.searchsorted(np.array(BBASE[1:]), idx_j, side="right")  # 0..2
    key = idx_i * NBUCK + jb
    order = np.argsort(key, kind="stable")
    si = idx_i[order]
    sj = idx_j[order]
    sl = lengths[order]
    sjb = jb[order]

    # count edges per (atom, bucket)
    cnt = np.bincount(idx_i * NBUCK + jb, minlength=NPAD * NBUCK).reshape(NPAD, NBUCK)
    grp = (cnt + GS - 1) // GS  # groups per (atom,bucket)
    # CSR offsets into sorted edge array for (atom,bucket)
    flat_cnt = cnt.reshape(-1)
    edge_off = np.zeros(NPAD * NBUCK + 1, np.int64)
    np.cumsum(flat_cnt, out=edge_off[1:])

    # group quota per bucket (max over cores, rounded to chunk multiple)
    grp_cb = grp.reshape(NCORES, NA, NBUCK).sum(axis=1)  # [core, bucket]
    NGBS = []
    for b in range(NBUCK):
        m = int(grp_cb[:, b].max())
        NGBS.append(((m + GCH - 1) // GCH) * GCH)
    NG = sum(NGBS)                       # groups per core
    SLOTS = NG * GS                      # slots per core
    NCH = SLOTS // CHSLOTS               # compute chunks
    assert SLOTS % CHSLOTS == 0
    # chunk -> bucket map (buckets are whole chunks)
    ch_bucket = []
    for b in range(NBUCK):
        ch_bucket += [b] * (NGBS[b] * GS // CHSLOTS)
    gb_off = np.concatenate([[0], np.cumsum(NGBS)])  # group offset per bucket

    meta = dict(NGBS=NGBS, NG=NG, SLOTS=SLOTS, NCH=NCH, ch_bucket=ch_bucket)

    per_core = []
    for c in range(NCORES):
        a0 = c * NA
        g_c = grp[a0 : a0 + NA]                 # [NA, NBUCK]
        gofs = np.zeros((NA + 1, NBUCK), np.int64)
        np.cumsum(g_c, axis=0, out=gofs[1:])
        ng_b = gofs[NA]                          # real groups per bucket
        for b in range(NBUCK):
            assert ng_b[b] <= NGBS[b]

        # atom id of each core-local group (bucket-sectioned, then padded)
        atom_of_G = np.full(NG, -1, np.int32)   # pads -> -1 (trash rows)
        for b in range(NBUCK):
            rep = np.repeat(np.arange(NA, dtype=np.int32), g_c[:, b])
            atom_of_G[gb_off[b] : gb_off[b] + len(rep)] = rep

        # slot position for each real edge:
        atom_l = si - a0
        core_mask = (atom_l >= 0) & (atom_l < NA)
        e_sel = np.nonzero(core_mask)[0]
        al = atom_l[e_sel]
        eb = sjb[e_sel]
        flat_id = (si[e_sel] * NBUCK + eb)
        rank = (e_sel - edge_off[flat_id])
        grank = rank // GS
        lane = rank % GS
        G = gb_off[eb] + gofs[al, eb] + grank    # core-local group id
        # group cell mapping: scatter token index == group rank within the
        # chunk (atom-monotone), so token prefixes map to atom ranges.
        # token t -> (partition t%128, cell t//128); slots of a group are GS
        # consecutive columns of one partition.
        c_ch = G // GCH
        pp = G % 128
        tg = (G % GCH) // 128
        pos = c_ch * CHSLOTS + (tg * GS + lane) * P + pp

        # per-slot streams (defaults for pad slots)
        r_s = np.full(SLOTS, 1.0e4, np.float32)
        rcj_s = np.ones(SLOTS, np.float32)
        enj_s = np.ones(SLOTS, np.float32)
        rci_s = np.ones(SLOTS, np.float32)
        eni_s = np.ones(SLOTS, np.float32)
        si_s = np.ones(SLOTS, np.float32)
        sj_s = np.ones(SLOTS, np.float32)
        jl_s = np.zeros(SLOTS, np.int32)

        r_s[pos] = sl[e_sel]
        jl_s[pos] = sj[e_sel] - np.array(BBASE, np.int64)[eb]

        # pass-B scatter idx per chunk: token t == group rank in chunk
        sidx = np.zeros((NCH, 16, GCH // 16), np.int16)
        for ch in range(NCH):
            av = atom_of_G[ch * GCH : (ch + 1) * GCH].copy()
            av[av < 0] = 0   # pad groups sum to ~0; row 0 is harmless
            sidx[ch] = _wrap16(av)



        per_core.append(dict(
            pos=pos, e_sel=e_sel, sj=sj[e_sel], sp_i=spec_pad[si[e_sel]],
            sp_j=spec_pad[sj[e_sel]], atom_of_G=atom_of_G,
            r_s=r_s, rcj_s=rcj_s, enj_s=enj_s, rci_s=rci_s, eni_s=eni_s,
            si_s=si_s, sj_s=sj_s, jl_s=jl_s, sidx=sidx,
            spec_slice=spec_pad[a0 : a0 + NA], chg_slice=chg_pad[a0 : a0 + NA],
        ))
    # pass-A sub-scatter structure: split ncoord into lo [0,SECB) / hi
    # [SECB,NA) tables so stage 2 can start before pass A finishes. Token
    # ranges are uniform across cores (SPMD); out-of-section tokens in the
    # overlap zone hit a trash row.
    asub = []      # per slot-chunk: list of (section, tok0, ntok, rowid)
    nrow = 0
    for ch in range(NCH):
        t1s, t0s = [], []
        for pc in per_core:
            av = pc["atom_of_G"][ch * GCH : (ch + 1) * GCH]
            lo = (av >= 0) & (av < SECB)
            hi = av >= SECB
            if lo.any():
                t1s.append(int(np.nonzero(lo)[0][-1]) + 1)
            if hi.any():
                t0s.append(int(np.nonzero(hi)[0][0]))
        subs = []
        if t1s:
            t1 = min((max(t1s) + 127) // 128 * 128, GCH)
            subs.append((0, 0, t1, nrow)); nrow += 1
        if t0s:
            t0 = min(t0s) // 128 * 128
            subs.append((1, t0, GCH - t0, nrow)); nrow += 1
        asub.append(subs)
    meta["asub"] = asub
    meta["NASC"] = nrow
    for pc in per_core:
        aidx = np.zeros((nrow, 16, GCH // 16), np.int16)
        for ch in range(NCH):
            av = pc["atom_of_G"][ch * GCH : (ch + 1) * GCH]
            for (sec, t0, ntok, row) in asub[ch]:
                iv = av[t0:t0 + ntok].copy()
                if sec == 0:
                    bad = ~((iv >= 0) & (iv < SECB))
                    iv[bad] = SECB
                else:
                    sel = iv >= SECB
                    iv = np.where(sel, iv - SECB, NA - SECB)
                aidx[row, :, :ntok // 16] = _wrap16(iv)
        pc["aidx"] = aidx
    return per_core, meta


def build_core_inputs(pc, meta, rcov, en, sqrt_r4r2):
    """Fill species-derived streams + wrapped idx arrays for one core."""
    SLOTS, NCH = meta["SLOTS"], meta["NCH"]
    pos = pc["pos"]
    pc["rcj_s"][pos] = rcov[pc["sp_j"]]
    pc["enj_s"][pos] = en[pc["sp_j"]]
    pc["rci_s"][pos] = rcov[pc["sp_i"]]
    pc["eni_s"][pos] = en[pc["sp_i"]]
    pc["si_s"][pos] = sqrt_r4r2[pc["sp_i"]]
    pc["sj_s"][pos] = sqrt_r4r2[pc["sp_j"]]

    # jidx16: one gather call per chunk, wrapped
    jl = pc["jl_s"]
    jw = np.zeros((NCH, 16, CALL // 16), np.int16)
    for k in range(NCH):
        jw[k] = _wrap16(jl[k * CALL : (k + 1) * CALL])

    # species wrap per atom chunk (2048 atoms): idx position u*128+p ->
    # local atom (16k+u)*128+p  (atom id = col*128 + p)
    spw = np.zeros((NACH, 16, (ACH * P) // 16), np.int16)
    spec = pc["spec_slice"].reshape(ACOLS, P)
    for k in range(NACH):
        lin = spec[k * ACH : (k + 1) * ACH, :].reshape(-1)  # [u, p] -> u*128+p
        spw[k] = _wrap16(lin)

    pa_pack = np.stack([
        pc["r_s"].reshape(NCH, CHSLOTS), pc["rcj_s"].reshape(NCH, CHSLOTS),
        pc["rci_s"].reshape(NCH, CHSLOTS)], axis=1)
    pa2_pack = np.stack([
        pc["enj_s"].reshape(NCH, CHSLOTS),
        pc["eni_s"].reshape(NCH, CHSLOTS)], axis=1)
    pb_pack = np.stack([
        pc["r_s"].reshape(NCH, CHSLOTS), pc["si_s"].reshape(NCH, CHSLOTS),
        pc["sj_s"].reshape(NCH, CHSLOTS)], axis=1)
    return dict(
        pa_pack=pa_pack, pa2_pack=pa2_pack, pb_pack=pb_pack,
        jw=jw.reshape(-1), spw=spw.reshape(-1), sidx=pc["sidx"].reshape(-1),
        aidx=pc["aidx"].reshape(-1),
        chg=np.ascontiguousarray(
            pc["chg_slice"].reshape(ACOLS, P).T).astype(np.float32),
    )


def _bc(ap, n):
    """Broadcast AP: append a step-0 inner dim of size n."""
    return bass.AP(tensor=ap.tensor, offset=ap.offset, ap=[*ap.ap, [0, n]])


def _dma_gather_raw(nc, out_ap, in_ap, idxs_ap, num_idxs, elem_size, elem_step):
    """dma_gather without the elem_size%256 restriction (payload < row pitch).
    Mirrors bass.BassGpSimd.dma_gather (non-transpose, DRAM source)."""
    eng = nc.gpsimd
    assert idxs_ap.dtype == mybir.dt.int16
    assert in_ap.dtype == out_ap.dtype
    stride_bytes = elem_step * mybir.dt.size(in_ap.dtype)
    assert stride_bytes % 256 == 0
    stride_bytes_256 = stride_bytes // 256
    assert in_ap.ap[0][0] == elem_step
    assert in_ap.ap[-1][1] == elem_size
    assert out_ap.ap[-1][1] == elem_size
    _in_ap = eng.lower_ap_dma(in_ap, for_custom_bir_dma=True)
    _idxs_ap = eng.lower_ap(idxs_ap)
    _out_ap = eng.lower_ap(out_ap)
    return eng.add_instruction(
        mybir.InstDMAGatherAnt(
            name=nc.get_next_instruction_name(),
            ins=[*_in_ap, _idxs_ap, eng.lower_val_access(eng.to_reg(num_idxs))],
            outs=[_out_ap],
            transpose=False,
            num_idxs=num_idxs,
            elem_size=elem_size,
            stride_bytes_256=stride_bytes_256,
            gen_mode=0,
            single_packet=True,
            queue_num=0,
            sbuf_tokens_per_rank=0,
            sbuf_free_dim_per_rank=0,
            sbuf_free_dim_pad_per_rank=0,
            sbuf_byte_offset=0,
        )
    )


def build_program(meta):
    SLOTS, NCH = meta["SLOTS"], meta["NCH"]
    ch_bucket = meta["ch_bucket"]
    A = mybir.AluOpType
    AF = mybir.ActivationFunctionType

    nc = bacc.Bacc(None, num_devices=NCORES, dynamic_dma_scratch_size=40960)

    def din(name, shape, dt=F32):
        return nc.dram_tensor(name, shape, dt, kind="ExternalInput")

    # per-slot streams, packed stream-major per chunk (one DMA per chunk)
    pa_d = din("pa_pack", [NCH, 3, CHSLOTS])
    pa2_d = din("pa2_pack", [NCH, 2, CHSLOTS], BF16)
    pb_d = din("pb_pack", [NCH, 3, CHSLOTS])
    jw_d = din("jw", [NCH, 16, CALL // 16], I16)
    spw_d = din("spw", [NACH, 16, (ACH * P) // 16], I16)
    sidx_d = din("sidx", [NCH, 16, GCH // 16], I16)
    aidx_d = din("aidx", [meta["NASC"], 16, GCH // 16], I16)
    chg_d = din("chg", [P, ACOLS])
    # tables
    zeffr_d = din("zeff_r", [Z, NREF]); sscr_d = din("sscale_r", [Z, NREF])
    gamr_d = din("gam_r", [Z, NREF]); refh_d = din("refh", [Z, NREF])
    asc_d = din("ascale", [Z, NREF]); hcnt_d = din("hcount", [Z, NREF])
    refq_d = din("refq", [Z, NREF])
    secr_d = din("secaiw_r", [Z, NREF * NW]); aiw_d = din("alphaiw", [Z, NREF * NW])
    gam_d = din("gam", [Z]); zeff_d = din("zeff", [Z]); sr4_d = din("sqrt_r4r2", [Z])
    cnw_d = din("ncount_weight", [Z, NREF * NC]); cnd_d = din("cn", [Z, NREF * NC])
    msk_d = din("ncount_mask", [Z, NREF * NC])
    cpw_d = din("cpw", [NW])
    s6_d = din("s6_raw", [1]); s8_d = din("s8_raw", [1])
    a1_d = din("a1_raw", [1]); a2_d = din("a2_raw", [1]); sq_d = din("scale_q_raw", [1])

    srowA_d = nc.dram_tensor("srowad", [Z, 128], F32)
    srowB_d = nc.dram_tensor("srowbd", [Z, 256], BF16)
    ncoL_d = nc.dram_tensor("ncold", [SECB + 16, BTW], F32)
    ncoH_d = nc.dram_tensor("ncohd", [NA - SECB + 16, BTW], F32)
    btab_d = nc.dram_tensor("btabd", [NA, 2 * BTW], BF16)
    t2sb_d = nc.dram_tensor("t2sb", [NA, NW], BF16)
    t2f_d = nc.dram_tensor("t2f", [NPAD, P], BF16, addr_space="Shared")
    e_d = nc.dram_tensor("e_out", [NA], F32, kind="ExternalOutput")

    def brc(dram, parts, width):
        """AP reading a [width] DRAM tensor broadcast across `parts` partitions."""
        return bass.AP(tensor=dram.tensor if hasattr(dram, "tensor") else dram,
                       offset=0, ap=[[0, parts], [1, width]])

    with tile.TileContext(nc) as tc:
        import contextlib
        with contextlib.ExitStack() as ctx:
            const = ctx.enter_context(tc.tile_pool(name="const", bufs=1))
            _srowcm = tc.tile_pool(name="srowp", bufs=1)
            srowp = _srowcm.__enter__()
            _wcm = tc.tile_pool(name="p0", bufs=2)
            work = _wcm.__enter__()

            nc.gpsimd.load_library(mlp_library)

            # dedicated index tiles: ucode reads only rows 0:16, so uploads
            # write just those rows; memset once here to satisfy init checks
            jwt_a = const.tile([P, CALL // 16], I16, tag="jwt0")
            jwt_b = const.tile([P, CALL // 16], I16, tag="jwt1")
            sxt_a = const.tile([P, GCH // 16], I16, tag="sxt0")
            sxt_b = const.tile([P, GCH // 16], I16, tag="sxt1")
            spwt = const.tile([P, (ACH * P) // 16], I16, tag="spwt")
            axt_a = const.tile([P, GCH // 16], I16, tag="axt0")
            axt_b = const.tile([P, GCH // 16], I16, tag="axt1")
            axt_c = const.tile([P, GCH // 16], I16, tag="axt2")
            axt_d = const.tile([P, GCH // 16], I16, tag="axt3")
            jwt2 = [jwt_a, jwt_b]
            sxt2 = [sxt_a, sxt_b]
            axt4 = [axt_a, axt_b, axt_c, axt_d]
            for t_ in (*jwt2, *sxt2, *axt4, spwt):
                nc.gpsimd.memset(t_[:], 0)

            b3_87 = const.tile([Z, 1], F32)
            nc.vector.memset(b3_87[:], 3.0)
            b3_p = const.tile([P, 1], F32)
            nc.vector.memset(b3_p[:], 3.0)
            bk5_p = const.tile([P, 1], F32)
            nc.vector.memset(bk5_p[:], K5)
            bkk_p = const.tile([P, 1], F32)
            nc.vector.memset(bkk_p[:], KK)

            # zero rows of nco/btab tables (only the columns we touch)
            zcol = const.tile([P, ACOLS, 1], F32)
            nc.vector.memset(zcol[:], 0.0)
            nc.sync.dma_start(
                out=ncoL_d[0:SECB].rearrange("(a p) f -> p a f", p=P)[:, :, 0:1],
                in_=zcol[:, 0:SECB // P, :])
            nc.sync.dma_start(
                out=ncoH_d[0:NA - SECB].rearrange("(a p) f -> p a f", p=P)[:, :, 0:1],
                in_=zcol[:, 0:(NA - SECB) // P, :])
            zrow23 = const.tile([P, ACOLS, NW], BF16)
            nc.vector.memset(zrow23[:], 0.0)

            # ---------- P0: per-species row table ----------
            def ld87(dram, w):
                t = const.tile([Z, w], F32, tag=f"ld_{dram.name}")
                nc.sync.dma_start(out=t[:], in_=dram[:] if w > 1 else dram[:, None])
                return t

            zeffr = ld87(zeffr_d, NREF); sscr = ld87(sscr_d, NREF)
            gamr = ld87(gamr_d, NREF); refh = ld87(refh_d, NREF)
            asc = ld87(asc_d, NREF); hcnt = ld87(hcnt_d, NREF)
            refq = ld87(refq_d, NREF)
            secr = ld87(secr_d, NREF * NW); aiw = ld87(aiw_d, NREF * NW)
            gam1 = ld87(gam_d, 1); zeff1 = ld87(zeff_d, 1); sr41 = ld87(sr4_d, 1)
            cnw = ld87(cnw_d, NREF * NC); cnt_ = ld87(cnd_d, NREF * NC)
            msk = ld87(msk_d, NREF * NC)

            # softplus of all 5 scalar params in one Exp->Ln block
            params = const.tile([P, 5], F32)
            for ii, dd in enumerate([s6_d, s8_d, a1_d, a2_d, sq_d]):
                nc.sync.dma_start(out=params[:, ii:ii+1], in_=brc(dd, P, 1))
            nc.scalar.activation(out=params[:], in_=params[:], func=AF.Exp)
            nc.vector.tensor_scalar(out=params[:], in0=params[:], scalar1=1.0,
                                    scalar2=None, op0=A.add)
            nc.scalar.activation(out=params[:], in_=params[:], func=AF.Ln)
            s6p, s8p = params[:, 0:1], params[:, 1:2]
            a1p, a2p = params[:, 2:3], params[:, 3:4]
            spq = params[:, 4:5]
            sq87 = params[0:Z, 4:5]

            qmod = work.tile([Z, NREF], F32, tag="p0a")
            nc.vector.tensor_scalar(out=qmod[:], in0=refh[:], scalar1=sq87,
                                    scalar2=None, op0=A.mult)
            nc.vector.tensor_tensor(out=qmod[:], in0=qmod[:], in1=zeffr[:], op=A.add)
            qmsk = work.tile([Z, NREF], F32, tag="p0b")
            nc.vector.tensor_scalar(out=qmsk[:], in0=qmod[:], scalar1=1e-8,
                                    scalar2=None, op0=A.is_gt)
            qsafe = work.tile([Z, NREF], F32, tag="p0c")
            nc.vector.tensor_scalar(out=qsafe[:], in0=qmod[:], scalar1=1.0,
                                    scalar2=None, op0=A.subtract)
            nc.vector.tensor_tensor(out=qsafe[:], in0=qsafe[:], in1=qmsk[:],
                                    op=A.mult)
            nc.vector.tensor_scalar(out=qsafe[:], in0=qsafe[:], scalar1=1.0,
                                    scalar2=None, op0=A.add)
            rq = work.tile([Z, NREF], F32, tag="p0d")
            nc.vector.reciprocal(out=rq[:], in_=qsafe[:])
            t0 = work.tile([Z, NREF], F32, tag="p0e")
            nc.vector.tensor_tensor(out=t0[:], in0=zeffr[:], in1=rq[:], op=A.mult)
            nc.vector.tensor_tensor(out=t0[:], in0=t0[:], in1=gamr[:], op=A.mult)
            nc.vector.tensor_tensor(out=t0[:], in0=gamr[:], in1=t0[:], op=A.subtract)
            nc.scalar.activation(out=t0[:], in_=t0[:], func=AF.Exp, scale=2.0)
            nc.scalar.activation(out=t0[:], in_=t0[:], func=AF.Exp, scale=-3.0,
                                 bias=b3_87[:, 0:1])
            zfac = work.tile([Z, NREF], F32, tag="p0f")
            nc.vector.tensor_scalar(out=zfac[:], in0=t0[:], scalar1=E3,
                                    scalar2=None, op0=A.subtract)
            nc.vector.tensor_tensor(out=zfac[:], in0=zfac[:], in1=qmsk[:],
                                    op=A.mult)
            nc.vector.tensor_scalar(out=zfac[:], in0=zfac[:], scalar1=E3,
                                    scalar2=None, op0=A.add)
            al = work.tile([Z, NREF, NW], F32, tag="p0g")
            nc.vector.tensor_tensor(
                out=al[:], in0=secr[:].rearrange("z (a w) -> z a w", w=NW),
                in1=_bc(sscr[:], NW), op=A.mult)
            nc.vector.tensor_tensor(out=al[:], in0=al[:], in1=_bc(zfac[:], NW),
                                    op=A.mult)
            nc.vector.tensor_tensor(out=al[:], in0=al[:], in1=_bc(hcnt[:], NW),
                                    op=A.mult)
            nc.vector.tensor_tensor(
                out=al[:], in0=aiw[:].rearrange("z (a w) -> z a w", w=NW),
                in1=al[:], op=A.subtract)
            nc.vector.tensor_tensor(out=al[:], in0=al[:], in1=_bc(asc[:], NW),
                                    op=A.mult)
            nc.vector.tensor_scalar(out=al[:], in0=al[:], scalar1=0.0,
                                    scalar2=None, op0=A.max)
            cpw87 = const.tile([Z, NW], F32)
            nc.sync.dma_start(out=cpw87[:], in_=brc(cpw_d, Z, NW))
            nc.scalar.activation(out=cpw87[:], in_=cpw87[:], func=AF.Sqrt,
                                 scale=CPFAC)
            wb = bass.AP(tensor=cpw87[:].tensor, offset=cpw87[:].offset,
                         ap=[cpw87[:].ap[0], [0, NREF], [1, NW]])
            nc.vector.tensor_tensor(out=al[:], in0=al[:], in1=wb, op=A.mult)

            # assemble split species rows: f32 part + bf16 atil part
            srow = const.tile([Z, 128], F32)
            nc.vector.memset(srow[:], 0.0)
            nc.vector.tensor_copy(out=srow[:, 0:1], in_=gam1[:])
            nc.vector.tensor_copy(out=srow[:, 1:2], in_=zeff1[:])
            nc.vector.tensor_copy(out=srow[:, 2:9], in_=refq[:])
            nc.vector.tensor_copy(out=srow[:, 9:44], in_=cnw[:])
            nc.vector.tensor_copy(out=srow[:, 44:79], in_=cnt_[:])
            nc.vector.tensor_copy(out=srow[:, 79:114], in_=msk[:])
            nc.sync.dma_start(out=srowA_d[:], in_=srow[:])
            srowb = const.tile([Z, 256], BF16)
            nc.gpsimd.memset(srowb[:], 0)
            nc.scalar.activation(out=srowb[:, 0:161],
                                 in_=al[:].rearrange("z a w -> z (a w)"),
                                 func=AF.Copy)
            nc.sync.dma_start(out=srowB_d[:], in_=srowb[:])


            _wcm.__exit__(None, None, None)
            _wcm = tc.tile_pool(name="pA", bufs=3)
            work = _wcm.__enter__()

            # ---------- P1: pass A (coordination numbers) ----------
            def ldsidx(c, pool, tag):
                t = sxt2[c % 2]
                nc.sync.dma_start(out=t[0:16, :], in_=sidx_d[c])
                return t

            def scatter_add(out_ap, in_ap, idxs_t, num, elem, step):
                return nc.gpsimd.dma_scatter_add(
                    out_ap, in_ap, idxs_t, num, num, elem, elem_step=step)

            srow_t = []
            asub = meta["asub"]

            def emit_passA(c, pos_):
                st = work.tile([P, 3, TCH], F32, tag="a_st")
                nc.sync.dma_start(
                    out=st[:], in_=pa_d[c].rearrange("v (t p) -> p v t", p=P))
                st2 = work.tile([P, 2, TCH], BF16, tag="a_st2")
                nc.sync.dma_start(
                    out=st2[:], in_=pa2_d[c].rearrange("v (t p) -> p v t", p=P))
                r_t, rcj, rci = st[:, 0, :], st[:, 1, :], st[:, 2, :]
                enj, eni = st2[:, 0, :], st2[:, 1, :]
                # interleave stage-2 srow prefetch gathers (needed only by
                # stage 2) so their DMA doesn't starve pass-A stream loads
                if (pos_ % 2 == 1 or pos_ == NCH - 1) and len(srow_t) < NACH:
                    k = len(srow_t)
                    spw_t = spwt
                    nc.sync.dma_start(out=spw_t[0:16, :], in_=spw_d[k])
                    sg = srowp.tile([P, ACH, 115], F32, tag=f"pf_srow{k}")
                    _dma_gather_raw(nc, sg[:], srowA_d[:, 0:115], spw_t[:],
                                    ACH * P, 115, 128)
                    sgb = srowp.tile([P, ACH, 161], BF16, tag=f"pf_srowb{k}")
                    _dma_gather_raw(nc, sgb[:], srowB_d[:, 0:161], spw_t[:],
                                    ACH * P, 161, 256)
                    srow_t.append((sg, sgb))
                # rcv = 4/3*(rci+rcj)
                rcv = work.tile([P, TCH], F32, tag="a_rcv")
                nc.vector.tensor_tensor(out=rcv[:], in0=rci, in1=rcj, op=A.add)
                nc.vector.tensor_scalar(out=rcv[:], in0=rcv[:], scalar1=4.0 / 3.0,
                                        scalar2=None, op0=A.mult)
                # den = K4*exp(-((|eni-enj|+K5)^2)/K6) via sigmoid identity:
                # exp(-v) = 1/sigmoid(v) - 1
                den = work.tile([P, TCH], F32, tag="a_den")
                nc.vector.tensor_tensor(out=den[:], in0=eni, in1=enj,
                                        op=A.subtract)
                nc.scalar.activation(out=den[:], in_=den[:], func=AF.Abs)
                nc.scalar.activation(out=den[:], in_=den[:], func=AF.Square,
                                     bias=bk5_p[:, 0:1])
                nc.scalar.activation(out=den[:], in_=den[:], func=AF.Sigmoid,
                                     scale=1.0 / K6)
                nc.vector.reciprocal(out=den[:], in_=den[:])
                nc.vector.tensor_scalar(out=den[:], in0=den[:], scalar1=1.0,
                                        scalar2=0.5 * K4, op0=A.subtract,
                                        op1=A.mult)
                # erf(-KK*(rr-rcv)/rcv) = Erf(-KK*u + KK), u = rr/rcv
                cf = work.tile([P, TCH], F32, tag="a_cf")
                nc.vector.reciprocal(out=cf[:], in_=rcv[:])
                nc.vector.tensor_tensor(out=cf[:], in0=cf[:], in1=r_t, op=A.mult)
                nc.scalar.activation(out=cf[:], in_=cf[:], func=AF.Erf,
                                     scale=-KK / BOHR, bias=bkk_p[:, 0:1])
                # countf = (erf + 1) * den_scaled
                nc.vector.scalar_tensor_tensor(out=cf[:], in0=cf[:], scalar=1.0,
                                               in1=den[:], op0=A.add, op1=A.mult)
                # tree reduce GS -> 1
                l1 = work.tile([P, TCH // 2], F32, tag="a_l1")
                v = cf[:].rearrange("p (a two) -> p a two", two=2)
                nc.vector.tensor_tensor(out=l1[:], in0=v[:, :, 0], in1=v[:, :, 1],
                                        op=A.add)
                l2 = work.tile([P, TPG, 1], F32, tag="a_l2")
                v = l1[:].rearrange("p (a two) -> p a two", two=2)
                nc.vector.tensor_tensor(out=l2[:, :, 0],
                                        in0=v[:, :, 0], in1=v[:, :, 1],
                                        op=A.add)
                # scatter-add group sums into lo/hi per-atom ncoord tables
                for (sec, t0, ntok, row) in asub[c]:
                    it = axt4[(pos_ * 2 + sec) % 4]
                    nc.sync.dma_start(out=it[0:16, 0:ntok // 16],
                                      in_=aidx_d[row][:, 0:ntok // 16])
                    tab = ncoL_d if sec == 0 else ncoH_d
                    scatter_add(tab[:, 0:1],
                                l2[:, t0 // 128:(t0 + ntok) // 128, :],
                                it[:, 0:ntok // 16], ntok, 1, BTW)

            _s2cm = tc.tile_pool(name="pS2", bufs=2)
            works2 = _s2cm.__enter__()

            # ---------- P2: stage 2 (per-atom A~ rows) ----------
            def emit_s2(k):
                work = works2
                sr = srow_t[k][0][:]
                srb = srow_t[k][1][:]
                nco = work.tile([P, ACH, 1], F32, tag="s2_nco")
                if k * ACH < SECB // P:
                    nsrc = ncoL_d[0:SECB].rearrange("(a p) f -> p a f", p=P)[
                        :, k * ACH:(k + 1) * ACH, 0:1]
                else:
                    k2 = k - SECB // P // ACH
                    nsrc = ncoH_d[0:NA - SECB].rearrange(
                        "(a p) f -> p a f", p=P)[
                        :, k2 * ACH:(k2 + 1) * ACH, 0:1]
                nc.sync.dma_start(out=nco[:], in_=nsrc)

                # gaussian weights gw[P, ACH, NREF]
                gw35 = work.tile([P, ACH, NREF * NC], F32, tag="s2_gw35")
                nc.vector.tensor_tensor(out=gw35[:],
                                        in0=_bc(nco[:, :, 0], NREF * NC),
                                        in1=sr[:, :, 44:79], op=A.subtract)
                nc.vector.tensor_tensor(out=gw35[:], in0=gw35[:], in1=gw35[:],
                                        op=A.mult)
                nc.vector.tensor_tensor(out=gw35[:], in0=gw35[:],
                                        in1=sr[:, :, 9:44], op=A.mult)
                nc.scalar.activation(out=gw35[:], in_=gw35[:], func=AF.Exp,
                                     scale=-6.0)
                nc.vector.tensor_tensor(out=gw35[:], in0=gw35[:],
                                        in1=sr[:, :, 79:114], op=A.mult)
                gw = work.tile([P, ACH, NREF], F32, tag="s2_gw")
                g5 = gw35[:].rearrange("p c (a n) -> p c a n", n=NC)
                nc.vector.tensor_tensor(out=gw[:], in0=g5[:, :, :, 0],
                                        in1=g5[:, :, :, 1], op=A.add)
                for n5 in range(2, NC):
                    nc.vector.tensor_tensor(out=gw[:], in0=gw[:],
                                            in1=g5[:, :, :, n5], op=A.add)
                nrm = work.tile([P, ACH], F32, tag="s2_nrm")
                nc.vector.tensor_reduce(out=nrm[:], in_=gw[:],
                                        axis=mybir.AxisListType.X, op=A.add)
                nc.vector.tensor_scalar(out=nrm[:], in0=nrm[:], scalar1=1e-7,
                                        scalar2=None, op0=A.max)
                nc.vector.reciprocal(out=nrm[:], in_=nrm[:])
                nc.vector.tensor_tensor(out=gw[:], in0=gw[:], in1=_bc(nrm[:], NREF),
                                        op=A.mult)
                # zeta
                chg_t = work.tile([P, ACH], F32, tag="s2_chg")
                nc.sync.dma_start(out=chg_t[:], in_=chg_d[:, k * ACH:(k + 1) * ACH])
                qmod2 = work.tile([P, ACH], F32, tag="s2_qm")
                nc.vector.tensor_tensor(out=qmod2[:], in0=chg_t[:],
                                        in1=sr[:, :, 1], op=A.add)
                msk2 = work.tile([P, ACH], F32, tag="s2_msk")
                nc.vector.tensor_scalar(out=msk2[:], in0=qmod2[:], scalar1=1e-8,
                                        scalar2=None, op0=A.is_gt)
                qs2 = work.tile([P, ACH], F32, tag="s2_qs")
                nc.vector.tensor_scalar(out=qs2[:], in0=qmod2[:], scalar1=1.0,
                                        scalar2=None, op0=A.subtract)
                nc.vector.tensor_tensor(out=qs2[:], in0=qs2[:], in1=msk2[:],
                                        op=A.mult)
                nc.vector.tensor_scalar(out=qs2[:], in0=qs2[:], scalar1=1.0,
                                        scalar2=None, op0=A.add)
                nc.vector.reciprocal(out=qs2[:], in_=qs2[:])
                zt = work.tile([P, ACH, NREF], F32, tag="s2_zt")
                nc.vector.tensor_scalar(out=zt[:], in0=sr[:, :, 2:9],
                                        scalar1=spq, scalar2=None,
                                        op0=A.mult)
                nc.vector.tensor_tensor(out=zt[:], in0=zt[:],
                                        in1=_bc(sr[:, :, 1], NREF), op=A.add)
                nc.vector.tensor_tensor(out=zt[:], in0=zt[:],
                                        in1=_bc(qs2[:], NREF), op=A.mult)
                nc.vector.tensor_tensor(out=zt[:], in0=zt[:],
                                        in1=_bc(sr[:, :, 0], NREF), op=A.mult)
                nc.vector.tensor_tensor(out=zt[:], in0=_bc(sr[:, :, 0], NREF),
                                        in1=zt[:], op=A.subtract)
                nc.scalar.activation(out=zt[:], in_=zt[:], func=AF.Exp, scale=2.0)
                nc.scalar.activation(out=zt[:], in_=zt[:], func=AF.Exp,
                                     scale=-3.0, bias=b3_p[:, 0:1])
                zeta = work.tile([P, ACH, NREF], F32, tag="s2_zeta")
                mb = bass.AP(tensor=msk2[:].tensor, offset=msk2[:].offset,
                             ap=[*msk2[:].ap, [0, NREF]])
                nc.vector.tensor_scalar(out=zeta[:], in0=zt[:], scalar1=E3,
                                        scalar2=None, op0=A.subtract)
                nc.vector.tensor_tensor(out=zeta[:], in0=zeta[:], in1=mb,
                                        op=A.mult)
                nc.vector.tensor_scalar(out=zeta[:], in0=zeta[:], scalar1=E3,
                                        scalar2=None, op0=A.add)
                nc.vector.tensor_tensor(out=zeta[:], in0=zeta[:], in1=gw[:],
                                        op=A.mult)
                # A~_i[w] = sum_a zeta[a]*atil[a,w]; replicate zeta over w
                # and cast atil to bf16 on the Activation engine so the DVE
                # mult/add chain runs in the packed-bf16 2x mode
                zrep = work.tile([P, ACH, NREF, NW], BF16, tag="s2_zrep")
                nc.scalar.activation(out=zrep[:], in_=_bc(zeta[:], NW),
                                     func=AF.Copy)
                av = srb[:, :, 0:161].rearrange("p c (a w) -> p c a w", w=NW)
                t2row = work.tile([P, ACH, NW], BF16, tag="s2_t2row")
                for a_ in range(NREF):
                    if a_ == 0:
                        nc.vector.tensor_tensor(
                            out=t2row[:], in0=av[:, :, 0, :],
                            in1=zrep[:, :, 0, :], op=A.mult)
                    else:
                        tmp_ = work.tile([P, ACH, NW], BF16, tag="s2_tmp")
                        nc.vector.tensor_tensor(
                            out=tmp_[:], in0=av[:, :, a_, :],
                            in1=zrep[:, :, a_, :], op=A.mult)
                        nc.vector.tensor_tensor(out=t2row[:], in0=t2row[:],
                                                in1=tmp_[:], op=A.add)
                nc.sync.dma_start(
                    out=t2sb_d.rearrange("(a p) w -> p a w", p=P)[
                        :, k * ACH:(k + 1) * ACH, :],
                    in_=t2row[:])

            # ---------- driver: pass A round-robin with interleaved s2 ----
            cpb = [meta["NGBS"][b] * GS // CHSLOTS for b in range(NBUCK)]
            coff = [0]
            for b in range(NBUCK):
                coff.append(coff[-1] + cpb[b])
            order = []
            for cc in range(max(cpb)):
                for b in range(NBUCK):
                    if cc < cpb[b]:
                        order.append(coff[b] + cc)
            lo_done_pos = max(
                pos_ for pos_, c in enumerate(order)
                if any(s[0] == 0 for s in meta["asub"][c]))
            for pos_, c in enumerate(order):
                emit_passA(c, pos_)
                if pos_ == lo_done_pos:
                    for k_ in range(SECB // P // ACH):
                        emit_s2(k_)
            for k_ in range(SECB // P // ACH, NACH):
                emit_s2(k_)

            _s2cm.__exit__(None, None, None)
            _wcm.__exit__(None, None, None)
            _srowcm.__exit__(None, None, None)

            # zero the B table now — its only consumers are the pass-B
            # scatter-adds, so this write rides the idle DMA window here
            nc.sync.dma_start(
                out=btab_d[0:NA].rearrange("(a p) f -> p a f", p=P)[:, :, 0:NW],
                in_=zrow23[:])

            # ---------- P3: AllGather packed bf16 A~ rows into strided table --
            # (emitted before pass B1 in program order so the Pool engine
            # starts the collective while the DVE computes damping factors)
            nc.gpsimd.collective_compute(
                "AllGather", A.bypass,
                replica_groups=[list(range(NCORES))],
                ins=[t2sb_d[:]], outs=[t2f_d[:, 0:NW]])

            _wcm = tc.tile_pool(name="pB", bufs=2)
            work = _wcm.__enter__()

            # ---------- P4a: pass B1 — damping factors (overlaps AllGather) --
            dbts = []
            for c in range(NCH):
                sb = work.tile([P, 3, TCH], F32, tag="b_sb")
                nc.sync.dma_start(
                    out=sb[:], in_=pb_d[c].rearrange("v (t p) -> p v t", p=P))
                r_t, si_t, sj_t = sb[:, 0, :], sb[:, 1, :], sb[:, 2, :]
                r2 = work.tile([P, TCH], F32, tag="b_r2")
                nc.scalar.activation(out=r2[:], in_=r_t, func=AF.Square,
                                     scale=1.0 / BOHR)
                r4 = work.tile([P, TCH], F32, tag="b_r4")
                nc.scalar.activation(out=r4[:], in_=r2[:], func=AF.Square)
                r8 = work.tile([P, TCH], F32, tag="b_r8")
                nc.scalar.activation(out=r8[:], in_=r4[:], func=AF.Square)
                r6 = work.tile([P, TCH], F32, tag="b_r6")
                nc.vector.tensor_tensor(out=r6[:], in0=r4[:], in1=r2[:], op=A.mult)
                R3 = work.tile([P, TCH], F32, tag="b_R3")
                nc.vector.scalar_tensor_tensor(out=R3[:], in0=si_t, scalar=3.0,
                                               in1=sj_t, op0=A.mult,
                                               op1=A.mult)
                r0 = work.tile([P, TCH], F32, tag="b_r0")
                nc.scalar.activation(out=r0[:], in_=R3[:], func=AF.Sqrt)
                nc.vector.tensor_scalar(out=r0[:], in0=r0[:], scalar1=a1p,
                                        scalar2=a2p, op0=A.mult, op1=A.add)
                q2 = work.tile([P, TCH], F32, tag="b_q2")
                nc.scalar.activation(out=q2[:], in_=r0[:], func=AF.Square)
                c4 = work.tile([P, TCH], F32, tag="b_c4")
                nc.scalar.activation(out=c4[:], in_=q2[:], func=AF.Square)
                c3 = work.tile([P, TCH], F32, tag="b_c3")
                nc.vector.tensor_tensor(out=c3[:], in0=c4[:], in1=q2[:], op=A.mult)
                c8 = work.tile([P, TCH], F32, tag="b_c8")
                nc.scalar.activation(out=c8[:], in_=c4[:], func=AF.Square)
                d6 = work.tile([P, TCH], F32, tag="b_d6")
                nc.vector.tensor_tensor(out=d6[:], in0=r6[:], in1=c3[:], op=A.add)
                nc.vector.reciprocal(out=d6[:], in_=d6[:])
                d8 = work.tile([P, TCH], F32, tag="b_d8")
                nc.vector.tensor_tensor(out=d8[:], in0=r8[:], in1=c8[:], op=A.add)
                nc.vector.reciprocal(out=d8[:], in_=d8[:])
                nc.vector.tensor_tensor(out=d8[:], in0=d8[:], in1=R3[:], op=A.mult)
                nc.vector.tensor_scalar(out=d8[:], in0=d8[:], scalar1=s8p,
                                        scalar2=None, op0=A.mult)
                d6c = const.tile([P, TCH], F32, tag=f"b_d6_{c}")
                nc.vector.scalar_tensor_tensor(out=d6c[:], in0=d6[:], scalar=s6p,
                                               in1=d8[:], op0=A.mult, op1=A.add)
                dbts.append(d6c)

            # ---------- P4b: pass B2 (gather + scale + reduce + scatter) -----
            cpb = [meta["NGBS"][b] * GS // CHSLOTS for b in range(NBUCK)]
            coff = [0]
            for b in range(NBUCK):
                coff.append(coff[-1] + cpb[b])
            order = []
            for cc in range(max(cpb)):
                for b in range(NBUCK):
                    if cc < cpb[b]:
                        order.append(coff[b] + cc)
            lo_done_pos = max(
                pos_ for pos_, c in enumerate(order)
                if any(s[0] == 0 for s in meta["asub"][c]))

            def emit_passB2(c, pos_):
                b = ch_bucket[c]
                sidx_t = ldsidx(c, work, "b_sidx")
                jw_t = jwt2[pos_ % 2]
                nc.sync.dma_start(out=jw_t[0:16, :], in_=jw_d[c])
                gt = work.tile([P, TCH, NW], BF16, tag="b_g")
                _dma_gather_raw(
                    nc, gt[:],
                    t2f_d[BBASE[b]:BBASE[b] + BSIZE[b], 0:NW],
                    jw_t[:], CALL, NW, P)
                d6c = dbts[c]
                # replicate D over the 23 w-columns on the Activation engine
                # (bf16 cast + broadcast), keeping the DVE mult in 2x mode
                db = bass.AP(tensor=d6c[:].tensor, offset=d6c[:].offset,
                             ap=[*d6c[:].ap, [0, NW]])
                drep = work.tile([P, TCH, NW], BF16, tag="b_drep")
                nc.scalar.activation(out=drep[:], in_=db, func=AF.Copy)
                nc.vector.tensor_tensor(out=gt[:], in0=gt[:], in1=drep[:],
                                        op=A.mult)
                # tree reduce over GS slots
                m1 = work.tile([P, TCH // 2, NW], BF16, tag="b_m1")
                v = gt[:].rearrange("p (a two) f -> p a two f", two=2)
                nc.vector.tensor_tensor(out=m1[:], in0=v[:, :, 0, :],
                                        in1=v[:, :, 1, :], op=A.add)
                m2 = work.tile([P, TPG, NW], BF16, tag="b_m2")
                v = m1[:].rearrange("p (a two) f -> p a two f", two=2)
                nc.vector.tensor_tensor(out=m2[:], in0=v[:, :, 0, :],
                                        in1=v[:, :, 1, :], op=A.add)
                # scatter-add group rows into the per-atom B table
                scatter_add(btab_d[:, 0:NW], m2[:], sidx_t[:], GCH, NW, 2 * BTW)

            _ecm = tc.tile_pool(name="pE", bufs=2)
            worke = _ecm.__enter__()

            # ---------- P5: assemble E ----------
            def emit_p5(k):
                work = worke
                bsum = work.tile([P, ACH, NW], BF16, tag="e_bsum")
                nc.sync.dma_start(
                    out=bsum[:],
                    in_=btab_d[0:NA].rearrange("(a p) f -> p a f", p=P)[
                        :, k * ACH:(k + 1) * ACH, 0:NW])
                ai = work.tile([P, ACH, NW], BF16, tag="e_ai")
                nc.sync.dma_start(
                    out=ai[:],
                    in_=t2sb_d.rearrange("(a p) w -> p a w", p=P)[
                        :, k * ACH:(k + 1) * ACH, :])
                prod = work.tile([P, ACH, NW], F32, tag="e_prod")
                nc.vector.tensor_tensor(out=prod[:], in0=ai[:],
                                        in1=bsum[:], op=A.mult)
                ev = work.tile([P, ACH], F32, tag="e_ev")
                nc.vector.tensor_reduce(out=ev[:], in_=prod[:],
                                        axis=mybir.AxisListType.X, op=A.add)
                nc.vector.tensor_scalar(out=ev[:], in0=ev[:],
                                        scalar1=-0.5 * HARTREE, scalar2=None,
                                        op0=A.mult)
                nc.sync.dma_start(
                    out=e_d.rearrange("(a p) -> p a", p=P)[:, k * ACH:(k + 1) * ACH],
                    in_=ev[:])

            for c in range(NCH):
                emit_passB2(c, c)
            for k_ in range(NACH):
                emit_p5(k_)

            _ecm.__exit__(None, None, None)
            _wcm.__exit__(None, None, None)
    return nc


_PROG_CACHE = {}


def kernel(**inputs):
    species = np.asarray(inputs["species"])
    per_core, meta = preprocess(species, inputs["edge_index"],
                                inputs["lengths"], inputs["partial_charges"])
    rcov = np.asarray(inputs["rcov"], np.float32)
    en = np.asarray(inputs["en"], np.float32)
    sr4 = np.asarray(inputs["sqrt_r4r2"], np.float32)
    refsys = np.asarray(inputs["refsys"]).astype(np.int64)

    # refsys-expanded tables (pure host-side permutation of inputs)
    zeff = np.asarray(inputs["zeff"], np.float32)
    sscale = np.asarray(inputs["sscale"], np.float32)
    gam = np.asarray(inputs["gam"], np.float32)
    secaiw = np.asarray(inputs["secaiw"], np.float32)
    zeff_r = zeff[refsys]
    sscale_r = sscale[refsys]
    gam_r = gam[refsys]
    secaiw_r = secaiw[refsys].reshape(Z, NREF * NW)

    import os as _os
    _bedrock = _os.environ.get("BEDROCK") == "1"
    if not _bedrock:
        key = (tuple(meta["NGBS"]),
               tuple(tuple(s) for ss in meta["asub"] for s in ss))
        if key not in _PROG_CACHE:
            nc = build_program(meta)
            nc.finalize()
            _PROG_CACHE[key] = nc
        nc = _PROG_CACHE[key]

    shared = dict(
        zeff_r=zeff_r, sscale_r=sscale_r, gam_r=gam_r, secaiw_r=secaiw_r,
        refh=np.asarray(inputs["refh"], np.float32),
        ascale=np.asarray(inputs["ascale"], np.float32),
        hcount=np.asarray(inputs["hcount"], np.float32),
        refq=np.asarray(inputs["refq"], np.float32),
        alphaiw=np.asarray(inputs["alphaiw"], np.float32).reshape(Z, NREF * NW),
        gam=gam, zeff=zeff, sqrt_r4r2=sr4,
        ncount_weight=np.asarray(inputs["ncount_weight"], np.float32).reshape(Z, -1),
        cn=np.asarray(inputs["cn"], np.float32).reshape(Z, -1),
        ncount_mask=np.asarray(inputs["ncount_mask"], np.float32).reshape(Z, -1),
        cpw=np.asarray(inputs["cpw"], np.float32),
        s6_raw=np.asarray(inputs["s6_raw"], np.float32),
        s8_raw=np.asarray(inputs["s8_raw"], np.float32),
        a1_raw=np.asarray(inputs["a1_raw"], np.float32),
        a2_raw=np.asarray(inputs["a2_raw"], np.float32),
        scale_q_raw=np.asarray(inputs["scale_q_raw"], np.float32),
    )
    in_maps = []
    for c in range(NCORES):
        ci = build_core_inputs(per_core[c], meta, rcov, en, sr4)
        m = dict(shared)
        m.update(
            pa_pack=ci["pa_pack"], pa2_pack=ci["pa2_pack"],
            pb_pack=ci["pb_pack"],
            jw=ci["jw"].reshape(meta["NCH"], 16, CALL // 16),
            spw=ci["spw"].reshape(NACH, 16, (ACH * P) // 16),
            sidx=ci["sidx"].reshape(meta["NCH"], 16, GCH // 16),
            aidx=ci["aidx"].reshape(meta["NASC"], 16, GCH // 16),
            chg=ci["chg"],
        )
        in_maps.append(m)

    if _bedrock:
        outs = _sim_fallback(build_program(meta), in_maps)
    else:
        try:
            from concourse.bass_utils import run_bass_kernel_spmd
            res = run_bass_kernel_spmd(nc, in_maps, list(range(NCORES)))
            outs = [res.results[c]["e_out"] for c in range(NCORES)]
        except Exception:
            outs = _sim_fallback(build_program(meta), in_maps)
    e = np.concatenate(outs)
    return e[: species.shape[0]].astype(np.float32)


def _sim_fallback(nc, in_maps):
    import inspect
    import textwrap
    from scipy.special import erf as _scipy_erf
    from concourse import bass_interp
    src = textwrap.dedent(inspect.getsource(
        bass_interp.InstructionExecutor.visit_InstActivation))
    if "_scipy_erf" not in src:
        pat = ("    else:\n"
               "        # NOTE: If you are adding a new activation instruction")
        rep = ("    elif instruction.func == mb.ActivationFunctionType.Erf:\n"
               "        acted = _scipy_erf(scaled_and_biased)\n"
               "    else:\n"
               "        # NOTE: If you are adding a new activation instruction")
        assert pat in src
        src = src.replace(pat, rep)
        ns = dict(bass_interp.__dict__)
        ns["_scipy_erf"] = _scipy_erf
        exec(compile(src, "<erfpatch>", "exec"), ns)
        bass_interp.InstructionExecutor.visit_InstActivation = ns[
            "visit_InstActivation"]
    sim = bass_interp.MultiCoreSim(nc, NCORES, num_workers=1)
    for c in range(NCORES):
        for k, v in in_maps[c].items():
            sim.cores[c].tensor(k)[:] = v
    sim.simulate()
    global LAST_EXEC_TIME_NS
    LAST_EXEC_TIME_NS = int(getattr(sim, "global_time", 0))
    return [np.array(sim.cores[c].tensor("e_out")) for c in range(NCORES)]


LAST_EXEC_TIME_NS = None


# revision 55
# speedup vs baseline: 1.1897x; 1.0344x over previous
"""D4 dispersion energy kernel for 8 Trainium2 NeuronCores.

Strategy (v2):
- Host (numpy, integer/permutation work only): sort the edge list by (dst
  atom, j-range bucket), pad each (atom,bucket) edge run to a multiple of 8
  ("groups"), lay slots out in a fixed chunk/call/partition grid, and
  pre-permute all per-edge input data into that slot order.
- Device (all float math):
  * pass A computes per-edge coordination-number contributions and
    tree-reduces them into group sums, then dma_scatter_add's the group sums
    directly into a dense per-atom ncoord table;
  * stage 2 computes per-atom Gaussian weights / zeta / effective alpha
    table A~ from this core's atom slice (bf16, packed 23 floats/atom);
  * one AllGather shares the packed bf16 A~ rows into a 256B-stride table;
  * pass B gathers A~ rows for edge sources via dma_gather (bf16, 46B
    payload), applies Becke-Johnson damping, tree-reduces into group rows and
    dma_scatter_add's them into a dense per-atom B table;
  * E_i = -0.5*HARTREE * <A~_i, B_i>.
"""
import math
import numpy as np

import concourse.bass as bass
import concourse.bacc as bacc
import concourse.tile as tile
from concourse import mybir
from concourse.library_config import mlp as mlp_library

F32 = mybir.dt.float32
BF16 = mybir.dt.bfloat16
I16 = mybir.dt.int16

Z = 87
NREF = 7
NC = 5
NW = 23
BOHR = 0.5291772105638411
HARTREE = 27.211386024367243
K4, K5, K6, KK = 4.10451, 19.08857, 254.5553148552, 7.5
E3 = float(np.exp(3.0))
CPFAC = 3.0 / (2.0 * np.pi)

NCORES = 8
P = 128
ACOLS = 80              # atom columns per partition -> NA = 128*80
NA = P * ACOLS          # atoms per core (10240)
NPAD = NCORES * NA      # padded atom count (81920)
ACH = 16                # atom columns per stage-2 chunk (2048 atoms)
NACH = ACOLS // ACH     # atom chunks per core (5)
CALL = 32768            # idxs per dma_gather call (one per chunk)
TCH = 256               # slots per partition per compute chunk
GS = 4                  # slots per group
CHSLOTS = P * TCH       # slots per compute chunk (32768)
GCH = CHSLOTS // GS     # groups per chunk (4096)
TPG = TCH // GS         # group cells per partition per chunk (32)

# j-range buckets (dma_gather idx is int16)
NBUCK = 3
BBASE = [0, 27307, 54614]
BSIZE = [27307, 27307, NPAD - 54614]

SECB = 6144             # ncoord lo/hi section boundary (atoms, 3 s2 chunks)
SROWW = 320             # per-species row width (f32); 1280 B, 256-aligned
BTW = 64                # per-atom table row width (f32); 256 B stride


def _wrap16(idx_lin):
    """int linear idx list -> [16, ceil(n/16)] int16 wrapped tile.

    The gather/scatter ucode reads indices from the first 16 partitions
    only, so the upload carries just those rows (the SBUF tile is still
    128 partitions tall; rows 16-127 are never read)."""
    n = len(idx_lin)
    m = (n + 15) // 16
    pad = np.zeros(m * 16, np.int16)
    pad[:n] = idx_lin.astype(np.int16)
    return np.ascontiguousarray(pad.reshape(m, 16).T)  # [16, m]


def preprocess(species, edge_index, lengths, partial_charges):
    """Build per-core host-side data. Returns (per_core list of dicts, meta)."""
    n_at = species.shape[0]
    species = np.asarray(species).astype(np.int32)
    idx_i = np.asarray(edge_index[0]).astype(np.int64)
    idx_j = np.asarray(edge_index[1]).astype(np.int64)
    lengths = np.asarray(lengths).astype(np.float32)
    charges = np.asarray(partial_charges).astype(np.float32)

    spec_pad = np.zeros(NPAD, np.int32)
    spec_pad[:n_at] = species
    chg_pad = np.zeros(NPAD, np.float32)
    chg_pad[:n_at] = charges

    # bucket of each edge by j range
    jb = np.searchsorted(np.array(BBASE[1:]), idx_j, side="right")  # 0..2
    key = idx_i * NBUCK + jb
    order = np.argsort(key, kind="stable")
    si = idx_i[order]
    sj = idx_j[order]
    sl = lengths[order]
    sjb = jb[order]

    # count edges per (atom, bucket)
    cnt = np.bincount(idx_i * NBUCK + jb, minlength=NPAD * NBUCK).reshape(NPAD, NBUCK)
    grp = (cnt + GS - 1) // GS  # groups per (atom,bucket)
    # CSR offsets into sorted edge array for (atom,bucket)
    flat_cnt = cnt.reshape(-1)
    edge_off = np.zeros(NPAD * NBUCK + 1, np.int64)
    np.cumsum(flat_cnt, out=edge_off[1:])

    # group quota per bucket (max over cores, rounded to chunk multiple)
    grp_cb = grp.reshape(NCORES, NA, NBUCK).sum(axis=1)  # [core, bucket]
    NGBS = []
    for b in range(NBUCK):
        m = int(grp_cb[:, b].max())
        NGBS.append(((m + GCH - 1) // GCH) * GCH)
    NG = sum(NGBS)                       # groups per core
    SLOTS = NG * GS                      # slots per core
    NCH = SLOTS // CHSLOTS               # compute chunks
    assert SLOTS % CHSLOTS == 0
    # chunk -> bucket map (buckets are whole chunks)
    ch_bucket = []
    for b in range(NBUCK):
        ch_bucket += [b] * (NGBS[b] * GS // CHSLOTS)
    gb_off = np.concatenate([[0], np.cumsum(NGBS)])  # group offset per bucket

    meta = dict(NGBS=NGBS, NG=NG, SLOTS=SLOTS, NCH=NCH, ch_bucket=ch_bucket)

    per_core = []
    for c in range(NCORES):
        a0 = c * NA
        g_c = grp[a0 : a0 + NA]                 # [NA, NBUCK]
        gofs = np.zeros((NA + 1, NBUCK), np.int64)
        np.cumsum(g_c, axis=0, out=gofs[1:])
        ng_b = gofs[NA]                          # real groups per bucket
        for b in range(NBUCK):
            assert ng_b[b] <= NGBS[b]

        # atom id of each core-local group (bucket-sectioned, then padded)
        atom_of_G = np.full(NG, -1, np.int32)   # pads -> -1 (trash rows)
        for b in range(NBUCK):
            rep = np.repeat(np.arange(NA, dtype=np.int32), g_c[:, b])
            atom_of_G[gb_off[b] : gb_off[b] + len(rep)] = rep

        # slot position for each real edge:
        atom_l = si - a0
        core_mask = (atom_l >= 0) & (atom_l < NA)
        e_sel = np.nonzero(core_mask)[0]
        al = atom_l[e_sel]
        eb = sjb[e_sel]
        flat_id = (si[e_sel] * NBUCK + eb)
        rank = (e_sel - edge_off[flat_id])
        grank = rank // GS
        lane = rank % GS
        G = gb_off[eb] + gofs[al, eb] + grank    # core-local group id
        # group cell mapping: scatter token index == group rank within the
        # chunk (atom-monotone), so token prefixes map to atom ranges.
        # token t -> (partition t%128, cell t//128); slots of a group are GS
        # consecutive columns of one partition.
        c_ch = G // GCH
        pp = G % 128
        tg = (G % GCH) // 128
        pos = c_ch * CHSLOTS + (tg * GS + lane) * P + pp

        # per-slot streams (defaults for pad slots)
        r_s = np.full(SLOTS, 1.0e4, np.float32)
        rcj_s = np.ones(SLOTS, np.float32)
        enj_s = np.ones(SLOTS, np.float32)
        rci_s = np.ones(SLOTS, np.float32)
        eni_s = np.ones(SLOTS, np.float32)
        si_s = np.ones(SLOTS, np.float32)
        sj_s = np.ones(SLOTS, np.float32)
        jl_s = np.zeros(SLOTS, np.int32)

        r_s[pos] = sl[e_sel]
        jl_s[pos] = sj[e_sel] - np.array(BBASE, np.int64)[eb]

        # pass-B scatter idx per chunk: token t == group rank in chunk
        sidx = np.zeros((NCH, 16, GCH // 16), np.int16)
        for ch in range(NCH):
            av = atom_of_G[ch * GCH : (ch + 1) * GCH].copy()
            av[av < 0] = 0   # pad groups sum to ~0; row 0 is harmless
            sidx[ch] = _wrap16(av)



        per_core.append(dict(
            pos=pos, e_sel=e_sel, sj=sj[e_sel], sp_i=spec_pad[si[e_sel]],
            sp_j=spec_pad[sj[e_sel]], atom_of_G=atom_of_G,
            r_s=r_s, rcj_s=rcj_s, enj_s=enj_s, rci_s=rci_s, eni_s=eni_s,
            si_s=si_s, sj_s=sj_s, jl_s=jl_s, sidx=sidx,
            spec_slice=spec_pad[a0 : a0 + NA], chg_slice=chg_pad[a0 : a0 + NA],
        ))
    # pass-A sub-scatter structure: split ncoord into lo [0,SECB) / hi
    # [SECB,NA) tables so stage 2 can start before pass A finishes. Token
    # ranges are uniform across cores (SPMD); out-of-section tokens in the
    # overlap zone hit a trash row.
    asub = []      # per slot-chunk: list of (section, tok0, ntok, rowid)
    nrow = 0
    for ch in range(NCH):
        t1s, t0s = [], []
        for pc in per_core:
            av = pc["atom_of_G"][ch * GCH : (ch + 1) * GCH]
            lo = (av >= 0) & (av < SECB)
            hi = av >= SECB
            if lo.any():
                t1s.append(int(np.nonzero(lo)[0][-1]) + 1)
            if hi.any():
                t0s.append(int(np.nonzero(hi)[0][0]))
        subs = []
        if t1s:
            t1 = min((max(t1s) + 127) // 128 * 128, GCH)
            subs.append((0, 0, t1, nrow)); nrow += 1
        if t0s:
            t0 = min(t0s) // 128 * 128
            subs.append((1, t0, GCH - t0, nrow)); nrow += 1
        asub.append(subs)
    meta["asub"] = asub
    meta["NASC"] = nrow
    for pc in per_core:
        aidx = np.zeros((nrow, 16, GCH // 16), np.int16)
        for ch in range(NCH):
            av = pc["atom_of_G"][ch * GCH : (ch + 1) * GCH]
            for (sec, t0, ntok, row) in asub[ch]:
                iv = av[t0:t0 + ntok].copy()
                if sec == 0:
                    bad = ~((iv >= 0) & (iv < SECB))
                    iv[bad] = SECB
                else:
                    sel = iv >= SECB
                    iv = np.where(sel, iv - SECB, NA - SECB)
                aidx[row, :, :ntok // 16] = _wrap16(iv)
        pc["aidx"] = aidx
    return per_core, meta


def build_core_inputs(pc, meta, rcov, en, sqrt_r4r2):
    """Fill species-derived streams + wrapped idx arrays for one core."""
    SLOTS, NCH = meta["SLOTS"], meta["NCH"]
    pos = pc["pos"]
    pc["rcj_s"][pos] = rcov[pc["sp_j"]]
    pc["enj_s"][pos] = en[pc["sp_j"]]
    pc["rci_s"][pos] = rcov[pc["sp_i"]]
    pc["eni_s"][pos] = en[pc["sp_i"]]
    pc["si_s"][pos] = sqrt_r4r2[pc["sp_i"]]
    pc["sj_s"][pos] = sqrt_r4r2[pc["sp_j"]]

    # jidx16: one gather call per chunk, wrapped
    jl = pc["jl_s"]
    jw = np.zeros((NCH, 16, CALL // 16), np.int16)
    for k in range(NCH):
        jw[k] = _wrap16(jl[k * CALL : (k + 1) * CALL])

    # species wrap per atom chunk (2048 atoms): idx position u*128+p ->
    # local atom (16k+u)*128+p  (atom id = col*128 + p)
    spw = np.zeros((NACH, 16, (ACH * P) // 16), np.int16)
    spec = pc["spec_slice"].reshape(ACOLS, P)
    for k in range(NACH):
        lin = spec[k * ACH : (k + 1) * ACH, :].reshape(-1)  # [u, p] -> u*128+p
        spw[k] = _wrap16(lin)

    pa_pack = np.stack([
        pc["r_s"].reshape(NCH, CHSLOTS), pc["rcj_s"].reshape(NCH, CHSLOTS),
        pc["rci_s"].reshape(NCH, CHSLOTS)], axis=1)
    pa2_pack = np.stack([
        pc["enj_s"].reshape(NCH, CHSLOTS),
        pc["eni_s"].reshape(NCH, CHSLOTS)], axis=1)
    pb_pack = np.stack([
        pc["r_s"].reshape(NCH, CHSLOTS), pc["si_s"].reshape(NCH, CHSLOTS),
        pc["sj_s"].reshape(NCH, CHSLOTS)], axis=1)
    return dict(
        pa_pack=pa_pack, pa2_pack=pa2_pack, pb_pack=pb_pack,
        jw=jw.reshape(-1), spw=spw.reshape(-1), sidx=pc["sidx"].reshape(-1),
        aidx=pc["aidx"].reshape(-1),
        chg=np.ascontiguousarray(
            pc["chg_slice"].reshape(ACOLS, P).T).astype(np.float32),
    )


def _bc(ap, n):
    """Broadcast AP: append a step-0 inner dim of size n."""
    return bass.AP(tensor=ap.tensor, offset=ap.offset, ap=[*ap.ap, [0, n]])


def _dma_gather_raw(nc, out_ap, in_ap, idxs_ap, num_idxs, elem_size, elem_step):
    """dma_gather without the elem_size%256 restriction (payload < row pitch).
    Mirrors bass.BassGpSimd.dma_gather (non-transpose, DRAM source)."""
    eng = nc.gpsimd
    assert idxs_ap.dtype == mybir.dt.int16
    assert in_ap.dtype == out_ap.dtype
    stride_bytes = elem_step * mybir.dt.size(in_ap.dtype)
    assert stride_bytes % 256 == 0
    stride_bytes_256 = stride_bytes // 256
    assert in_ap.ap[0][0] == elem_step
    assert in_ap.ap[-1][1] == elem_size
    assert out_ap.ap[-1][1] == elem_size
    _in_ap = eng.lower_ap_dma(in_ap, for_custom_bir_dma=True)
    _idxs_ap = eng.lower_ap(idxs_ap)
    _out_ap = eng.lower_ap(out_ap)
    return eng.add_instruction(
        mybir.InstDMAGatherAnt(
            name=nc.get_next_instruction_name(),
            ins=[*_in_ap, _idxs_ap, eng.lower_val_access(eng.to_reg(num_idxs))],
            outs=[_out_ap],
            transpose=False,
            num_idxs=num_idxs,
            elem_size=elem_size,
            stride_bytes_256=stride_bytes_256,
            gen_mode=0,
            single_packet=True,
            queue_num=0,
            sbuf_tokens_per_rank=0,
            sbuf_free_dim_per_rank=0,
            sbuf_free_dim_pad_per_rank=0,
            sbuf_byte_offset=0,
        )
    )


def build_program(meta):
    SLOTS, NCH = meta["SLOTS"], meta["NCH"]
    ch_bucket = meta["ch_bucket"]
    A = mybir.AluOpType
    AF = mybir.ActivationFunctionType

    nc = bacc.Bacc(None, num_devices=NCORES, dynamic_dma_scratch_size=40960)

    def din(name, shape, dt=F32):
        return nc.dram_tensor(name, shape, dt, kind="ExternalInput")

    # per-slot streams, packed stream-major per chunk (one DMA per chunk)
    pa_d = din("pa_pack", [NCH, 3, CHSLOTS])
    pa2_d = din("pa2_pack", [NCH, 2, CHSLOTS], BF16)
    pb_d = din("pb_pack", [NCH, 3, CHSLOTS])
    jw_d = din("jw", [NCH, 16, CALL // 16], I16)
    spw_d = din("spw", [NACH, 16, (ACH * P) // 16], I16)
    sidx_d = din("sidx", [NCH, 16, GCH // 16], I16)
    aidx_d = din("aidx", [meta["NASC"], 16, GCH // 16], I16)
    chg_d = din("chg", [P, ACOLS])
    # all per-species tables concatenated into one upload (one DMA issue):
    # 0:7 zeff_r | 7:14 sscale_r | 14:21 gam_r | 21:28 refh | 28:35 ascale |
    # 35:42 hcount | 42:49 refq | 49:210 secaiw_r | 210:371 alphaiw |
    # 371 gam | 372 zeff | 373:408 cnw | 408:443 cn | 443:478 mask
    tab87_d = din("tab87", [Z, 478])
    cpw_d = din("cpw", [NW])
    s6_d = din("s6_raw", [1]); s8_d = din("s8_raw", [1])
    a1_d = din("a1_raw", [1]); a2_d = din("a2_raw", [1]); sq_d = din("scale_q_raw", [1])

    srowA_d = nc.dram_tensor("srowad", [Z, 64], F32)
    srowB_d = nc.dram_tensor("srowbd", [Z, 256], BF16)
    ncoL_d = nc.dram_tensor("ncold", [SECB + 16, BTW], F32)
    ncoH_d = nc.dram_tensor("ncohd", [NA - SECB + 16, BTW], F32)
    btab_d = nc.dram_tensor("btabd", [NA, 2 * BTW], BF16)
    t2sb_d = nc.dram_tensor("t2sb", [NA, NW], BF16)
    t2f_d = nc.dram_tensor("t2f", [NPAD, P], BF16, addr_space="Shared")
    e_d = nc.dram_tensor("e_out", [NA], F32, kind="ExternalOutput")

    def brc(dram, parts, width):
        """AP reading a [width] DRAM tensor broadcast across `parts` partitions."""
        return bass.AP(tensor=dram.tensor if hasattr(dram, "tensor") else dram,
                       offset=0, ap=[[0, parts], [1, width]])

    with tile.TileContext(nc) as tc:
        import contextlib
        with contextlib.ExitStack() as ctx:
            const = ctx.enter_context(tc.tile_pool(name="const", bufs=1))
            _srowcm = tc.tile_pool(name="srowp", bufs=1)
            srowp = _srowcm.__enter__()
            _wcm = tc.tile_pool(name="p0", bufs=2)
            work = _wcm.__enter__()

            nc.gpsimd.load_library(mlp_library)

            # dedicated index tiles: ucode reads only rows 0:16, so uploads
            # write just those rows; memset once here to satisfy init checks
            jwt_a = const.tile([P, CALL // 16], I16, tag="jwt0")
            jwt_b = const.tile([P, CALL // 16], I16, tag="jwt1")
            sxt_a = const.tile([P, GCH // 16], I16, tag="sxt0")
            sxt_b = const.tile([P, GCH // 16], I16, tag="sxt1")
            spwt = const.tile([P, (ACH * P) // 16], I16, tag="spwt")
            axt_a = const.tile([P, GCH // 16], I16, tag="axt0")
            axt_b = const.tile([P, GCH // 16], I16, tag="axt1")
            axt_c = const.tile([P, GCH // 16], I16, tag="axt2")
            axt_d = const.tile([P, GCH // 16], I16, tag="axt3")
            jwt2 = [jwt_a, jwt_b]
            sxt2 = [sxt_a, sxt_b]
            axt4 = [axt_a, axt_b, axt_c, axt_d]
            for t_ in (*jwt2, *sxt2, *axt4, spwt):
                nc.gpsimd.memset(t_[:], 0)

            b3_87 = const.tile([Z, 1], F32)
            nc.vector.memset(b3_87[:], 3.0)
            b3_p = const.tile([P, 1], F32)
            nc.vector.memset(b3_p[:], 3.0)
            bk5_p = const.tile([P, 1], F32)
            nc.vector.memset(bk5_p[:], K5)
            bkk_p = const.tile([P, 1], F32)
            nc.vector.memset(bkk_p[:], KK)

            # zero rows of nco/btab tables (only the columns we touch)
            zcol = const.tile([P, ACOLS, 1], F32)
            nc.vector.memset(zcol[:], 0.0)
            nc.sync.dma_start(
                out=ncoL_d[0:SECB].rearrange("(a p) f -> p a f", p=P)[:, :, 0:1],
                in_=zcol[:, 0:SECB // P, :])
            nc.sync.dma_start(
                out=ncoH_d[0:NA - SECB].rearrange("(a p) f -> p a f", p=P)[:, :, 0:1],
                in_=zcol[:, 0:(NA - SECB) // P, :])
            zrow23 = const.tile([P, ACOLS, NW], BF16)
            nc.vector.memset(zrow23[:], 0.0)

            # ---------- P0: per-species row table ----------
            tab = const.tile([Z, 478], F32)
            nc.sync.dma_start(out=tab[:], in_=tab87_d[:])
            t_ = tab[:]
            zeffr, sscr, gamr = t_[:, 0:7], t_[:, 7:14], t_[:, 14:21]
            refh, asc, hcnt = t_[:, 21:28], t_[:, 28:35], t_[:, 35:42]
            refq = t_[:, 42:49]
            secr, aiw = t_[:, 49:210], t_[:, 210:371]
            gam1, zeff1 = t_[:, 371:372], t_[:, 372:373]
            cnw, cnt_, msk = t_[:, 373:408], t_[:, 408:443], t_[:, 443:478]

            # softplus of all 5 scalar params in one Exp->Ln block
            params = const.tile([P, 5], F32)
            for ii, dd in enumerate([s6_d, s8_d, a1_d, a2_d, sq_d]):
                nc.sync.dma_start(out=params[:, ii:ii+1], in_=brc(dd, P, 1))
            nc.scalar.activation(out=params[:], in_=params[:], func=AF.Exp)
            nc.vector.tensor_scalar(out=params[:], in0=params[:], scalar1=1.0,
                                    scalar2=None, op0=A.add)
            nc.scalar.activation(out=params[:], in_=params[:], func=AF.Ln)
            s6p, s8p = params[:, 0:1], params[:, 1:2]
            a1p, a2p = params[:, 2:3], params[:, 3:4]
            spq = params[:, 4:5]
            sq87 = params[0:Z, 4:5]

            qmod = work.tile([Z, NREF], F32, tag="p0a")
            nc.vector.tensor_scalar(out=qmod[:], in0=refh, scalar1=sq87,
                                    scalar2=None, op0=A.mult)
            nc.vector.tensor_tensor(out=qmod[:], in0=qmod[:], in1=zeffr, op=A.add)
            qmsk = work.tile([Z, NREF], F32, tag="p0b")
            nc.vector.tensor_scalar(out=qmsk, in0=qmod[:], scalar1=1e-8,
                                    scalar2=None, op0=A.is_gt)
            qsafe = work.tile([Z, NREF], F32, tag="p0c")
            nc.vector.tensor_scalar(out=qsafe[:], in0=qmod[:], scalar1=1.0,
                                    scalar2=None, op0=A.subtract)
            nc.vector.tensor_tensor(out=qsafe[:], in0=qsafe[:], in1=qmsk,
                                    op=A.mult)
            nc.vector.tensor_scalar(out=qsafe[:], in0=qsafe[:], scalar1=1.0,
                                    scalar2=None, op0=A.add)
            rq = work.tile([Z, NREF], F32, tag="p0d")
            nc.vector.reciprocal(out=rq[:], in_=qsafe[:])
            t0 = work.tile([Z, NREF], F32, tag="p0e")
            nc.vector.tensor_tensor(out=t0[:], in0=zeffr, in1=rq[:], op=A.mult)
            nc.vector.tensor_tensor(out=t0[:], in0=t0[:], in1=gamr, op=A.mult)
            nc.vector.tensor_tensor(out=t0[:], in0=gamr, in1=t0[:], op=A.subtract)
            nc.scalar.activation(out=t0[:], in_=t0[:], func=AF.Exp, scale=2.0)
            nc.scalar.activation(out=t0[:], in_=t0[:], func=AF.Exp, scale=-3.0,
                                 bias=b3_87[:, 0:1])
            zfac = work.tile([Z, NREF], F32, tag="p0f")
            nc.vector.tensor_scalar(out=zfac[:], in0=t0[:], scalar1=E3,
                                    scalar2=None, op0=A.subtract)
            nc.vector.tensor_tensor(out=zfac[:], in0=zfac[:], in1=qmsk,
                                    op=A.mult)
            nc.vector.tensor_scalar(out=zfac[:], in0=zfac[:], scalar1=E3,
                                    scalar2=None, op0=A.add)
            al = work.tile([Z, NREF, NW], F32, tag="p0g")
            nc.vector.tensor_tensor(
                out=al[:], in0=secr.rearrange("z (a w) -> z a w", w=NW),
                in1=_bc(sscr, NW), op=A.mult)
            nc.vector.tensor_tensor(out=al[:], in0=al[:], in1=_bc(zfac[:], NW),
                                    op=A.mult)
            nc.vector.tensor_tensor(out=al[:], in0=al[:], in1=_bc(hcnt, NW),
                                    op=A.mult)
            nc.vector.tensor_tensor(
                out=al[:], in0=aiw.rearrange("z (a w) -> z a w", w=NW),
                in1=al[:], op=A.subtract)
            nc.vector.tensor_tensor(out=al[:], in0=al[:], in1=_bc(asc, NW),
                                    op=A.mult)
            nc.vector.tensor_scalar(out=al[:], in0=al[:], scalar1=0.0,
                                    scalar2=None, op0=A.max)
            cpw87 = const.tile([Z, NW], F32)
            nc.sync.dma_start(out=cpw87[:], in_=brc(cpw_d, Z, NW))
            nc.scalar.activation(out=cpw87[:], in_=cpw87[:], func=AF.Sqrt,
                                 scale=CPFAC)
            wb = bass.AP(tensor=cpw87[:].tensor, offset=cpw87[:].offset,
                         ap=[cpw87[:].ap[0], [0, NREF], [1, NW]])
            nc.vector.tensor_tensor(out=al[:], in0=al[:], in1=wb, op=A.mult)

            # assemble split species rows: f32 part (gam, zeff, refq, cn)
            # + bf16 part (atil, cnw, mask)
            srow = const.tile([Z, 64], F32)
            nc.vector.memset(srow[:], 0.0)
            nc.vector.tensor_copy(out=srow[:, 0:1], in_=gam1)
            nc.vector.tensor_copy(out=srow[:, 1:2], in_=zeff1)
            nc.vector.tensor_copy(out=srow[:, 2:9], in_=refq)
            nc.vector.tensor_copy(out=srow[:, 9:44], in_=cnt_)
            nc.sync.dma_start(out=srowA_d[:], in_=srow[:])
            srowb = const.tile([Z, 256], BF16)
            nc.gpsimd.memset(srowb[:], 0)
            nc.scalar.activation(out=srowb[:, 0:161],
                                 in_=al[:].rearrange("z a w -> z (a w)"),
                                 func=AF.Copy)
            nc.scalar.activation(out=srowb[:, 161:196], in_=cnw, func=AF.Copy)
            nc.scalar.activation(out=srowb[:, 196:231], in_=msk, func=AF.Copy)
            nc.sync.dma_start(out=srowB_d[:], in_=srowb[:])


            _wcm.__exit__(None, None, None)
            _wcm = tc.tile_pool(name="pA", bufs=3)
            work = _wcm.__enter__()

            # ---------- P1: pass A (coordination numbers) ----------
            def ldsidx(c, pool, tag):
                t = sxt2[c % 2]
                nc.sync.dma_start(out=t[0:16, :], in_=sidx_d[c])
                return t

            def scatter_add(out_ap, in_ap, idxs_t, num, elem, step):
                return nc.gpsimd.dma_scatter_add(
                    out_ap, in_ap, idxs_t, num, num, elem, elem_step=step)

            srow_t = []
            asub = meta["asub"]

            def emit_passA(c, pos_):
                st = work.tile([P, 3, TCH], F32, tag="a_st")
                nc.sync.dma_start(
                    out=st[:], in_=pa_d[c].rearrange("v (t p) -> p v t", p=P))
                st2 = work.tile([P, 2, TCH], BF16, tag="a_st2")
                nc.sync.dma_start(
                    out=st2[:], in_=pa2_d[c].rearrange("v (t p) -> p v t", p=P))
                r_t, rcj, rci = st[:, 0, :], st[:, 1, :], st[:, 2, :]
                enj, eni = st2[:, 0, :], st2[:, 1, :]
                # interleave stage-2 srow prefetch gathers (needed only by
                # stage 2) so their DMA doesn't starve pass-A stream loads
                if (pos_ % 2 == 1 or pos_ == NCH - 1) and len(srow_t) < NACH:
                    k = len(srow_t)
                    spw_t = spwt
                    nc.sync.dma_start(out=spw_t[0:16, :], in_=spw_d[k])
                    sg = srowp.tile([P, ACH, 44], F32, tag=f"pf_srow{k}")
                    _dma_gather_raw(nc, sg[:], srowA_d[:, 0:44], spw_t[:],
                                    ACH * P, 44, 64)
                    sgb = srowp.tile([P, ACH, 231], BF16, tag=f"pf_srowb{k}")
                    _dma_gather_raw(nc, sgb[:], srowB_d[:, 0:231], spw_t[:],
                                    ACH * P, 231, 256)
                    srow_t.append((sg, sgb))
                # rcv = 4/3*(rci+rcj)
                rcv = work.tile([P, TCH], F32, tag="a_rcv")
                nc.vector.tensor_tensor(out=rcv[:], in0=rci, in1=rcj, op=A.add)
                nc.vector.tensor_scalar(out=rcv[:], in0=rcv[:], scalar1=4.0 / 3.0,
                                        scalar2=None, op0=A.mult)
                # den = K4*exp(-((|eni-enj|+K5)^2)/K6) via sigmoid identity:
                # exp(-v) = 1/sigmoid(v) - 1
                den = work.tile([P, TCH], F32, tag="a_den")
                nc.vector.tensor_tensor(out=den[:], in0=eni, in1=enj,
                                        op=A.subtract)
                nc.scalar.activation(out=den[:], in_=den[:], func=AF.Abs)
                nc.scalar.activation(out=den[:], in_=den[:], func=AF.Square,
                                     bias=bk5_p[:, 0:1])
                nc.scalar.activation(out=den[:], in_=den[:], func=AF.Sigmoid,
                                     scale=1.0 / K6)
                nc.vector.reciprocal(out=den[:], in_=den[:])
                nc.vector.tensor_scalar(out=den[:], in0=den[:], scalar1=1.0,
                                        scalar2=0.5 * K4, op0=A.subtract,
                                        op1=A.mult)
                # erf(-KK*(rr-rcv)/rcv) = Erf(-KK*u + KK), u = rr/rcv
                cf = work.tile([P, TCH], F32, tag="a_cf")
                nc.vector.reciprocal(out=cf[:], in_=rcv[:])
                nc.vector.tensor_tensor(out=cf[:], in0=cf[:], in1=r_t, op=A.mult)
                nc.scalar.activation(out=cf[:], in_=cf[:], func=AF.Erf,
                                     scale=-KK / BOHR, bias=bkk_p[:, 0:1])
                # countf = (erf + 1) * den_scaled
                nc.vector.scalar_tensor_tensor(out=cf[:], in0=cf[:], scalar=1.0,
                                               in1=den[:], op0=A.add, op1=A.mult)
                # tree reduce GS -> 1
                l1 = work.tile([P, TCH // 2], F32, tag="a_l1")
                v = cf[:].rearrange("p (a two) -> p a two", two=2)
                nc.vector.tensor_tensor(out=l1[:], in0=v[:, :, 0], in1=v[:, :, 1],
                                        op=A.add)
                l2 = work.tile([P, TPG, 1], F32, tag="a_l2")
                v = l1[:].rearrange("p (a two) -> p a two", two=2)
                nc.vector.tensor_tensor(out=l2[:, :, 0],
                                        in0=v[:, :, 0], in1=v[:, :, 1],
                                        op=A.add)
                # scatter-add group sums into lo/hi per-atom ncoord tables
                for (sec, t0, ntok, row) in asub[c]:
                    it = axt4[(pos_ * 2 + sec) % 4]
                    nc.sync.dma_start(out=it[0:16, 0:ntok // 16],
                                      in_=aidx_d[row][:, 0:ntok // 16])
                    tab = ncoL_d if sec == 0 else ncoH_d
                    scatter_add(tab[:, 0:1],
                                l2[:, t0 // 128:(t0 + ntok) // 128, :],
                                it[:, 0:ntok // 16], ntok, 1, BTW)

            _s2cm = tc.tile_pool(name="pS2", bufs=2)
            works2 = _s2cm.__enter__()

            # ---------- P2: stage 2 (per-atom A~ rows) ----------
            def emit_s2(k):
                work = works2
                sr = srow_t[k][0][:]
                srb = srow_t[k][1][:]
                nco = work.tile([P, ACH, 1], F32, tag="s2_nco")
                if k * ACH < SECB // P:
                    nsrc = ncoL_d[0:SECB].rearrange("(a p) f -> p a f", p=P)[
                        :, k * ACH:(k + 1) * ACH, 0:1]
                else:
                    k2 = k - SECB // P // ACH
                    nsrc = ncoH_d[0:NA - SECB].rearrange(
                        "(a p) f -> p a f", p=P)[
                        :, k2 * ACH:(k2 + 1) * ACH, 0:1]
                nc.sync.dma_start(out=nco[:], in_=nsrc)

                # gaussian weights gw[P, ACH, NREF]
                gw35 = work.tile([P, ACH, NREF * NC], F32, tag="s2_gw35")
                nc.vector.tensor_tensor(out=gw35[:],
                                        in0=_bc(nco[:, :, 0], NREF * NC),
                                        in1=sr[:, :, 9:44], op=A.subtract)
                nc.vector.tensor_tensor(out=gw35[:], in0=gw35[:], in1=gw35[:],
                                        op=A.mult)
                nc.vector.tensor_tensor(out=gw35[:], in0=gw35[:],
                                        in1=srb[:, :, 161:196], op=A.mult)
                nc.scalar.activation(out=gw35[:], in_=gw35[:], func=AF.Exp,
                                     scale=-6.0)
                nc.vector.tensor_tensor(out=gw35[:], in0=gw35[:],
                                        in1=srb[:, :, 196:231], op=A.mult)
                gw = work.tile([P, ACH, NREF], F32, tag="s2_gw")
                g5 = gw35[:].rearrange("p c (a n) -> p c a n", n=NC)
                nc.vector.tensor_tensor(out=gw[:], in0=g5[:, :, :, 0],
                                        in1=g5[:, :, :, 1], op=A.add)
                for n5 in range(2, NC):
                    nc.vector.tensor_tensor(out=gw[:], in0=gw[:],
                                            in1=g5[:, :, :, n5], op=A.add)
                nrm = work.tile([P, ACH], F32, tag="s2_nrm")
                nc.vector.tensor_reduce(out=nrm[:], in_=gw[:],
                                        axis=mybir.AxisListType.X, op=A.add)
                nc.vector.tensor_scalar(out=nrm[:], in0=nrm[:], scalar1=1e-7,
                                        scalar2=None, op0=A.max)
                nc.vector.reciprocal(out=nrm[:], in_=nrm[:])
                nc.vector.tensor_tensor(out=gw[:], in0=gw[:], in1=_bc(nrm[:], NREF),
                                        op=A.mult)
                # zeta
                chg_t = work.tile([P, ACH], F32, tag="s2_chg")
                nc.sync.dma_start(out=chg_t[:], in_=chg_d[:, k * ACH:(k + 1) * ACH])
                qmod2 = work.tile([P, ACH], F32, tag="s2_qm")
                nc.vector.tensor_tensor(out=qmod2[:], in0=chg_t[:],
                                        in1=sr[:, :, 1], op=A.add)
                msk2 = work.tile([P, ACH], F32, tag="s2_msk")
                nc.vector.tensor_scalar(out=msk2[:], in0=qmod2[:], scalar1=1e-8,
                                        scalar2=None, op0=A.is_gt)
                qs2 = work.tile([P, ACH], F32, tag="s2_qs")
                nc.vector.tensor_scalar(out=qs2[:], in0=qmod2[:], scalar1=1.0,
                                        scalar2=None, op0=A.subtract)
                nc.vector.tensor_tensor(out=qs2[:], in0=qs2[:], in1=msk2[:],
                                        op=A.mult)
                nc.vector.tensor_scalar(out=qs2[:], in0=qs2[:], scalar1=1.0,
                                        scalar2=None, op0=A.add)
                nc.vector.reciprocal(out=qs2[:], in_=qs2[:])
                zt = work.tile([P, ACH, NREF], F32, tag="s2_zt")
                nc.vector.tensor_scalar(out=zt[:], in0=sr[:, :, 2:9],
                                        scalar1=spq, scalar2=None,
                                        op0=A.mult)
                nc.vector.tensor_tensor(out=zt[:], in0=zt[:],
                                        in1=_bc(sr[:, :, 1], NREF), op=A.add)
                nc.vector.tensor_tensor(out=zt[:], in0=zt[:],
                                        in1=_bc(qs2[:], NREF), op=A.mult)
                nc.vector.tensor_tensor(out=zt[:], in0=zt[:],
                                        in1=_bc(sr[:, :, 0], NREF), op=A.mult)
                nc.vector.tensor_tensor(out=zt[:], in0=_bc(sr[:, :, 0], NREF),
                                        in1=zt[:], op=A.subtract)
                nc.scalar.activation(out=zt[:], in_=zt[:], func=AF.Exp, scale=2.0)
                nc.scalar.activation(out=zt[:], in_=zt[:], func=AF.Exp,
                                     scale=-3.0, bias=b3_p[:, 0:1])
                zeta = work.tile([P, ACH, NREF], F32, tag="s2_zeta")
                mb = bass.AP(tensor=msk2[:].tensor, offset=msk2[:].offset,
                             ap=[*msk2[:].ap, [0, NREF]])
                nc.vector.tensor_scalar(out=zeta[:], in0=zt[:], scalar1=E3,
                                        scalar2=None, op0=A.subtract)
                nc.vector.tensor_tensor(out=zeta[:], in0=zeta[:], in1=mb,
                                        op=A.mult)
                nc.vector.tensor_scalar(out=zeta[:], in0=zeta[:], scalar1=E3,
                                        scalar2=None, op0=A.add)
                nc.vector.tensor_tensor(out=zeta[:], in0=zeta[:], in1=gw[:],
                                        op=A.mult)
                # A~_i[w] = sum_a zeta[a]*atil[a,w]; replicate zeta over w
                # and cast atil to bf16 on the Activation engine so the DVE
                # mult/add chain runs in the packed-bf16 2x mode
                zrep = work.tile([P, ACH, NREF, NW], BF16, tag="s2_zrep")
                nc.scalar.activation(out=zrep[:], in_=_bc(zeta[:], NW),
                                     func=AF.Copy)
                av = srb[:, :, 0:161].rearrange("p c (a w) -> p c a w", w=NW)
                t2row = work.tile([P, ACH, NW], BF16, tag="s2_t2row")
                for a_ in range(NREF):
                    if a_ == 0:
                        nc.vector.tensor_tensor(
                            out=t2row[:], in0=av[:, :, 0, :],
                            in1=zrep[:, :, 0, :], op=A.mult)
                    else:
                        tmp_ = work.tile([P, ACH, NW], BF16, tag="s2_tmp")
                        nc.vector.tensor_tensor(
                            out=tmp_[:], in0=av[:, :, a_, :],
                            in1=zrep[:, :, a_, :], op=A.mult)
                        nc.vector.tensor_tensor(out=t2row[:], in0=t2row[:],
                                                in1=tmp_[:], op=A.add)
                nc.sync.dma_start(
                    out=t2sb_d.rearrange("(a p) w -> p a w", p=P)[
                        :, k * ACH:(k + 1) * ACH, :],
                    in_=t2row[:])

            # ---------- driver: pass A round-robin with interleaved s2 ----
            cpb = [meta["NGBS"][b] * GS // CHSLOTS for b in range(NBUCK)]
            coff = [0]
            for b in range(NBUCK):
                coff.append(coff[-1] + cpb[b])
            order = []
            for cc in range(max(cpb)):
                for b in range(NBUCK):
                    if cc < cpb[b]:
                        order.append(coff[b] + cc)
            lo_done_pos = max(
                pos_ for pos_, c in enumerate(order)
                if any(s[0] == 0 for s in meta["asub"][c]))
            for pos_, c in enumerate(order):
                emit_passA(c, pos_)
                if pos_ == lo_done_pos:
                    for k_ in range(SECB // P // ACH):
                        emit_s2(k_)
            for k_ in range(SECB // P // ACH, NACH):
                emit_s2(k_)

            _s2cm.__exit__(None, None, None)
            _wcm.__exit__(None, None, None)
            _srowcm.__exit__(None, None, None)

            # zero the B table now — its only consumers are the pass-B
            # scatter-adds, so this write rides the idle DMA window here
            nc.sync.dma_start(
                out=btab_d[0:NA].rearrange("(a p) f -> p a f", p=P)[:, :, 0:NW],
                in_=zrow23[:])

            # ---------- P3: AllGather packed bf16 A~ rows into strided table --
            # (emitted before pass B1 in program order so the Pool engine
            # starts the collective while the DVE computes damping factors)
            nc.gpsimd.collective_compute(
                "AllGather", A.bypass,
                replica_groups=[list(range(NCORES))],
                ins=[t2sb_d[:]], outs=[t2f_d[:, 0:NW]])

            _wcm = tc.tile_pool(name="pB", bufs=2)
            work = _wcm.__enter__()

            # ---------- P4a: pass B1 — damping factors (overlaps AllGather) --
            dbts = []
            for c in range(NCH):
                sb = work.tile([P, 3, TCH], F32, tag="b_sb")
                nc.sync.dma_start(
                    out=sb[:], in_=pb_d[c].rearrange("v (t p) -> p v t", p=P))
                r_t, si_t, sj_t = sb[:, 0, :], sb[:, 1, :], sb[:, 2, :]
                r2 = work.tile([P, TCH], F32, tag="b_r2")
                nc.scalar.activation(out=r2[:], in_=r_t, func=AF.Square,
                                     scale=1.0 / BOHR)
                r4 = work.tile([P, TCH], F32, tag="b_r4")
                nc.scalar.activation(out=r4[:], in_=r2[:], func=AF.Square)
                r8 = work.tile([P, TCH], F32, tag="b_r8")
                nc.scalar.activation(out=r8[:], in_=r4[:], func=AF.Square)
                r6 = work.tile([P, TCH], F32, tag="b_r6")
                nc.vector.tensor_tensor(out=r6[:], in0=r4[:], in1=r2[:], op=A.mult)
                R3 = work.tile([P, TCH], F32, tag="b_R3")
                nc.vector.scalar_tensor_tensor(out=R3[:], in0=si_t, scalar=3.0,
                                               in1=sj_t, op0=A.mult,
                                               op1=A.mult)
                r0 = work.tile([P, TCH], F32, tag="b_r0")
                nc.scalar.activation(out=r0[:], in_=R3[:], func=AF.Sqrt)
                nc.vector.tensor_scalar(out=r0[:], in0=r0[:], scalar1=a1p,
                                        scalar2=a2p, op0=A.mult, op1=A.add)
                q2 = work.tile([P, TCH], F32, tag="b_q2")
                nc.scalar.activation(out=q2[:], in_=r0[:], func=AF.Square)
                c4 = work.tile([P, TCH], F32, tag="b_c4")
                nc.scalar.activation(out=c4[:], in_=q2[:], func=AF.Square)
                c3 = work.tile([P, TCH], F32, tag="b_c3")
                nc.vector.tensor_tensor(out=c3[:], in0=c4[:], in1=q2[:], op=A.mult)
                c8 = work.tile([P, TCH], F32, tag="b_c8")
                nc.scalar.activation(out=c8[:], in_=c4[:], func=AF.Square)
                d6 = work.tile([P, TCH], F32, tag="b_d6")
                nc.vector.tensor_tensor(out=d6[:], in0=r6[:], in1=c3[:], op=A.add)
                nc.vector.reciprocal(out=d6[:], in_=d6[:])
                d8 = work.tile([P, TCH], F32, tag="b_d8")
                nc.vector.tensor_tensor(out=d8[:], in0=r8[:], in1=c8[:], op=A.add)
                nc.vector.reciprocal(out=d8[:], in_=d8[:])
                nc.vector.tensor_tensor(out=d8[:], in0=d8[:], in1=R3[:], op=A.mult)
                nc.vector.tensor_scalar(out=d8[:], in0=d8[:], scalar1=s8p,
                                        scalar2=None, op0=A.mult)
                d6c = const.tile([P, TCH], F32, tag=f"b_d6_{c}")
                nc.vector.scalar_tensor_tensor(out=d6c[:], in0=d6[:], scalar=s6p,
                                               in1=d8[:], op0=A.mult, op1=A.add)
                dbts.append(d6c)

            # ---------- P4b: pass B2 (gather + scale + reduce + scatter) -----
            cpb = [meta["NGBS"][b] * GS // CHSLOTS for b in range(NBUCK)]
            coff = [0]
            for b in range(NBUCK):
                coff.append(coff[-1] + cpb[b])
            order = []
            for cc in range(max(cpb)):
                for b in range(NBUCK):
                    if cc < cpb[b]:
                        order.append(coff[b] + cc)
            lo_done_pos = max(
                pos_ for pos_, c in enumerate(order)
                if any(s[0] == 0 for s in meta["asub"][c]))

            def emit_passB2(c, pos_):
                b = ch_bucket[c]
                sidx_t = ldsidx(c, work, "b_sidx")
                jw_t = jwt2[pos_ % 2]
                nc.sync.dma_start(out=jw_t[0:16, :], in_=jw_d[c])
                gt = work.tile([P, TCH, NW], BF16, tag="b_g")
                _dma_gather_raw(
                    nc, gt[:],
                    t2f_d[BBASE[b]:BBASE[b] + BSIZE[b], 0:NW],
                    jw_t[:], CALL, NW, P)
                d6c = dbts[c]
                # replicate D over the 23 w-columns on the Activation engine
                # (bf16 cast + broadcast), keeping the DVE mult in 2x mode
                db = bass.AP(tensor=d6c[:].tensor, offset=d6c[:].offset,
                             ap=[*d6c[:].ap, [0, NW]])
                drep = work.tile([P, TCH, NW], BF16, tag="b_drep")
                nc.scalar.activation(out=drep[:], in_=db, func=AF.Copy)
                nc.vector.tensor_tensor(out=gt[:], in0=gt[:], in1=drep[:],
                                        op=A.mult)
                # tree reduce over GS slots
                m1 = work.tile([P, TCH // 2, NW], BF16, tag="b_m1")
                v = gt[:].rearrange("p (a two) f -> p a two f", two=2)
                nc.vector.tensor_tensor(out=m1[:], in0=v[:, :, 0, :],
                                        in1=v[:, :, 1, :], op=A.add)
                m2 = work.tile([P, TPG, NW], BF16, tag="b_m2")
                v = m1[:].rearrange("p (a two) f -> p a two f", two=2)
                nc.vector.tensor_tensor(out=m2[:], in0=v[:, :, 0, :],
                                        in1=v[:, :, 1, :], op=A.add)
                # scatter-add group rows into the per-atom B table
                scatter_add(btab_d[:, 0:NW], m2[:], sidx_t[:], GCH, NW, 2 * BTW)

            _ecm = tc.tile_pool(name="pE", bufs=2)
            worke = _ecm.__enter__()

            # ---------- P5: assemble E ----------
            def emit_p5(k):
                work = worke
                bsum = work.tile([P, ACH, NW], BF16, tag="e_bsum")
                nc.sync.dma_start(
                    out=bsum[:],
                    in_=btab_d[0:NA].rearrange("(a p) f -> p a f", p=P)[
                        :, k * ACH:(k + 1) * ACH, 0:NW])
                ai = work.tile([P, ACH, NW], BF16, tag="e_ai")
                nc.sync.dma_start(
                    out=ai[:],
                    in_=t2sb_d.rearrange("(a p) w -> p a w", p=P)[
                        :, k * ACH:(k + 1) * ACH, :])
                prod = work.tile([P, ACH, NW], F32, tag="e_prod")
                nc.vector.tensor_tensor(out=prod[:], in0=ai[:],
                                        in1=bsum[:], op=A.mult)
                ev = work.tile([P, ACH], F32, tag="e_ev")
                nc.vector.tensor_reduce(out=ev[:], in_=prod[:],
                                        axis=mybir.AxisListType.X, op=A.add)
                nc.vector.tensor_scalar(out=ev[:], in0=ev[:],
                                        scalar1=-0.5 * HARTREE, scalar2=None,
                                        op0=A.mult)
                nc.sync.dma_start(
                    out=e_d.rearrange("(a p) -> p a", p=P)[:, k * ACH:(k + 1) * ACH],
                    in_=ev[:])

            for c in range(NCH):
                emit_passB2(c, c)
            for k_ in range(NACH):
                emit_p5(k_)

            _ecm.__exit__(None, None, None)
            _wcm.__exit__(None, None, None)
    return nc


_PROG_CACHE = {}


def kernel(**inputs):
    species = np.asarray(inputs["species"])
    per_core, meta = preprocess(species, inputs["edge_index"],
                                inputs["lengths"], inputs["partial_charges"])
    rcov = np.asarray(inputs["rcov"], np.float32)
    en = np.asarray(inputs["en"], np.float32)
    sr4 = np.asarray(inputs["sqrt_r4r2"], np.float32)
    refsys = np.asarray(inputs["refsys"]).astype(np.int64)

    # refsys-expanded tables (pure host-side permutation of inputs)
    zeff = np.asarray(inputs["zeff"], np.float32)
    sscale = np.asarray(inputs["sscale"], np.float32)
    gam = np.asarray(inputs["gam"], np.float32)
    secaiw = np.asarray(inputs["secaiw"], np.float32)
    zeff_r = zeff[refsys]
    sscale_r = sscale[refsys]
    gam_r = gam[refsys]
    secaiw_r = secaiw[refsys].reshape(Z, NREF * NW)

    import os as _os
    _bedrock = _os.environ.get("BEDROCK") == "1"
    if not _bedrock:
        key = (tuple(meta["NGBS"]),
               tuple(tuple(s) for ss in meta["asub"] for s in ss))
        if key not in _PROG_CACHE:
            nc = build_program(meta)
            nc.finalize()
            _PROG_CACHE[key] = nc
        nc = _PROG_CACHE[key]

    tab87 = np.concatenate([
        zeff_r, sscale_r, gam_r,
        np.asarray(inputs["refh"], np.float32),
        np.asarray(inputs["ascale"], np.float32),
        np.asarray(inputs["hcount"], np.float32),
        np.asarray(inputs["refq"], np.float32),
        secaiw_r,
        np.asarray(inputs["alphaiw"], np.float32).reshape(Z, NREF * NW),
        gam[:, None], zeff[:, None],
        np.asarray(inputs["ncount_weight"], np.float32).reshape(Z, -1),
        np.asarray(inputs["cn"], np.float32).reshape(Z, -1),
        np.asarray(inputs["ncount_mask"], np.float32).reshape(Z, -1),
    ], axis=1).astype(np.float32)
    shared = dict(
        tab87=tab87,
        cpw=np.asarray(inputs["cpw"], np.float32),
        s6_raw=np.asarray(inputs["s6_raw"], np.float32),
        s8_raw=np.asarray(inputs["s8_raw"], np.float32),
        a1_raw=np.asarray(inputs["a1_raw"], np.float32),
        a2_raw=np.asarray(inputs["a2_raw"], np.float32),
        scale_q_raw=np.asarray(inputs["scale_q_raw"], np.float32),
    )
    in_maps = []
    for c in range(NCORES):
        ci = build_core_inputs(per_core[c], meta, rcov, en, sr4)
        m = dict(shared)
        m.update(
            pa_pack=ci["pa_pack"], pa2_pack=ci["pa2_pack"],
            pb_pack=ci["pb_pack"],
            jw=ci["jw"].reshape(meta["NCH"], 16, CALL // 16),
            spw=ci["spw"].reshape(NACH, 16, (ACH * P) // 16),
            sidx=ci["sidx"].reshape(meta["NCH"], 16, GCH // 16),
            aidx=ci["aidx"].reshape(meta["NASC"], 16, GCH // 16),
            chg=ci["chg"],
        )
        in_maps.append(m)

    if _bedrock:
        outs = _sim_fallback(build_program(meta), in_maps)
    else:
        try:
            from concourse.bass_utils import run_bass_kernel_spmd
            res = run_bass_kernel_spmd(nc, in_maps, list(range(NCORES)))
            outs = [res.results[c]["e_out"] for c in range(NCORES)]
        except Exception:
            outs = _sim_fallback(build_program(meta), in_maps)
    e = np.concatenate(outs)
    return e[: species.shape[0]].astype(np.float32)


def _sim_fallback(nc, in_maps):
    import inspect
    import textwrap
    from scipy.special import erf as _scipy_erf
    from concourse import bass_interp
    src = textwrap.dedent(inspect.getsource(
        bass_interp.InstructionExecutor.visit_InstActivation))
    if "_scipy_erf" not in src:
        pat = ("    else:\n"
               "        # NOTE: If you are adding a new activation instruction")
        rep = ("    elif instruction.func == mb.ActivationFunctionType.Erf:\n"
               "        acted = _scipy_erf(scaled_and_biased)\n"
               "    else:\n"
               "        # NOTE: If you are adding a new activation instruction")
        assert pat in src
        src = src.replace(pat, rep)
        ns = dict(bass_interp.__dict__)
        ns["_scipy_erf"] = _scipy_erf
        exec(compile(src, "<erfpatch>", "exec"), ns)
        bass_interp.InstructionExecutor.visit_InstActivation = ns[
            "visit_InstActivation"]
    sim = bass_interp.MultiCoreSim(nc, NCORES, num_workers=1)
    for c in range(NCORES):
        for k, v in in_maps[c].items():
            sim.cores[c].tensor(k)[:] = v
    sim.simulate()
    global LAST_EXEC_TIME_NS
    LAST_EXEC_TIME_NS = int(getattr(sim, "global_time", 0))
    return [np.array(sim.cores[c].tensor("e_out")) for c in range(NCORES)]


LAST_EXEC_TIME_NS = None


# revision 63
# speedup vs baseline: 1.2067x; 1.0143x over previous
"""D4 dispersion energy kernel for 8 Trainium2 NeuronCores.

Strategy (v2):
- Host (numpy, integer/permutation work only): sort the edge list by (dst
  atom, j-range bucket), pad each (atom,bucket) edge run to a multiple of 8
  ("groups"), lay slots out in a fixed chunk/call/partition grid, and
  pre-permute all per-edge input data into that slot order.
- Device (all float math):
  * pass A computes per-edge coordination-number contributions and
    tree-reduces them into group sums, then dma_scatter_add's the group sums
    directly into a dense per-atom ncoord table;
  * stage 2 computes per-atom Gaussian weights / zeta / effective alpha
    table A~ from this core's atom slice (bf16, packed 23 floats/atom);
  * one AllGather shares the packed bf16 A~ rows into a 256B-stride table;
  * pass B gathers A~ rows for edge sources via dma_gather (bf16, 46B
    payload), applies Becke-Johnson damping, tree-reduces into group rows and
    dma_scatter_add's them into a dense per-atom B table;
  * E_i = -0.5*HARTREE * <A~_i, B_i>.
"""
import math
import numpy as np

import concourse.bass as bass
import concourse.bacc as bacc
import concourse.tile as tile
from concourse import mybir
from concourse.library_config import mlp as mlp_library

F32 = mybir.dt.float32
BF16 = mybir.dt.bfloat16
I16 = mybir.dt.int16

Z = 87
NREF = 7
NC = 5
NW = 23
BOHR = 0.5291772105638411
HARTREE = 27.211386024367243
K4, K5, K6, KK = 4.10451, 19.08857, 254.5553148552, 7.5
E3 = float(np.exp(3.0))
CPFAC = 3.0 / (2.0 * np.pi)

NCORES = 8
P = 128
ACOLS = 80              # atom columns per partition -> NA = 128*80
NA = P * ACOLS          # atoms per core (10240)
NPAD = NCORES * NA      # padded atom count (81920)
ACH = 16                # atom columns per stage-2 chunk (2048 atoms)
NACH = ACOLS // ACH     # atom chunks per core (5)
CALL = 32768            # idxs per dma_gather call (one per chunk)
TCH = 256               # slots per partition per compute chunk
GS = 4                  # slots per group
CHSLOTS = P * TCH       # slots per compute chunk (32768)
GCH = CHSLOTS // GS     # groups per chunk (4096)
TPG = TCH // GS         # group cells per partition per chunk (32)

# j-range buckets (dma_gather idx is int16)
NBUCK = 3
BBASE = [0, 27307, 54614]
BSIZE = [27307, 27307, NPAD - 54614]

SECB = 6144             # ncoord lo/hi section boundary (atoms, 3 s2 chunks)
SROWW = 320             # per-species row width (f32); 1280 B, 256-aligned
BTW = 64                # per-atom table row width (f32); 256 B stride


def _wrap16(idx_lin):
    """int linear idx list -> [16, ceil(n/16)] int16 wrapped tile.

    The gather/scatter ucode reads indices from the first 16 partitions
    only, so the upload carries just those rows (the SBUF tile is still
    128 partitions tall; rows 16-127 are never read)."""
    n = len(idx_lin)
    m = (n + 15) // 16
    pad = np.zeros(m * 16, np.int16)
    pad[:n] = idx_lin.astype(np.int16)
    return np.ascontiguousarray(pad.reshape(m, 16).T)  # [16, m]


def preprocess(species, edge_index, lengths, partial_charges):
    """Build per-core host-side data. Returns (per_core list of dicts, meta)."""
    n_at = species.shape[0]
    species = np.asarray(species).astype(np.int32)
    idx_i = np.asarray(edge_index[0]).astype(np.int64)
    idx_j = np.asarray(edge_index[1]).astype(np.int64)
    lengths = np.asarray(lengths).astype(np.float32)
    charges = np.asarray(partial_charges).astype(np.float32)

    spec_pad = np.zeros(NPAD, np.int32)
    spec_pad[:n_at] = species
    chg_pad = np.zeros(NPAD, np.float32)
    chg_pad[:n_at] = charges

    # bucket of each edge by j range
    jb = np.searchsorted(np.array(BBASE[1:]), idx_j, side="right")  # 0..2
    key = idx_i * NBUCK + jb
    order = np.argsort(key, kind="stable")
    si = idx_i[order]
    sj = idx_j[order]
    sl = lengths[order]
    sjb = jb[order]

    # count edges per (atom, bucket)
    cnt = np.bincount(idx_i * NBUCK + jb, minlength=NPAD * NBUCK).reshape(NPAD, NBUCK)
    grp = (cnt + GS - 1) // GS  # groups per (atom,bucket)
    # CSR offsets into sorted edge array for (atom,bucket)
    flat_cnt = cnt.reshape(-1)
    edge_off = np.zeros(NPAD * NBUCK + 1, np.int64)
    np.cumsum(flat_cnt, out=edge_off[1:])

    # group quota per bucket (max over cores, rounded to chunk multiple)
    grp_cb = grp.reshape(NCORES, NA, NBUCK).sum(axis=1)  # [core, bucket]
    NGBS = []
    for b in range(NBUCK):
        m = int(grp_cb[:, b].max())
        NGBS.append(((m + GCH - 1) // GCH) * GCH)
    NG = sum(NGBS)                       # groups per core
    SLOTS = NG * GS                      # slots per core
    NCH = SLOTS // CHSLOTS               # compute chunks
    assert SLOTS % CHSLOTS == 0
    # chunk -> bucket map (buckets are whole chunks)
    ch_bucket = []
    for b in range(NBUCK):
        ch_bucket += [b] * (NGBS[b] * GS // CHSLOTS)
    gb_off = np.concatenate([[0], np.cumsum(NGBS)])  # group offset per bucket

    meta = dict(NGBS=NGBS, NG=NG, SLOTS=SLOTS, NCH=NCH, ch_bucket=ch_bucket)

    per_core = []
    for c in range(NCORES):
        a0 = c * NA
        g_c = grp[a0 : a0 + NA]                 # [NA, NBUCK]
        gofs = np.zeros((NA + 1, NBUCK), np.int64)
        np.cumsum(g_c, axis=0, out=gofs[1:])
        ng_b = gofs[NA]                          # real groups per bucket
        for b in range(NBUCK):
            assert ng_b[b] <= NGBS[b]

        # atom id of each core-local group (bucket-sectioned, then padded)
        atom_of_G = np.full(NG, -1, np.int32)   # pads -> -1 (trash rows)
        for b in range(NBUCK):
            rep = np.repeat(np.arange(NA, dtype=np.int32), g_c[:, b])
            atom_of_G[gb_off[b] : gb_off[b] + len(rep)] = rep

        # slot position for each real edge:
        atom_l = si - a0
        core_mask = (atom_l >= 0) & (atom_l < NA)
        e_sel = np.nonzero(core_mask)[0]
        al = atom_l[e_sel]
        eb = sjb[e_sel]
        flat_id = (si[e_sel] * NBUCK + eb)
        rank = (e_sel - edge_off[flat_id])
        grank = rank // GS
        lane = rank % GS
        G = gb_off[eb] + gofs[al, eb] + grank    # core-local group id
        # group cell mapping: scatter token index == group rank within the
        # chunk (atom-monotone), so token prefixes map to atom ranges.
        # token t -> (partition t%128, cell t//128); slots of a group are GS
        # consecutive columns of one partition.
        c_ch = G // GCH
        pp = G % 128
        tg = (G % GCH) // 128
        pos = c_ch * CHSLOTS + (tg * GS + lane) * P + pp

        # per-slot streams (defaults for pad slots)
        r_s = np.full(SLOTS, 1.0e4, np.float32)
        rcj_s = np.ones(SLOTS, np.float32)
        enj_s = np.ones(SLOTS, np.float32)
        rci_s = np.ones(SLOTS, np.float32)
        eni_s = np.ones(SLOTS, np.float32)
        si_s = np.ones(SLOTS, np.float32)
        sj_s = np.ones(SLOTS, np.float32)
        jl_s = np.zeros(SLOTS, np.int32)

        r_s[pos] = sl[e_sel]
        jl_s[pos] = sj[e_sel] - np.array(BBASE, np.int64)[eb]

        # pass-B scatter idx per chunk: token t == group rank in chunk
        sidx = np.zeros((NCH, 16, GCH // 16), np.int16)
        for ch in range(NCH):
            av = atom_of_G[ch * GCH : (ch + 1) * GCH].copy()
            av[av < 0] = 0   # pad groups sum to ~0; row 0 is harmless
            sidx[ch] = _wrap16(av)



        per_core.append(dict(
            pos=pos, e_sel=e_sel, sj=sj[e_sel], sp_i=spec_pad[si[e_sel]],
            sp_j=spec_pad[sj[e_sel]], atom_of_G=atom_of_G,
            r_s=r_s, rcj_s=rcj_s, enj_s=enj_s, rci_s=rci_s, eni_s=eni_s,
            si_s=si_s, sj_s=sj_s, jl_s=jl_s, sidx=sidx,
            spec_slice=spec_pad[a0 : a0 + NA], chg_slice=chg_pad[a0 : a0 + NA],
        ))
    # pass-A sub-scatter structure: split ncoord into lo [0,SECB) / hi
    # [SECB,NA) tables so stage 2 can start before pass A finishes. Token
    # ranges are uniform across cores (SPMD); out-of-section tokens in the
    # overlap zone hit a trash row.
    asub = []      # per slot-chunk: list of (section, tok0, ntok, rowid)
    nrow = 0
    for ch in range(NCH):
        t1s, t0s = [], []
        for pc in per_core:
            av = pc["atom_of_G"][ch * GCH : (ch + 1) * GCH]
            lo = (av >= 0) & (av < SECB)
            hi = av >= SECB
            if lo.any():
                t1s.append(int(np.nonzero(lo)[0][-1]) + 1)
            if hi.any():
                t0s.append(int(np.nonzero(hi)[0][0]))
        subs = []
        if t1s:
            t1 = min((max(t1s) + 127) // 128 * 128, GCH)
            subs.append((0, 0, t1, nrow)); nrow += 1
        if t0s:
            t0 = min(t0s) // 128 * 128
            subs.append((1, t0, GCH - t0, nrow)); nrow += 1
        asub.append(subs)
    meta["asub"] = asub
    meta["NASC"] = nrow
    for pc in per_core:
        aidx = np.zeros((nrow, 16, GCH // 16), np.int16)
        for ch in range(NCH):
            av = pc["atom_of_G"][ch * GCH : (ch + 1) * GCH]
            for (sec, t0, ntok, row) in asub[ch]:
                iv = av[t0:t0 + ntok].copy()
                if sec == 0:
                    bad = ~((iv >= 0) & (iv < SECB))
                    iv[bad] = SECB
                else:
                    sel = iv >= SECB
                    iv = np.where(sel, iv - SECB, NA - SECB)
                aidx[row, :, :ntok // 16] = _wrap16(iv)
        pc["aidx"] = aidx
    return per_core, meta


def build_core_inputs(pc, meta, rcov, en, sqrt_r4r2):
    """Fill species-derived streams + wrapped idx arrays for one core."""
    SLOTS, NCH = meta["SLOTS"], meta["NCH"]
    pos = pc["pos"]
    pc["rcj_s"][pos] = rcov[pc["sp_j"]]
    pc["enj_s"][pos] = en[pc["sp_j"]]
    pc["rci_s"][pos] = rcov[pc["sp_i"]]
    pc["eni_s"][pos] = en[pc["sp_i"]]
    pc["si_s"][pos] = sqrt_r4r2[pc["sp_i"]]
    pc["sj_s"][pos] = sqrt_r4r2[pc["sp_j"]]

    # jidx16: one gather call per chunk, wrapped
    jl = pc["jl_s"]
    jw = np.zeros((NCH, 16, CALL // 16), np.int16)
    for k in range(NCH):
        jw[k] = _wrap16(jl[k * CALL : (k + 1) * CALL])

    # species wrap per atom chunk (2048 atoms): idx position u*128+p ->
    # local atom (16k+u)*128+p  (atom id = col*128 + p)
    spw = np.zeros((NACH, 16, (ACH * P) // 16), np.int16)
    spec = pc["spec_slice"].reshape(ACOLS, P)
    for k in range(NACH):
        lin = spec[k * ACH : (k + 1) * ACH, :].reshape(-1)  # [u, p] -> u*128+p
        spw[k] = _wrap16(lin)

    pa_pack = np.stack([
        pc["r_s"].reshape(NCH, CHSLOTS), pc["rcj_s"].reshape(NCH, CHSLOTS),
        pc["rci_s"].reshape(NCH, CHSLOTS)], axis=1)
    pa2_pack = np.stack([
        pc["enj_s"].reshape(NCH, CHSLOTS),
        pc["eni_s"].reshape(NCH, CHSLOTS)], axis=1)
    pb_pack = np.stack([
        pc["r_s"].reshape(NCH, CHSLOTS), pc["si_s"].reshape(NCH, CHSLOTS),
        pc["sj_s"].reshape(NCH, CHSLOTS)], axis=1)
    return dict(
        pa_pack=pa_pack, pa2_pack=pa2_pack, pb_pack=pb_pack,
        jw=jw.reshape(-1), spw=spw.reshape(-1), sidx=pc["sidx"].reshape(-1),
        aidx=pc["aidx"].reshape(-1),
        chg=np.ascontiguousarray(
            pc["chg_slice"].reshape(ACOLS, P).T).astype(np.float32),
    )


def _bc(ap, n):
    """Broadcast AP: append a step-0 inner dim of size n."""
    return bass.AP(tensor=ap.tensor, offset=ap.offset, ap=[*ap.ap, [0, n]])


def _dma_gather_raw(nc, out_ap, in_ap, idxs_ap, num_idxs, elem_size, elem_step):
    """dma_gather without the elem_size%256 restriction (payload < row pitch).
    Mirrors bass.BassGpSimd.dma_gather (non-transpose, DRAM source)."""
    eng = nc.gpsimd
    assert idxs_ap.dtype == mybir.dt.int16
    assert in_ap.dtype == out_ap.dtype
    stride_bytes = elem_step * mybir.dt.size(in_ap.dtype)
    assert stride_bytes % 256 == 0
    stride_bytes_256 = stride_bytes // 256
    assert in_ap.ap[0][0] == elem_step
    assert in_ap.ap[-1][1] == elem_size
    assert out_ap.ap[-1][1] == elem_size
    _in_ap = eng.lower_ap_dma(in_ap, for_custom_bir_dma=True)
    _idxs_ap = eng.lower_ap(idxs_ap)
    _out_ap = eng.lower_ap(out_ap)
    return eng.add_instruction(
        mybir.InstDMAGatherAnt(
            name=nc.get_next_instruction_name(),
            ins=[*_in_ap, _idxs_ap, eng.lower_val_access(eng.to_reg(num_idxs))],
            outs=[_out_ap],
            transpose=False,
            num_idxs=num_idxs,
            elem_size=elem_size,
            stride_bytes_256=stride_bytes_256,
            gen_mode=0,
            single_packet=True,
            queue_num=0,
            sbuf_tokens_per_rank=0,
            sbuf_free_dim_per_rank=0,
            sbuf_free_dim_pad_per_rank=0,
            sbuf_byte_offset=0,
        )
    )


def build_program(meta):
    SLOTS, NCH = meta["SLOTS"], meta["NCH"]
    ch_bucket = meta["ch_bucket"]
    A = mybir.AluOpType
    AF = mybir.ActivationFunctionType

    nc = bacc.Bacc(None, num_devices=NCORES, dynamic_dma_scratch_size=40960)

    def din(name, shape, dt=F32):
        return nc.dram_tensor(name, shape, dt, kind="ExternalInput")

    # per-slot streams, packed stream-major per chunk (one DMA per chunk)
    pa_d = din("pa_pack", [NCH, 3, CHSLOTS])
    pa2_d = din("pa2_pack", [NCH, 2, CHSLOTS], BF16)
    pb_d = din("pb_pack", [NCH, 3, CHSLOTS])
    jw_d = din("jw", [NCH, 16, CALL // 16], I16)
    spw_d = din("spw", [NACH, 16, (ACH * P) // 16], I16)
    sidx_d = din("sidx", [NCH, 16, GCH // 16], I16)
    aidx_d = din("aidx", [meta["NASC"], 16, GCH // 16], I16)
    chg_d = din("chg", [P, ACOLS])
    # all per-species tables concatenated into one upload (one DMA issue):
    # 0:7 zeff_r | 7:14 sscale_r | 14:21 gam_r | 21:28 refh | 28:35 ascale |
    # 35:42 hcount | 42:49 refq | 49:210 secaiw_r | 210:371 alphaiw |
    # 371 gam | 372 zeff | 373:408 cnw | 408:443 cn | 443:478 mask
    tab87_d = din("tab87", [Z, 478])
    cpw_d = din("cpw", [NW])
    s6_d = din("s6_raw", [1]); s8_d = din("s8_raw", [1])
    a1_d = din("a1_raw", [1]); a2_d = din("a2_raw", [1]); sq_d = din("scale_q_raw", [1])

    srowA_d = nc.dram_tensor("srowad", [Z, 64], F32)
    srowB_d = nc.dram_tensor("srowbd", [Z, 256], BF16)
    ncoL_d = nc.dram_tensor("ncold", [SECB + 16, BTW], F32)
    ncoH_d = nc.dram_tensor("ncohd", [NA - SECB + 16, BTW], F32)
    btab_d = nc.dram_tensor("btabd", [NA, 2 * BTW], BF16)
    t2sb_d = nc.dram_tensor("t2sb", [NA, NW], BF16)
    t2f_d = nc.dram_tensor("t2f", [NPAD, P], BF16, addr_space="Shared")
    e_d = nc.dram_tensor("e_out", [NA], F32, kind="ExternalOutput")

    def brc(dram, parts, width):
        """AP reading a [width] DRAM tensor broadcast across `parts` partitions."""
        return bass.AP(tensor=dram.tensor if hasattr(dram, "tensor") else dram,
                       offset=0, ap=[[0, parts], [1, width]])

    with tile.TileContext(nc) as tc:
        import contextlib
        with contextlib.ExitStack() as ctx:
            const = ctx.enter_context(tc.tile_pool(name="const", bufs=1))
            _srowcm = tc.tile_pool(name="srowp", bufs=1)
            srowp = _srowcm.__enter__()
            _wcm = tc.tile_pool(name="p0", bufs=2)
            work = _wcm.__enter__()

            nc.gpsimd.load_library(mlp_library)

            # dedicated index tiles: ucode reads only rows 0:16, so uploads
            # write just those rows; memset once here to satisfy init checks
            jwt_a = const.tile([P, CALL // 16], I16, tag="jwt0")
            jwt_b = const.tile([P, CALL // 16], I16, tag="jwt1")
            sxt_a = const.tile([P, GCH // 16], I16, tag="sxt0")
            sxt_b = const.tile([P, GCH // 16], I16, tag="sxt1")
            spwt = const.tile([P, (ACH * P) // 16], I16, tag="spwt")
            axt_a = const.tile([P, GCH // 16], I16, tag="axt0")
            axt_b = const.tile([P, GCH // 16], I16, tag="axt1")
            axt_c = const.tile([P, GCH // 16], I16, tag="axt2")
            axt_d = const.tile([P, GCH // 16], I16, tag="axt3")
            jwt2 = [jwt_a, jwt_b]
            sxt2 = [sxt_a, sxt_b]
            axt4 = [axt_a, axt_b, axt_c, axt_d]
            for t_ in (*jwt2, *sxt2, *axt4, spwt):
                nc.gpsimd.memset(t_[:], 0)

            b3_87 = const.tile([Z, 1], F32)
            nc.vector.memset(b3_87[:], 3.0)
            b3_p = const.tile([P, 1], F32)
            nc.vector.memset(b3_p[:], 3.0)
            bk5_p = const.tile([P, 1], F32)
            nc.vector.memset(bk5_p[:], K5)
            bkk_p = const.tile([P, 1], F32)
            nc.vector.memset(bkk_p[:], KK)

            # zero rows of nco/btab tables (only the columns we touch)
            zcol = const.tile([P, ACOLS, 1], F32)
            nc.vector.memset(zcol[:], 0.0)
            nc.sync.dma_start(
                out=ncoL_d[0:SECB].rearrange("(a p) f -> p a f", p=P)[:, :, 0:1],
                in_=zcol[:, 0:SECB // P, :])
            nc.sync.dma_start(
                out=ncoH_d[0:NA - SECB].rearrange("(a p) f -> p a f", p=P)[:, :, 0:1],
                in_=zcol[:, 0:(NA - SECB) // P, :])
            zrow23 = const.tile([P, ACOLS, NW], BF16)
            nc.vector.memset(zrow23[:], 0.0)

            # ---------- P0: per-species row table ----------
            tab = const.tile([Z, 478], F32)
            nc.sync.dma_start(out=tab[:], in_=tab87_d[:])
            t_ = tab[:]
            zeffr, sscr, gamr = t_[:, 0:7], t_[:, 7:14], t_[:, 14:21]
            refh, asc, hcnt = t_[:, 21:28], t_[:, 28:35], t_[:, 35:42]
            refq = t_[:, 42:49]
            secr, aiw = t_[:, 49:210], t_[:, 210:371]
            gam1, zeff1 = t_[:, 371:372], t_[:, 372:373]
            cnw, cnt_, msk = t_[:, 373:408], t_[:, 408:443], t_[:, 443:478]

            # softplus of all 5 scalar params in one Exp->Ln block
            params = const.tile([P, 5], F32)
            for ii, dd in enumerate([s6_d, s8_d, a1_d, a2_d, sq_d]):
                nc.sync.dma_start(out=params[:, ii:ii+1], in_=brc(dd, P, 1))
            nc.scalar.activation(out=params[:], in_=params[:], func=AF.Exp)
            nc.vector.tensor_scalar(out=params[:], in0=params[:], scalar1=1.0,
                                    scalar2=None, op0=A.add)
            nc.scalar.activation(out=params[:], in_=params[:], func=AF.Ln)
            s6p, s8p = params[:, 0:1], params[:, 1:2]
            a1p, a2p = params[:, 2:3], params[:, 3:4]
            spq = params[:, 4:5]
            sq87 = params[0:Z, 4:5]

            qmod = work.tile([Z, NREF], F32, tag="p0a")
            nc.vector.tensor_scalar(out=qmod[:], in0=refh, scalar1=sq87,
                                    scalar2=None, op0=A.mult)
            nc.vector.tensor_tensor(out=qmod[:], in0=qmod[:], in1=zeffr, op=A.add)
            qmsk = work.tile([Z, NREF], F32, tag="p0b")
            nc.vector.tensor_scalar(out=qmsk, in0=qmod[:], scalar1=1e-8,
                                    scalar2=None, op0=A.is_gt)
            qsafe = work.tile([Z, NREF], F32, tag="p0c")
            nc.vector.tensor_scalar(out=qsafe[:], in0=qmod[:], scalar1=1.0,
                                    scalar2=None, op0=A.subtract)
            nc.vector.tensor_tensor(out=qsafe[:], in0=qsafe[:], in1=qmsk,
                                    op=A.mult)
            nc.vector.tensor_scalar(out=qsafe[:], in0=qsafe[:], scalar1=1.0,
                                    scalar2=None, op0=A.add)
            rq = work.tile([Z, NREF], F32, tag="p0d")
            nc.vector.reciprocal(out=rq[:], in_=qsafe[:])
            t0 = work.tile([Z, NREF], F32, tag="p0e")
            nc.vector.tensor_tensor(out=t0[:], in0=zeffr, in1=rq[:], op=A.mult)
            nc.vector.tensor_tensor(out=t0[:], in0=t0[:], in1=gamr, op=A.mult)
            nc.vector.tensor_tensor(out=t0[:], in0=gamr, in1=t0[:], op=A.subtract)
            nc.scalar.activation(out=t0[:], in_=t0[:], func=AF.Exp, scale=2.0)
            nc.scalar.activation(out=t0[:], in_=t0[:], func=AF.Exp, scale=-3.0,
                                 bias=b3_87[:, 0:1])
            zfac = work.tile([Z, NREF], F32, tag="p0f")
            nc.vector.tensor_scalar(out=zfac[:], in0=t0[:], scalar1=E3,
                                    scalar2=None, op0=A.subtract)
            nc.vector.tensor_tensor(out=zfac[:], in0=zfac[:], in1=qmsk,
                                    op=A.mult)
            nc.vector.tensor_scalar(out=zfac[:], in0=zfac[:], scalar1=E3,
                                    scalar2=None, op0=A.add)
            al = work.tile([Z, NREF, NW], F32, tag="p0g")
            nc.vector.tensor_tensor(
                out=al[:], in0=secr.rearrange("z (a w) -> z a w", w=NW),
                in1=_bc(sscr, NW), op=A.mult)
            nc.vector.tensor_tensor(out=al[:], in0=al[:], in1=_bc(zfac[:], NW),
                                    op=A.mult)
            nc.vector.tensor_tensor(out=al[:], in0=al[:], in1=_bc(hcnt, NW),
                                    op=A.mult)
            nc.vector.tensor_tensor(
                out=al[:], in0=aiw.rearrange("z (a w) -> z a w", w=NW),
                in1=al[:], op=A.subtract)
            nc.vector.tensor_tensor(out=al[:], in0=al[:], in1=_bc(asc, NW),
                                    op=A.mult)
            nc.vector.tensor_scalar(out=al[:], in0=al[:], scalar1=0.0,
                                    scalar2=None, op0=A.max)
            cpw87 = const.tile([Z, NW], F32)
            nc.sync.dma_start(out=cpw87[:], in_=brc(cpw_d, Z, NW))
            nc.scalar.activation(out=cpw87[:], in_=cpw87[:], func=AF.Sqrt,
                                 scale=CPFAC)
            wb = bass.AP(tensor=cpw87[:].tensor, offset=cpw87[:].offset,
                         ap=[cpw87[:].ap[0], [0, NREF], [1, NW]])
            nc.vector.tensor_tensor(out=al[:], in0=al[:], in1=wb, op=A.mult)

            # assemble split species rows: f32 part (gam, zeff, refq, cn)
            # + bf16 part (atil, cnw, mask)
            srow = const.tile([Z, 64], F32)
            nc.vector.memset(srow[:], 0.0)
            nc.vector.tensor_copy(out=srow[:, 0:1], in_=gam1)
            nc.vector.tensor_copy(out=srow[:, 1:2], in_=zeff1)
            nc.vector.tensor_copy(out=srow[:, 2:9], in_=refq)
            nc.vector.tensor_copy(out=srow[:, 9:44], in_=cnt_)
            nc.sync.dma_start(out=srowA_d[:], in_=srow[:])
            srowb = const.tile([Z, 256], BF16)
            nc.gpsimd.memset(srowb[:], 0)
            nc.scalar.activation(out=srowb[:, 0:161],
                                 in_=al[:].rearrange("z a w -> z (a w)"),
                                 func=AF.Copy)
            nc.scalar.activation(out=srowb[:, 161:196], in_=cnw, func=AF.Copy)
            nc.scalar.activation(out=srowb[:, 196:231], in_=msk, func=AF.Copy)
            nc.sync.dma_start(out=srowB_d[:], in_=srowb[:])


            _wcm.__exit__(None, None, None)
            _wcm = tc.tile_pool(name="pA", bufs=3)
            work = _wcm.__enter__()

            # ---------- P1: pass A (coordination numbers) ----------
            def ldsidx(c, pool, tag):
                t = sxt2[c % 2]
                nc.sync.dma_start(out=t[0:16, :], in_=sidx_d[c])
                return t

            def scatter_add(out_ap, in_ap, idxs_t, num, elem, step):
                return nc.gpsimd.dma_scatter_add(
                    out_ap, in_ap, idxs_t, num, num, elem, elem_step=step)

            srow_t = []
            asub = meta["asub"]

            def emit_passA(c, pos_):
                st = work.tile([P, 3, TCH], F32, tag="a_st")
                nc.sync.dma_start(
                    out=st[:], in_=pa_d[c].rearrange("v (t p) -> p v t", p=P))
                st2 = work.tile([P, 2, TCH], BF16, tag="a_st2")
                nc.sync.dma_start(
                    out=st2[:], in_=pa2_d[c].rearrange("v (t p) -> p v t", p=P))
                r_t, rcj, rci = st[:, 0, :], st[:, 1, :], st[:, 2, :]
                enj, eni = st2[:, 0, :], st2[:, 1, :]
                # interleave stage-2 srow prefetch gathers (needed only by
                # stage 2) so their DMA doesn't starve pass-A stream loads
                if (pos_ % 2 == 1 or pos_ == NCH - 1) and len(srow_t) < NACH:
                    k = len(srow_t)
                    spw_t = spwt
                    nc.sync.dma_start(out=spw_t[0:16, :], in_=spw_d[k])
                    sg = srowp.tile([P, ACH, 44], F32, tag=f"pf_srow{k}")
                    _dma_gather_raw(nc, sg[:], srowA_d[:, 0:44], spw_t[:],
                                    ACH * P, 44, 64)
                    sgb = srowp.tile([P, ACH, 231], BF16, tag=f"pf_srowb{k}")
                    _dma_gather_raw(nc, sgb[:], srowB_d[:, 0:231], spw_t[:],
                                    ACH * P, 231, 256)
                    srow_t.append((sg, sgb))
                # rcv = 4/3*(rci+rcj)
                rcv = work.tile([P, TCH], F32, tag="a_rcv")
                nc.vector.tensor_tensor(out=rcv[:], in0=rci, in1=rcj, op=A.add)
                nc.vector.tensor_scalar(out=rcv[:], in0=rcv[:], scalar1=4.0 / 3.0,
                                        scalar2=None, op0=A.mult)
                # den = K4*exp(-((|eni-enj|+K5)^2)/K6) via sigmoid identity:
                # exp(-v) = 1/sigmoid(v) - 1
                den = work.tile([P, TCH], F32, tag="a_den")
                nc.vector.tensor_tensor(out=den[:], in0=eni, in1=enj,
                                        op=A.subtract)
                nc.scalar.activation(out=den[:], in_=den[:], func=AF.Abs)
                nc.scalar.activation(out=den[:], in_=den[:], func=AF.Square,
                                     bias=bk5_p[:, 0:1])
                nc.scalar.activation(out=den[:], in_=den[:], func=AF.Sigmoid,
                                     scale=1.0 / K6)
                nc.vector.reciprocal(out=den[:], in_=den[:])
                nc.vector.tensor_scalar(out=den[:], in0=den[:], scalar1=1.0,
                                        scalar2=0.5 * K4, op0=A.subtract,
                                        op1=A.mult)
                # erf(-KK*(rr-rcv)/rcv) = Erf(-KK*u + KK), u = rr/rcv
                cf = work.tile([P, TCH], F32, tag="a_cf")
                nc.vector.reciprocal(out=cf[:], in_=rcv[:])
                nc.vector.tensor_tensor(out=cf[:], in0=cf[:], in1=r_t, op=A.mult)
                nc.scalar.activation(out=cf[:], in_=cf[:], func=AF.Erf,
                                     scale=-KK / BOHR, bias=bkk_p[:, 0:1])
                # countf = (erf + 1) * den_scaled
                nc.vector.scalar_tensor_tensor(out=cf[:], in0=cf[:], scalar=1.0,
                                               in1=den[:], op0=A.add, op1=A.mult)
                # tree reduce GS -> 1
                l1 = work.tile([P, TCH // 2], F32, tag="a_l1")
                v = cf[:].rearrange("p (a two) -> p a two", two=2)
                nc.vector.tensor_tensor(out=l1[:], in0=v[:, :, 0], in1=v[:, :, 1],
                                        op=A.add)
                l2 = work.tile([P, TPG, 1], F32, tag="a_l2")
                v = l1[:].rearrange("p (a two) -> p a two", two=2)
                nc.vector.tensor_tensor(out=l2[:, :, 0],
                                        in0=v[:, :, 0], in1=v[:, :, 1],
                                        op=A.add)
                # scatter-add group sums into lo/hi per-atom ncoord tables
                for (sec, t0, ntok, row) in asub[c]:
                    it = axt4[(pos_ * 2 + sec) % 4]
                    nc.sync.dma_start(out=it[0:16, 0:ntok // 16],
                                      in_=aidx_d[row][:, 0:ntok // 16])
                    tab = ncoL_d if sec == 0 else ncoH_d
                    scatter_add(tab[:, 0:1],
                                l2[:, t0 // 128:(t0 + ntok) // 128, :],
                                it[:, 0:ntok // 16], ntok, 1, BTW)

            _s2cm = tc.tile_pool(name="pS2", bufs=2)
            works2 = _s2cm.__enter__()

            # ---------- P2: stage 2 (per-atom A~ rows) ----------
            def emit_s2(k):
                work = works2
                sr = srow_t[k][0][:]
                srb = srow_t[k][1][:]
                nco = work.tile([P, ACH, 1], F32, tag="s2_nco")
                if k * ACH < SECB // P:
                    nsrc = ncoL_d[0:SECB].rearrange("(a p) f -> p a f", p=P)[
                        :, k * ACH:(k + 1) * ACH, 0:1]
                else:
                    k2 = k - SECB // P // ACH
                    nsrc = ncoH_d[0:NA - SECB].rearrange(
                        "(a p) f -> p a f", p=P)[
                        :, k2 * ACH:(k2 + 1) * ACH, 0:1]
                nc.sync.dma_start(out=nco[:], in_=nsrc)

                # gaussian weights gw[P, ACH, NREF]
                gw35 = work.tile([P, ACH, NREF * NC], F32, tag="s2_gw35")
                nc.vector.tensor_tensor(out=gw35[:],
                                        in0=_bc(nco[:, :, 0], NREF * NC),
                                        in1=sr[:, :, 9:44], op=A.subtract)
                nc.vector.tensor_tensor(out=gw35[:], in0=gw35[:], in1=gw35[:],
                                        op=A.mult)
                nc.vector.tensor_tensor(out=gw35[:], in0=gw35[:],
                                        in1=srb[:, :, 161:196], op=A.mult)
                nc.scalar.activation(out=gw35[:], in_=gw35[:], func=AF.Exp,
                                     scale=-6.0)
                nc.vector.tensor_tensor(out=gw35[:], in0=gw35[:],
                                        in1=srb[:, :, 196:231], op=A.mult)
                gw = work.tile([P, ACH, NREF], F32, tag="s2_gw")
                g5 = gw35[:].rearrange("p c (a n) -> p c a n", n=NC)
                nc.vector.tensor_tensor(out=gw[:], in0=g5[:, :, :, 0],
                                        in1=g5[:, :, :, 1], op=A.add)
                for n5 in range(2, NC):
                    nc.vector.tensor_tensor(out=gw[:], in0=gw[:],
                                            in1=g5[:, :, :, n5], op=A.add)
                nrm = work.tile([P, ACH], F32, tag="s2_nrm")
                nc.vector.tensor_reduce(out=nrm[:], in_=gw[:],
                                        axis=mybir.AxisListType.X, op=A.add)
                nc.vector.tensor_scalar(out=nrm[:], in0=nrm[:], scalar1=1e-7,
                                        scalar2=None, op0=A.max)
                nc.vector.reciprocal(out=nrm[:], in_=nrm[:])
                nc.vector.tensor_tensor(out=gw[:], in0=gw[:], in1=_bc(nrm[:], NREF),
                                        op=A.mult)
                # zeta
                chg_t = work.tile([P, ACH], F32, tag="s2_chg")
                nc.sync.dma_start(out=chg_t[:], in_=chg_d[:, k * ACH:(k + 1) * ACH])
                qmod2 = work.tile([P, ACH], F32, tag="s2_qm")
                nc.vector.tensor_tensor(out=qmod2[:], in0=chg_t[:],
                                        in1=sr[:, :, 1], op=A.add)
                msk2 = work.tile([P, ACH], F32, tag="s2_msk")
                nc.vector.tensor_scalar(out=msk2[:], in0=qmod2[:], scalar1=1e-8,
                                        scalar2=None, op0=A.is_gt)
                qs2 = work.tile([P, ACH], F32, tag="s2_qs")
                nc.vector.tensor_scalar(out=qs2[:], in0=qmod2[:], scalar1=1.0,
                                        scalar2=None, op0=A.subtract)
                nc.vector.tensor_tensor(out=qs2[:], in0=qs2[:], in1=msk2[:],
                                        op=A.mult)
                nc.vector.tensor_scalar(out=qs2[:], in0=qs2[:], scalar1=1.0,
                                        scalar2=None, op0=A.add)
                nc.vector.reciprocal(out=qs2[:], in_=qs2[:])
                zt = work.tile([P, ACH, NREF], F32, tag="s2_zt")
                nc.vector.tensor_scalar(out=zt[:], in0=sr[:, :, 2:9],
                                        scalar1=spq, scalar2=None,
                                        op0=A.mult)
                nc.vector.tensor_tensor(out=zt[:], in0=zt[:],
                                        in1=_bc(sr[:, :, 1], NREF), op=A.add)
                nc.vector.tensor_tensor(out=zt[:], in0=zt[:],
                                        in1=_bc(qs2[:], NREF), op=A.mult)
                nc.vector.tensor_tensor(out=zt[:], in0=zt[:],
                                        in1=_bc(sr[:, :, 0], NREF), op=A.mult)
                nc.vector.tensor_tensor(out=zt[:], in0=_bc(sr[:, :, 0], NREF),
                                        in1=zt[:], op=A.subtract)
                nc.scalar.activation(out=zt[:], in_=zt[:], func=AF.Exp, scale=2.0)
                nc.scalar.activation(out=zt[:], in_=zt[:], func=AF.Exp,
                                     scale=-3.0, bias=b3_p[:, 0:1])
                zeta = work.tile([P, ACH, NREF], F32, tag="s2_zeta")
                mb = bass.AP(tensor=msk2[:].tensor, offset=msk2[:].offset,
                             ap=[*msk2[:].ap, [0, NREF]])
                nc.vector.tensor_scalar(out=zeta[:], in0=zt[:], scalar1=E3,
                                        scalar2=None, op0=A.subtract)
                nc.vector.tensor_tensor(out=zeta[:], in0=zeta[:], in1=mb,
                                        op=A.mult)
                nc.vector.tensor_scalar(out=zeta[:], in0=zeta[:], scalar1=E3,
                                        scalar2=None, op0=A.add)
                nc.vector.tensor_tensor(out=zeta[:], in0=zeta[:], in1=gw[:],
                                        op=A.mult)
                # A~_i[w] = sum_a zeta[a]*atil[a,w]; replicate zeta over w
                # and cast atil to bf16 on the Activation engine so the DVE
                # mult/add chain runs in the packed-bf16 2x mode
                zrep = work.tile([P, ACH, NREF, NW], BF16, tag="s2_zrep")
                nc.scalar.activation(out=zrep[:], in_=_bc(zeta[:], NW),
                                     func=AF.Copy)
                av = srb[:, :, 0:161].rearrange("p c (a w) -> p c a w", w=NW)
                t2row = work.tile([P, ACH, NW], BF16, tag="s2_t2row")
                for a_ in range(NREF):
                    if a_ == 0:
                        nc.vector.tensor_tensor(
                            out=t2row[:], in0=av[:, :, 0, :],
                            in1=zrep[:, :, 0, :], op=A.mult)
                    else:
                        tmp_ = work.tile([P, ACH, NW], BF16, tag="s2_tmp")
                        nc.vector.tensor_tensor(
                            out=tmp_[:], in0=av[:, :, a_, :],
                            in1=zrep[:, :, a_, :], op=A.mult)
                        nc.vector.tensor_tensor(out=t2row[:], in0=t2row[:],
                                                in1=tmp_[:], op=A.add)
                nc.sync.dma_start(
                    out=t2sb_d.rearrange("(a p) w -> p a w", p=P)[
                        :, k * ACH:(k + 1) * ACH, :],
                    in_=t2row[:])

            # ---------- driver: pass A round-robin with interleaved s2 ----
            cpb = [meta["NGBS"][b] * GS // CHSLOTS for b in range(NBUCK)]
            coff = [0]
            for b in range(NBUCK):
                coff.append(coff[-1] + cpb[b])
            order = []
            for cc in range(max(cpb)):
                for b in range(NBUCK):
                    if cc < cpb[b]:
                        order.append(coff[b] + cc)
            lo_done_pos = max(
                pos_ for pos_, c in enumerate(order)
                if any(s[0] == 0 for s in meta["asub"][c]))
            for pos_, c in enumerate(order):
                emit_passA(c, pos_)
                if pos_ == lo_done_pos:
                    for k_ in range(SECB // P // ACH):
                        emit_s2(k_)
            for k_ in range(SECB // P // ACH, NACH):
                emit_s2(k_)

            _s2cm.__exit__(None, None, None)
            _wcm.__exit__(None, None, None)
            _srowcm.__exit__(None, None, None)

            # zero the B table now — its only consumers are the pass-B
            # scatter-adds, so this write rides the idle DMA window here
            nc.sync.dma_start(
                out=btab_d[0:NA].rearrange("(a p) f -> p a f", p=P)[:, :, 0:NW],
                in_=zrow23[:])

            # ---------- P3: AllGather packed bf16 A~ rows into strided table --
            # (emitted before pass B1 in program order so the Pool engine
            # starts the collective while the DVE computes damping factors)
            nc.gpsimd.collective_compute(
                "AllGather", A.bypass,
                replica_groups=[list(range(NCORES))],
                ins=[t2sb_d[:]], outs=[t2f_d[:, 0:NW]])

            _wcm = tc.tile_pool(name="pB", bufs=3)
            work = _wcm.__enter__()

            # ---------- P4a: pass B1 — damping factors (overlaps AllGather) --
            dbts = []
            for c in range(NCH):
                sb = work.tile([P, 3, TCH], F32, tag="b_sb")
                nc.sync.dma_start(
                    out=sb[:], in_=pb_d[c].rearrange("v (t p) -> p v t", p=P))
                r_t, si_t, sj_t = sb[:, 0, :], sb[:, 1, :], sb[:, 2, :]
                r2 = work.tile([P, TCH], F32, tag="b_r2")
                nc.scalar.activation(out=r2[:], in_=r_t, func=AF.Square,
                                     scale=1.0 / BOHR)
                r4 = work.tile([P, TCH], F32, tag="b_r4")
                nc.scalar.activation(out=r4[:], in_=r2[:], func=AF.Square)
                r8 = work.tile([P, TCH], F32, tag="b_r8")
                nc.scalar.activation(out=r8[:], in_=r4[:], func=AF.Square)
                r6 = work.tile([P, TCH], F32, tag="b_r6")
                nc.vector.tensor_tensor(out=r6[:], in0=r4[:], in1=r2[:], op=A.mult)
                R3 = work.tile([P, TCH], F32, tag="b_R3")
                nc.vector.scalar_tensor_tensor(out=R3[:], in0=si_t, scalar=3.0,
                                               in1=sj_t, op0=A.mult,
                                               op1=A.mult)
                r0 = work.tile([P, TCH], F32, tag="b_r0")
                nc.scalar.activation(out=r0[:], in_=R3[:], func=AF.Sqrt)
                nc.vector.tensor_scalar(out=r0[:], in0=r0[:], scalar1=a1p,
                                        scalar2=a2p, op0=A.mult, op1=A.add)
                q2 = work.tile([P, TCH], F32, tag="b_q2")
                nc.scalar.activation(out=q2[:], in_=r0[:], func=AF.Square)
                c4 = work.tile([P, TCH], F32, tag="b_c4")
                nc.scalar.activation(out=c4[:], in_=q2[:], func=AF.Square)
                c3 = work.tile([P, TCH], F32, tag="b_c3")
                nc.vector.tensor_tensor(out=c3[:], in0=c4[:], in1=q2[:], op=A.mult)
                c8 = work.tile([P, TCH], F32, tag="b_c8")
                nc.scalar.activation(out=c8[:], in_=c4[:], func=AF.Square)
                d6 = work.tile([P, TCH], F32, tag="b_d6")
                nc.vector.tensor_tensor(out=d6[:], in0=r6[:], in1=c3[:], op=A.add)
                nc.vector.reciprocal(out=d6[:], in_=d6[:])
                d8 = work.tile([P, TCH], F32, tag="b_d8")
                nc.vector.tensor_tensor(out=d8[:], in0=r8[:], in1=c8[:], op=A.add)
                nc.vector.reciprocal(out=d8[:], in_=d8[:])
                nc.vector.tensor_tensor(out=d8[:], in0=d8[:], in1=R3[:], op=A.mult)
                nc.vector.tensor_scalar(out=d8[:], in0=d8[:], scalar1=s8p,
                                        scalar2=None, op0=A.mult)
                d6c = const.tile([P, TCH], F32, tag=f"b_d6_{c}")
                nc.vector.scalar_tensor_tensor(out=d6c[:], in0=d6[:], scalar=s6p,
                                               in1=d8[:], op0=A.mult, op1=A.add)
                dbts.append(d6c)

            # ---------- P4b: pass B2 (gather + scale + reduce + scatter) -----
            cpb = [meta["NGBS"][b] * GS // CHSLOTS for b in range(NBUCK)]
            coff = [0]
            for b in range(NBUCK):
                coff.append(coff[-1] + cpb[b])
            order = []
            for cc in range(max(cpb)):
                for b in range(NBUCK):
                    if cc < cpb[b]:
                        order.append(coff[b] + cc)
            lo_done_pos = max(
                pos_ for pos_, c in enumerate(order)
                if any(s[0] == 0 for s in meta["asub"][c]))

            def emit_passB2(c, pos_):
                b = ch_bucket[c]
                sidx_t = ldsidx(c, work, "b_sidx")
                jw_t = jwt2[pos_ % 2]
                nc.sync.dma_start(out=jw_t[0:16, :], in_=jw_d[c])
                gt = work.tile([P, TCH, NW], BF16, tag="b_g")
                _dma_gather_raw(
                    nc, gt[:],
                    t2f_d[BBASE[b]:BBASE[b] + BSIZE[b], 0:NW],
                    jw_t[:], CALL, NW, P)
                d6c = dbts[c]
                # replicate D over the 23 w-columns on the Activation engine
                # (bf16 cast + broadcast), keeping the DVE mult in 2x mode
                db = bass.AP(tensor=d6c[:].tensor, offset=d6c[:].offset,
                             ap=[*d6c[:].ap, [0, NW]])
                drep = work.tile([P, TCH, NW], BF16, tag="b_drep")
                nc.scalar.activation(out=drep[:], in_=db, func=AF.Copy)
                nc.vector.tensor_tensor(out=gt[:], in0=gt[:], in1=drep[:],
                                        op=A.mult)
                # tree reduce over GS slots
                m1 = work.tile([P, TCH // 2, NW], BF16, tag="b_m1")
                v = gt[:].rearrange("p (a two) f -> p a two f", two=2)
                nc.vector.tensor_tensor(out=m1[:], in0=v[:, :, 0, :],
                                        in1=v[:, :, 1, :], op=A.add)
                m2 = work.tile([P, TPG, NW], BF16, tag="b_m2")
                v = m1[:].rearrange("p (a two) f -> p a two f", two=2)
                nc.vector.tensor_tensor(out=m2[:], in0=v[:, :, 0, :],
                                        in1=v[:, :, 1, :], op=A.add)
                # scatter-add group rows into the per-atom B table
                scatter_add(btab_d[:, 0:NW], m2[:], sidx_t[:], GCH, NW, 2 * BTW)

            _ecm = tc.tile_pool(name="pE", bufs=3)
            worke = _ecm.__enter__()

            # ---------- P5: assemble E ----------
            def emit_p5(k):
                work = worke
                bsum = work.tile([P, ACH, NW], BF16, tag="e_bsum")
                nc.sync.dma_start(
                    out=bsum[:],
                    in_=btab_d[0:NA].rearrange("(a p) f -> p a f", p=P)[
                        :, k * ACH:(k + 1) * ACH, 0:NW])
                ai = work.tile([P, ACH, NW], BF16, tag="e_ai")
                nc.sync.dma_start(
                    out=ai[:],
                    in_=t2sb_d.rearrange("(a p) w -> p a w", p=P)[
                        :, k * ACH:(k + 1) * ACH, :])
                prod = work.tile([P, ACH, NW], F32, tag="e_prod")
                nc.vector.tensor_tensor(out=prod[:], in0=ai[:],
                                        in1=bsum[:], op=A.mult)
                ev = work.tile([P, ACH], F32, tag="e_ev")
                nc.vector.tensor_reduce(out=ev[:], in_=prod[:],
                                        axis=mybir.AxisListType.X, op=A.add)
                nc.vector.tensor_scalar(out=ev[:], in0=ev[:],
                                        scalar1=-0.5 * HARTREE, scalar2=None,
                                        op0=A.mult)
                nc.sync.dma_start(
                    out=e_d.rearrange("(a p) -> p a", p=P)[:, k * ACH:(k + 1) * ACH],
                    in_=ev[:])

            for c in range(NCH):
                emit_passB2(c, c)
            for k_ in range(NACH):
                emit_p5(k_)

            _ecm.__exit__(None, None, None)
            _wcm.__exit__(None, None, None)
    return nc


_PROG_CACHE = {}


def kernel(**inputs):
    species = np.asarray(inputs["species"])
    per_core, meta = preprocess(species, inputs["edge_index"],
                                inputs["lengths"], inputs["partial_charges"])
    rcov = np.asarray(inputs["rcov"], np.float32)
    en = np.asarray(inputs["en"], np.float32)
    sr4 = np.asarray(inputs["sqrt_r4r2"], np.float32)
    refsys = np.asarray(inputs["refsys"]).astype(np.int64)

    # refsys-expanded tables (pure host-side permutation of inputs)
    zeff = np.asarray(inputs["zeff"], np.float32)
    sscale = np.asarray(inputs["sscale"], np.float32)
    gam = np.asarray(inputs["gam"], np.float32)
    secaiw = np.asarray(inputs["secaiw"], np.float32)
    zeff_r = zeff[refsys]
    sscale_r = sscale[refsys]
    gam_r = gam[refsys]
    secaiw_r = secaiw[refsys].reshape(Z, NREF * NW)

    import os as _os
    _bedrock = _os.environ.get("BEDROCK") == "1"
    if not _bedrock:
        key = (tuple(meta["NGBS"]),
               tuple(tuple(s) for ss in meta["asub"] for s in ss))
        if key not in _PROG_CACHE:
            nc = build_program(meta)
            nc.finalize()
            _PROG_CACHE[key] = nc
        nc = _PROG_CACHE[key]

    tab87 = np.concatenate([
        zeff_r, sscale_r, gam_r,
        np.asarray(inputs["refh"], np.float32),
        np.asarray(inputs["ascale"], np.float32),
        np.asarray(inputs["hcount"], np.float32),
        np.asarray(inputs["refq"], np.float32),
        secaiw_r,
        np.asarray(inputs["alphaiw"], np.float32).reshape(Z, NREF * NW),
        gam[:, None], zeff[:, None],
        np.asarray(inputs["ncount_weight"], np.float32).reshape(Z, -1),
        np.asarray(inputs["cn"], np.float32).reshape(Z, -1),
        np.asarray(inputs["ncount_mask"], np.float32).reshape(Z, -1),
    ], axis=1).astype(np.float32)
    shared = dict(
        tab87=tab87,
        cpw=np.asarray(inputs["cpw"], np.float32),
        s6_raw=np.asarray(inputs["s6_raw"], np.float32),
        s8_raw=np.asarray(inputs["s8_raw"], np.float32),
        a1_raw=np.asarray(inputs["a1_raw"], np.float32),
        a2_raw=np.asarray(inputs["a2_raw"], np.float32),
        scale_q_raw=np.asarray(inputs["scale_q_raw"], np.float32),
    )
    in_maps = []
    for c in range(NCORES):
        ci = build_core_inputs(per_core[c], meta, rcov, en, sr4)
        m = dict(shared)
        m.update(
            pa_pack=ci["pa_pack"], pa2_pack=ci["pa2_pack"],
            pb_pack=ci["pb_pack"],
            jw=ci["jw"].reshape(meta["NCH"], 16, CALL // 16),
            spw=ci["spw"].reshape(NACH, 16, (ACH * P) // 16),
            sidx=ci["sidx"].reshape(meta["NCH"], 16, GCH // 16),
            aidx=ci["aidx"].reshape(meta["NASC"], 16, GCH // 16),
            chg=ci["chg"],
        )
        in_maps.append(m)

    if _bedrock:
        outs = _sim_fallback(build_program(meta), in_maps)
    else:
        try:
            from concourse.bass_utils import run_bass_kernel_spmd
            res = run_bass_kernel_spmd(nc, in_maps, list(range(NCORES)))
            outs = [res.results[c]["e_out"] for c in range(NCORES)]
        except Exception:
            outs = _sim_fallback(build_program(meta), in_maps)
    e = np.concatenate(outs)
    return e[: species.shape[0]].astype(np.float32)


def _sim_fallback(nc, in_maps):
    import inspect
    import textwrap
    from scipy.special import erf as _scipy_erf
    from concourse import bass_interp
    src = textwrap.dedent(inspect.getsource(
        bass_interp.InstructionExecutor.visit_InstActivation))
    if "_scipy_erf" not in src:
        pat = ("    else:\n"
               "        # NOTE: If you are adding a new activation instruction")
        rep = ("    elif instruction.func == mb.ActivationFunctionType.Erf:\n"
               "        acted = _scipy_erf(scaled_and_biased)\n"
               "    else:\n"
               "        # NOTE: If you are adding a new activation instruction")
        assert pat in src
        src = src.replace(pat, rep)
        ns = dict(bass_interp.__dict__)
        ns["_scipy_erf"] = _scipy_erf
        exec(compile(src, "<erfpatch>", "exec"), ns)
        bass_interp.InstructionExecutor.visit_InstActivation = ns[
            "visit_InstActivation"]
    sim = bass_interp.MultiCoreSim(nc, NCORES, num_workers=1)
    for c in range(NCORES):
        for k, v in in_maps[c].items():
            sim.cores[c].tensor(k)[:] = v
    sim.simulate()
    global LAST_EXEC_TIME_NS
    LAST_EXEC_TIME_NS = int(getattr(sim, "global_time", 0))
    return [np.array(sim.cores[c].tensor("e_out")) for c in range(NCORES)]


LAST_EXEC_TIME_NS = None


# revision 69
# speedup vs baseline: 1.2325x; 1.0214x over previous
"""D4 dispersion energy kernel for 8 Trainium2 NeuronCores.

Strategy (v2):
- Host (numpy, integer/permutation work only): sort the edge list by (dst
  atom, j-range bucket), pad each (atom,bucket) edge run to a multiple of 8
  ("groups"), lay slots out in a fixed chunk/call/partition grid, and
  pre-permute all per-edge input data into that slot order.
- Device (all float math):
  * pass A computes per-edge coordination-number contributions and
    tree-reduces them into group sums, then dma_scatter_add's the group sums
    directly into a dense per-atom ncoord table;
  * stage 2 computes per-atom Gaussian weights / zeta / effective alpha
    table A~ from this core's atom slice (bf16, packed 23 floats/atom);
  * one AllGather shares the packed bf16 A~ rows into a 256B-stride table;
  * pass B gathers A~ rows for edge sources via dma_gather (bf16, 46B
    payload), applies Becke-Johnson damping, tree-reduces into group rows and
    dma_scatter_add's them into a dense per-atom B table;
  * E_i = -0.5*HARTREE * <A~_i, B_i>.
"""
import math
import numpy as np

import concourse.bass as bass
import concourse.bacc as bacc
import concourse.tile as tile
from concourse import mybir
from concourse.library_config import mlp as mlp_library

F32 = mybir.dt.float32
BF16 = mybir.dt.bfloat16
I16 = mybir.dt.int16

Z = 87
NREF = 7
NC = 5
NW = 23
BOHR = 0.5291772105638411
HARTREE = 27.211386024367243
K4, K5, K6, KK = 4.10451, 19.08857, 254.5553148552, 7.5
E3 = float(np.exp(3.0))
CPFAC = 3.0 / (2.0 * np.pi)

NCORES = 8
P = 128
ACOLS = 80              # atom columns per partition -> NA = 128*80
NA = P * ACOLS          # atoms per core (10240)
NPAD = NCORES * NA      # padded atom count (81920)
ACH = 16                # atom columns per stage-2 chunk (2048 atoms)
NACH = ACOLS // ACH     # atom chunks per core (5)
CALL = 32768            # idxs per dma_gather call (one per chunk)
TCH = 256               # slots per partition per compute chunk
GS = 4                  # slots per group
CHSLOTS = P * TCH       # slots per compute chunk (32768)
GCH = CHSLOTS // GS     # groups per chunk (4096)
TPG = TCH // GS         # group cells per partition per chunk (32)

# j-range buckets (dma_gather idx is int16)
NBUCK = 3
BBASE = [0, 27307, 54614]
BSIZE = [27307, 27307, NPAD - 54614]

SECB = 6144             # ncoord lo/hi section boundary (atoms, 3 s2 chunks)
SROWW = 320             # per-species row width (f32); 1280 B, 256-aligned
BTW = 64                # per-atom table row width (f32); 256 B stride


def _wrap16(idx_lin):
    """int linear idx list -> [16, ceil(n/16)] int16 wrapped tile.

    The gather/scatter ucode reads indices from the first 16 partitions
    only, so the upload carries just those rows (the SBUF tile is still
    128 partitions tall; rows 16-127 are never read)."""
    n = len(idx_lin)
    m = (n + 15) // 16
    pad = np.zeros(m * 16, np.int16)
    pad[:n] = idx_lin.astype(np.int16)
    return np.ascontiguousarray(pad.reshape(m, 16).T)  # [16, m]


def preprocess(species, edge_index, lengths, partial_charges):
    """Build per-core host-side data. Returns (per_core list of dicts, meta)."""
    n_at = species.shape[0]
    species = np.asarray(species).astype(np.int32)
    idx_i = np.asarray(edge_index[0]).astype(np.int64)
    idx_j = np.asarray(edge_index[1]).astype(np.int64)
    lengths = np.asarray(lengths).astype(np.float32)
    charges = np.asarray(partial_charges).astype(np.float32)

    spec_pad = np.zeros(NPAD, np.int32)
    spec_pad[:n_at] = species
    chg_pad = np.zeros(NPAD, np.float32)
    chg_pad[:n_at] = charges

    # bucket of each edge by j range
    jb = np.searchsorted(np.array(BBASE[1:]), idx_j, side="right")  # 0..2
    key = idx_i * NBUCK + jb
    order = np.argsort(key, kind="stable")
    si = idx_i[order]
    sj = idx_j[order]
    sl = lengths[order]
    sjb = jb[order]

    # count edges per (atom, bucket)
    cnt = np.bincount(idx_i * NBUCK + jb, minlength=NPAD * NBUCK).reshape(NPAD, NBUCK)
    grp = (cnt + GS - 1) // GS  # groups per (atom,bucket)
    # CSR offsets into sorted edge array for (atom,bucket)
    flat_cnt = cnt.reshape(-1)
    edge_off = np.zeros(NPAD * NBUCK + 1, np.int64)
    np.cumsum(flat_cnt, out=edge_off[1:])

    # group quota per bucket (max over cores, rounded to chunk multiple)
    grp_cb = grp.reshape(NCORES, NA, NBUCK).sum(axis=1)  # [core, bucket]
    NGBS = []
    for b in range(NBUCK):
        m = int(grp_cb[:, b].max())
        NGBS.append(((m + GCH - 1) // GCH) * GCH)
    NG = sum(NGBS)                       # groups per core
    SLOTS = NG * GS                      # slots per core
    NCH = SLOTS // CHSLOTS               # compute chunks
    assert SLOTS % CHSLOTS == 0
    # chunk -> bucket map (buckets are whole chunks)
    ch_bucket = []
    for b in range(NBUCK):
        ch_bucket += [b] * (NGBS[b] * GS // CHSLOTS)
    gb_off = np.concatenate([[0], np.cumsum(NGBS)])  # group offset per bucket

    meta = dict(NGBS=NGBS, NG=NG, SLOTS=SLOTS, NCH=NCH, ch_bucket=ch_bucket)

    per_core = []
    for c in range(NCORES):
        a0 = c * NA
        g_c = grp[a0 : a0 + NA]                 # [NA, NBUCK]
        gofs = np.zeros((NA + 1, NBUCK), np.int64)
        np.cumsum(g_c, axis=0, out=gofs[1:])
        ng_b = gofs[NA]                          # real groups per bucket
        for b in range(NBUCK):
            assert ng_b[b] <= NGBS[b]

        # atom id of each core-local group (bucket-sectioned, then padded)
        atom_of_G = np.full(NG, -1, np.int32)   # pads -> -1 (trash rows)
        for b in range(NBUCK):
            rep = np.repeat(np.arange(NA, dtype=np.int32), g_c[:, b])
            atom_of_G[gb_off[b] : gb_off[b] + len(rep)] = rep

        # slot position for each real edge:
        atom_l = si - a0
        core_mask = (atom_l >= 0) & (atom_l < NA)
        e_sel = np.nonzero(core_mask)[0]
        al = atom_l[e_sel]
        eb = sjb[e_sel]
        flat_id = (si[e_sel] * NBUCK + eb)
        rank = (e_sel - edge_off[flat_id])
        grank = rank // GS
        lane = rank % GS
        G = gb_off[eb] + gofs[al, eb] + grank    # core-local group id
        # group cell mapping: scatter token index == group rank within the
        # chunk (atom-monotone), so token prefixes map to atom ranges.
        # token t -> (partition t%128, cell t//128); slots of a group are GS
        # consecutive columns of one partition.
        c_ch = G // GCH
        pp = G % 128
        tg = (G % GCH) // 128
        pos = c_ch * CHSLOTS + (tg * GS + lane) * P + pp

        # per-slot streams (defaults for pad slots)
        r_s = np.full(SLOTS, 1.0e4, np.float32)
        rcj_s = np.ones(SLOTS, np.float32)
        enj_s = np.ones(SLOTS, np.float32)
        rci_s = np.ones(SLOTS, np.float32)
        eni_s = np.ones(SLOTS, np.float32)
        si_s = np.ones(SLOTS, np.float32)
        sj_s = np.ones(SLOTS, np.float32)
        jl_s = np.zeros(SLOTS, np.int32)

        r_s[pos] = sl[e_sel]
        jl_s[pos] = sj[e_sel] - np.array(BBASE, np.int64)[eb]

        # pass-B scatter idx per chunk: token t == group rank in chunk
        sidx = np.zeros((NCH, 16, GCH // 16), np.int16)
        for ch in range(NCH):
            av = atom_of_G[ch * GCH : (ch + 1) * GCH].copy()
            av[av < 0] = 0   # pad groups sum to ~0; row 0 is harmless
            sidx[ch] = _wrap16(av)



        per_core.append(dict(
            pos=pos, e_sel=e_sel, sj=sj[e_sel], sp_i=spec_pad[si[e_sel]],
            sp_j=spec_pad[sj[e_sel]], atom_of_G=atom_of_G,
            r_s=r_s, rcj_s=rcj_s, enj_s=enj_s, rci_s=rci_s, eni_s=eni_s,
            si_s=si_s, sj_s=sj_s, jl_s=jl_s, sidx=sidx,
            spec_slice=spec_pad[a0 : a0 + NA], chg_slice=chg_pad[a0 : a0 + NA],
        ))
    # pass-A sub-scatter structure: split ncoord into lo [0,SECB) / hi
    # [SECB,NA) tables so stage 2 can start before pass A finishes. Token
    # ranges are uniform across cores (SPMD); out-of-section tokens in the
    # overlap zone hit a trash row.
    asub = []      # per slot-chunk: list of (section, tok0, ntok, rowid)
    nrow = 0
    for ch in range(NCH):
        t1s, t0s = [], []
        for pc in per_core:
            av = pc["atom_of_G"][ch * GCH : (ch + 1) * GCH]
            lo = (av >= 0) & (av < SECB)
            hi = av >= SECB
            if lo.any():
                t1s.append(int(np.nonzero(lo)[0][-1]) + 1)
            if hi.any():
                t0s.append(int(np.nonzero(hi)[0][0]))
        subs = []
        if t1s:
            t1 = min((max(t1s) + 127) // 128 * 128, GCH)
            subs.append((0, 0, t1, nrow)); nrow += 1
        if t0s:
            t0 = min(t0s) // 128 * 128
            subs.append((1, t0, GCH - t0, nrow)); nrow += 1
        asub.append(subs)
    meta["asub"] = asub
    meta["NASC"] = nrow
    for pc in per_core:
        aidx = np.zeros((nrow, 16, GCH // 16), np.int16)
        for ch in range(NCH):
            av = pc["atom_of_G"][ch * GCH : (ch + 1) * GCH]
            for (sec, t0, ntok, row) in asub[ch]:
                iv = av[t0:t0 + ntok].copy()
                if sec == 0:
                    bad = ~((iv >= 0) & (iv < SECB))
                    iv[bad] = SECB
                else:
                    sel = iv >= SECB
                    iv = np.where(sel, iv - SECB, NA - SECB)
                aidx[row, :, :ntok // 16] = _wrap16(iv)
        pc["aidx"] = aidx
    return per_core, meta


def build_core_inputs(pc, meta, rcov, en, sqrt_r4r2):
    """Fill species-derived streams + wrapped idx arrays for one core."""
    SLOTS, NCH = meta["SLOTS"], meta["NCH"]
    pos = pc["pos"]
    pc["rcj_s"][pos] = rcov[pc["sp_j"]]
    pc["enj_s"][pos] = en[pc["sp_j"]]
    pc["rci_s"][pos] = rcov[pc["sp_i"]]
    pc["eni_s"][pos] = en[pc["sp_i"]]
    pc["si_s"][pos] = sqrt_r4r2[pc["sp_i"]]
    pc["sj_s"][pos] = sqrt_r4r2[pc["sp_j"]]

    # jidx16: one gather call per chunk, wrapped
    jl = pc["jl_s"]
    jw = np.zeros((NCH, 16, CALL // 16), np.int16)
    for k in range(NCH):
        jw[k] = _wrap16(jl[k * CALL : (k + 1) * CALL])

    # species wrap per atom chunk (2048 atoms): idx position u*128+p ->
    # local atom (16k+u)*128+p  (atom id = col*128 + p)
    spw = np.zeros((NACH, 16, (ACH * P) // 16), np.int16)
    spec = pc["spec_slice"].reshape(ACOLS, P)
    for k in range(NACH):
        lin = spec[k * ACH : (k + 1) * ACH, :].reshape(-1)  # [u, p] -> u*128+p
        spw[k] = _wrap16(lin)

    pa_pack = np.stack([
        pc["r_s"].reshape(NCH, CHSLOTS), pc["rcj_s"].reshape(NCH, CHSLOTS),
        pc["rci_s"].reshape(NCH, CHSLOTS)], axis=1)
    pa2_pack = np.stack([
        pc["enj_s"].reshape(NCH, CHSLOTS),
        pc["eni_s"].reshape(NCH, CHSLOTS)], axis=1)
    pb_pack = np.stack([
        pc["r_s"].reshape(NCH, CHSLOTS), pc["si_s"].reshape(NCH, CHSLOTS),
        pc["sj_s"].reshape(NCH, CHSLOTS)], axis=1)
    return dict(
        pa_pack=pa_pack, pa2_pack=pa2_pack, pb_pack=pb_pack,
        jw=jw.reshape(-1), spw=spw.reshape(-1), sidx=pc["sidx"].reshape(-1),
        aidx=pc["aidx"].reshape(-1),
        chg=np.ascontiguousarray(
            pc["chg_slice"].reshape(ACOLS, P).T).astype(np.float32),
    )


def _bc(ap, n):
    """Broadcast AP: append a step-0 inner dim of size n."""
    return bass.AP(tensor=ap.tensor, offset=ap.offset, ap=[*ap.ap, [0, n]])


def _dma_gather_raw(nc, out_ap, in_ap, idxs_ap, num_idxs, elem_size, elem_step):
    """dma_gather without the elem_size%256 restriction (payload < row pitch).
    Mirrors bass.BassGpSimd.dma_gather (non-transpose, DRAM source)."""
    eng = nc.gpsimd
    assert idxs_ap.dtype == mybir.dt.int16
    assert in_ap.dtype == out_ap.dtype
    stride_bytes = elem_step * mybir.dt.size(in_ap.dtype)
    assert stride_bytes % 256 == 0
    stride_bytes_256 = stride_bytes // 256
    assert in_ap.ap[0][0] == elem_step
    assert in_ap.ap[-1][1] == elem_size
    assert out_ap.ap[-1][1] == elem_size
    _in_ap = eng.lower_ap_dma(in_ap, for_custom_bir_dma=True)
    _idxs_ap = eng.lower_ap(idxs_ap)
    _out_ap = eng.lower_ap(out_ap)
    return eng.add_instruction(
        mybir.InstDMAGatherAnt(
            name=nc.get_next_instruction_name(),
            ins=[*_in_ap, _idxs_ap, eng.lower_val_access(eng.to_reg(num_idxs))],
            outs=[_out_ap],
            transpose=False,
            num_idxs=num_idxs,
            elem_size=elem_size,
            stride_bytes_256=stride_bytes_256,
            gen_mode=0,
            single_packet=True,
            queue_num=0,
            sbuf_tokens_per_rank=0,
            sbuf_free_dim_per_rank=0,
            sbuf_free_dim_pad_per_rank=0,
            sbuf_byte_offset=0,
        )
    )


def build_program(meta):
    SLOTS, NCH = meta["SLOTS"], meta["NCH"]
    ch_bucket = meta["ch_bucket"]
    A = mybir.AluOpType
    AF = mybir.ActivationFunctionType

    nc = bacc.Bacc(None, num_devices=NCORES, dynamic_dma_scratch_size=40960)

    def din(name, shape, dt=F32):
        return nc.dram_tensor(name, shape, dt, kind="ExternalInput")

    # per-slot streams, packed stream-major per chunk (one DMA per chunk)
    pa_d = din("pa_pack", [NCH, 3, CHSLOTS])
    pa2_d = din("pa2_pack", [NCH, 2, CHSLOTS], BF16)
    pb_d = din("pb_pack", [NCH, 3, CHSLOTS])
    jw_d = din("jw", [NCH, 16, CALL // 16], I16)
    spw_d = din("spw", [NACH, 16, (ACH * P) // 16], I16)
    sidx_d = din("sidx", [NCH, 16, GCH // 16], I16)
    aidx_d = din("aidx", [meta["NASC"], 16, GCH // 16], I16)
    chg_d = din("chg", [P, ACOLS])
    # all per-species tables concatenated into one upload (one DMA issue):
    # 0:7 zeff_r | 7:14 sscale_r | 14:21 gam_r | 21:28 refh | 28:35 ascale |
    # 35:42 hcount | 42:49 refq | 49:210 secaiw_r | 210:371 alphaiw |
    # 371 gam | 372 zeff | 373:408 cnw | 408:443 cn | 443:478 mask
    tab87_d = din("tab87", [Z, 478])
    cpw_d = din("cpw", [NW])
    s6_d = din("s6_raw", [1]); s8_d = din("s8_raw", [1])
    a1_d = din("a1_raw", [1]); a2_d = din("a2_raw", [1]); sq_d = din("scale_q_raw", [1])

    srowA_d = nc.dram_tensor("srowad", [Z, 64], F32)
    srowB_d = nc.dram_tensor("srowbd", [Z, 256], BF16)
    ncoL_d = nc.dram_tensor("ncold", [SECB + 16, BTW], F32)
    ncoH_d = nc.dram_tensor("ncohd", [NA - SECB + 16, BTW], F32)
    btab_d = nc.dram_tensor("btabd", [NA, 2 * BTW], BF16)
    t2sb_d = nc.dram_tensor("t2sb", [NA, NW], BF16)
    t2f_d = nc.dram_tensor("t2f", [NPAD, P], BF16, addr_space="Shared")
    e_d = nc.dram_tensor("e_out", [NA], F32, kind="ExternalOutput")

    def brc(dram, parts, width):
        """AP reading a [width] DRAM tensor broadcast across `parts` partitions."""
        return bass.AP(tensor=dram.tensor if hasattr(dram, "tensor") else dram,
                       offset=0, ap=[[0, parts], [1, width]])

    with tile.TileContext(nc) as tc:
        import contextlib
        with contextlib.ExitStack() as ctx:
            const = ctx.enter_context(tc.tile_pool(name="const", bufs=1))
            _srowcm = tc.tile_pool(name="srowp", bufs=1)
            srowp = _srowcm.__enter__()
            _wcm = tc.tile_pool(name="p0", bufs=2)
            work = _wcm.__enter__()

            nc.gpsimd.load_library(mlp_library)

            # dedicated index tiles: ucode reads only rows 0:16, so uploads
            # write just those rows; memset once here to satisfy init checks
            jwt_a = const.tile([P, CALL // 16], I16, tag="jwt0")
            jwt_b = const.tile([P, CALL // 16], I16, tag="jwt1")
            sxt_a = const.tile([P, GCH // 16], I16, tag="sxt0")
            sxt_b = const.tile([P, GCH // 16], I16, tag="sxt1")
            spwt = const.tile([P, (ACH * P) // 16], I16, tag="spwt")
            axt_a = const.tile([P, GCH // 16], I16, tag="axt0")
            axt_b = const.tile([P, GCH // 16], I16, tag="axt1")
            axt_c = const.tile([P, GCH // 16], I16, tag="axt2")
            axt_d = const.tile([P, GCH // 16], I16, tag="axt3")
            jwt2 = [jwt_a, jwt_b]
            sxt2 = [sxt_a, sxt_b]
            axt4 = [axt_a, axt_b, axt_c, axt_d]
            for t_ in (*jwt2, *sxt2, *axt4, spwt):
                nc.gpsimd.memset(t_[:], 0)

            b3_87 = const.tile([Z, 1], F32)
            nc.vector.memset(b3_87[:], 3.0)
            b3_p = const.tile([P, 1], F32)
            nc.vector.memset(b3_p[:], 3.0)
            bk5_p = const.tile([P, 1], F32)
            nc.vector.memset(bk5_p[:], K5)
            bkk_p = const.tile([P, 1], F32)
            nc.vector.memset(bkk_p[:], KK)

            # zero rows of nco/btab tables (only the columns we touch)
            zcol = const.tile([P, SECB // P, 1], F32)
            nc.vector.memset(zcol[:], 0.0)
            nc.sync.dma_start(
                out=ncoL_d[0:SECB].rearrange("(a p) f -> p a f", p=P)[:, :, 0:1],
                in_=zcol[:, 0:SECB // P, :])
            nc.sync.dma_start(
                out=ncoH_d[0:NA - SECB].rearrange("(a p) f -> p a f", p=P)[:, :, 0:1],
                in_=zcol[:, 0:(NA - SECB) // P, :])
            zrow23 = const.tile([P, ACH, NW], BF16)
            nc.vector.memset(zrow23[:], 0.0)

            # ---------- P0: per-species row table ----------
            tab = const.tile([Z, 478], F32)
            nc.sync.dma_start(out=tab[:], in_=tab87_d[:])
            t_ = tab[:]
            zeffr, sscr, gamr = t_[:, 0:7], t_[:, 7:14], t_[:, 14:21]
            refh, asc, hcnt = t_[:, 21:28], t_[:, 28:35], t_[:, 35:42]
            refq = t_[:, 42:49]
            secr, aiw = t_[:, 49:210], t_[:, 210:371]
            gam1, zeff1 = t_[:, 371:372], t_[:, 372:373]
            cnw, cnt_, msk = t_[:, 373:408], t_[:, 408:443], t_[:, 443:478]

            # softplus of all 5 scalar params in one Exp->Ln block
            params = const.tile([P, 5], F32)
            for ii, dd in enumerate([s6_d, s8_d, a1_d, a2_d, sq_d]):
                nc.sync.dma_start(out=params[:, ii:ii+1], in_=brc(dd, P, 1))
            nc.scalar.activation(out=params[:], in_=params[:], func=AF.Exp)
            nc.vector.tensor_scalar(out=params[:], in0=params[:], scalar1=1.0,
                                    scalar2=None, op0=A.add)
            nc.scalar.activation(out=params[:], in_=params[:], func=AF.Ln)
            s6p, s8p = params[:, 0:1], params[:, 1:2]
            a1p, a2p = params[:, 2:3], params[:, 3:4]
            spq = params[:, 4:5]
            sq87 = params[0:Z, 4:5]

            qmod = work.tile([Z, NREF], F32, tag="p0a")
            nc.vector.tensor_scalar(out=qmod[:], in0=refh, scalar1=sq87,
                                    scalar2=None, op0=A.mult)
            nc.vector.tensor_tensor(out=qmod[:], in0=qmod[:], in1=zeffr, op=A.add)
            qmsk = work.tile([Z, NREF], F32, tag="p0b")
            nc.vector.tensor_scalar(out=qmsk, in0=qmod[:], scalar1=1e-8,
                                    scalar2=None, op0=A.is_gt)
            qsafe = work.tile([Z, NREF], F32, tag="p0c")
            nc.vector.tensor_scalar(out=qsafe[:], in0=qmod[:], scalar1=1.0,
                                    scalar2=None, op0=A.subtract)
            nc.vector.tensor_tensor(out=qsafe[:], in0=qsafe[:], in1=qmsk,
                                    op=A.mult)
            nc.vector.tensor_scalar(out=qsafe[:], in0=qsafe[:], scalar1=1.0,
                                    scalar2=None, op0=A.add)
            rq = work.tile([Z, NREF], F32, tag="p0d")
            nc.vector.reciprocal(out=rq[:], in_=qsafe[:])
            t0 = work.tile([Z, NREF], F32, tag="p0e")
            nc.vector.tensor_tensor(out=t0[:], in0=zeffr, in1=rq[:], op=A.mult)
            nc.vector.tensor_tensor(out=t0[:], in0=t0[:], in1=gamr, op=A.mult)
            nc.vector.tensor_tensor(out=t0[:], in0=gamr, in1=t0[:], op=A.subtract)
            nc.scalar.activation(out=t0[:], in_=t0[:], func=AF.Exp, scale=2.0)
            nc.scalar.activation(out=t0[:], in_=t0[:], func=AF.Exp, scale=-3.0,
                                 bias=b3_87[:, 0:1])
            zfac = work.tile([Z, NREF], F32, tag="p0f")
            nc.vector.tensor_scalar(out=zfac[:], in0=t0[:], scalar1=E3,
                                    scalar2=None, op0=A.subtract)
            nc.vector.tensor_tensor(out=zfac[:], in0=zfac[:], in1=qmsk,
                                    op=A.mult)
            nc.vector.tensor_scalar(out=zfac[:], in0=zfac[:], scalar1=E3,
                                    scalar2=None, op0=A.add)
            al = work.tile([Z, NREF, NW], F32, tag="p0g")
            nc.vector.tensor_tensor(
                out=al[:], in0=secr.rearrange("z (a w) -> z a w", w=NW),
                in1=_bc(sscr, NW), op=A.mult)
            nc.vector.tensor_tensor(out=al[:], in0=al[:], in1=_bc(zfac[:], NW),
                                    op=A.mult)
            nc.vector.tensor_tensor(out=al[:], in0=al[:], in1=_bc(hcnt, NW),
                                    op=A.mult)
            nc.vector.tensor_tensor(
                out=al[:], in0=aiw.rearrange("z (a w) -> z a w", w=NW),
                in1=al[:], op=A.subtract)
            nc.vector.tensor_tensor(out=al[:], in0=al[:], in1=_bc(asc, NW),
                                    op=A.mult)
            nc.vector.tensor_scalar(out=al[:], in0=al[:], scalar1=0.0,
                                    scalar2=None, op0=A.max)
            cpw87 = const.tile([Z, NW], F32)
            nc.sync.dma_start(out=cpw87[:], in_=brc(cpw_d, Z, NW))
            nc.scalar.activation(out=cpw87[:], in_=cpw87[:], func=AF.Sqrt,
                                 scale=CPFAC)
            wb = bass.AP(tensor=cpw87[:].tensor, offset=cpw87[:].offset,
                         ap=[cpw87[:].ap[0], [0, NREF], [1, NW]])
            nc.vector.tensor_tensor(out=al[:], in0=al[:], in1=wb, op=A.mult)

            # assemble split species rows: f32 part (gam, zeff, refq, cn)
            # + bf16 part (atil, cnw, mask)
            srow = const.tile([Z, 64], F32)
            nc.vector.memset(srow[:], 0.0)
            nc.vector.tensor_copy(out=srow[:, 0:1], in_=gam1)
            nc.vector.tensor_copy(out=srow[:, 1:2], in_=zeff1)
            nc.vector.tensor_copy(out=srow[:, 2:9], in_=refq)
            nc.vector.tensor_copy(out=srow[:, 9:44], in_=cnt_)
            nc.sync.dma_start(out=srowA_d[:], in_=srow[:])
            srowb = const.tile([Z, 256], BF16)
            nc.gpsimd.memset(srowb[:], 0)
            nc.scalar.activation(out=srowb[:, 0:161],
                                 in_=al[:].rearrange("z a w -> z (a w)"),
                                 func=AF.Copy)
            nc.scalar.activation(out=srowb[:, 161:196], in_=cnw, func=AF.Copy)
            nc.scalar.activation(out=srowb[:, 196:231], in_=msk, func=AF.Copy)
            nc.sync.dma_start(out=srowB_d[:], in_=srowb[:])


            _wcm.__exit__(None, None, None)
            _wcm = tc.tile_pool(name="pA", bufs=3)
            work = _wcm.__enter__()

            # ---------- P1: pass A (coordination numbers) ----------
            def ldsidx(c, pool, tag):
                t = sxt2[c % 2]
                nc.sync.dma_start(out=t[0:16, :], in_=sidx_d[c])
                return t

            def scatter_add(out_ap, in_ap, idxs_t, num, elem, step):
                return nc.gpsimd.dma_scatter_add(
                    out_ap, in_ap, idxs_t, num, num, elem, elem_step=step)

            srow_t = []
            asub = meta["asub"]

            def emit_passA(c, pos_):
                st = work.tile([P, 3, TCH], F32, tag="a_st")
                nc.sync.dma_start(
                    out=st[:], in_=pa_d[c].rearrange("v (t p) -> p v t", p=P))
                st2 = work.tile([P, 2, TCH], BF16, tag="a_st2")
                nc.sync.dma_start(
                    out=st2[:], in_=pa2_d[c].rearrange("v (t p) -> p v t", p=P))
                r_t, rcj, rci = st[:, 0, :], st[:, 1, :], st[:, 2, :]
                enj, eni = st2[:, 0, :], st2[:, 1, :]
                # interleave stage-2 srow prefetch gathers (needed only by
                # stage 2) so their DMA doesn't starve pass-A stream loads
                if (pos_ % 2 == 1 or pos_ == NCH - 1) and len(srow_t) < NACH:
                    k = len(srow_t)
                    spw_t = spwt
                    nc.sync.dma_start(out=spw_t[0:16, :], in_=spw_d[k])
                    sg = srowp.tile([P, ACH, 44], F32, tag=f"pf_srow{k}")
                    _dma_gather_raw(nc, sg[:], srowA_d[:, 0:44], spw_t[:],
                                    ACH * P, 44, 64)
                    sgb = srowp.tile([P, ACH, 231], BF16, tag=f"pf_srowb{k}")
                    _dma_gather_raw(nc, sgb[:], srowB_d[:, 0:231], spw_t[:],
                                    ACH * P, 231, 256)
                    srow_t.append((sg, sgb))
                # rcv = 4/3*(rci+rcj)
                rcv = work.tile([P, TCH], F32, tag="a_rcv")
                nc.vector.tensor_tensor(out=rcv[:], in0=rci, in1=rcj, op=A.add)
                nc.vector.tensor_scalar(out=rcv[:], in0=rcv[:], scalar1=4.0 / 3.0,
                                        scalar2=None, op0=A.mult)
                # den = K4*exp(-((|eni-enj|+K5)^2)/K6) via sigmoid identity:
                # exp(-v) = 1/sigmoid(v) - 1
                den = work.tile([P, TCH], F32, tag="a_den")
                nc.vector.tensor_tensor(out=den[:], in0=eni, in1=enj,
                                        op=A.subtract)
                nc.scalar.activation(out=den[:], in_=den[:], func=AF.Abs)
                nc.scalar.activation(out=den[:], in_=den[:], func=AF.Square,
                                     bias=bk5_p[:, 0:1])
                nc.scalar.activation(out=den[:], in_=den[:], func=AF.Sigmoid,
                                     scale=1.0 / K6)
                nc.vector.reciprocal(out=den[:], in_=den[:])
                nc.vector.tensor_scalar(out=den[:], in0=den[:], scalar1=1.0,
                                        scalar2=0.5 * K4, op0=A.subtract,
                                        op1=A.mult)
                # erf(-KK*(rr-rcv)/rcv) = Erf(-KK*u + KK), u = rr/rcv
                cf = work.tile([P, TCH], F32, tag="a_cf")
                nc.vector.reciprocal(out=cf[:], in_=rcv[:])
                nc.vector.tensor_tensor(out=cf[:], in0=cf[:], in1=r_t, op=A.mult)
                nc.scalar.activation(out=cf[:], in_=cf[:], func=AF.Erf,
                                     scale=-KK / BOHR, bias=bkk_p[:, 0:1])
                # countf = (erf + 1) * den_scaled
                nc.vector.scalar_tensor_tensor(out=cf[:], in0=cf[:], scalar=1.0,
                                               in1=den[:], op0=A.add, op1=A.mult)
                # tree reduce GS -> 1
                l1 = work.tile([P, TCH // 2], F32, tag="a_l1")
                v = cf[:].rearrange("p (a two) -> p a two", two=2)
                nc.vector.tensor_tensor(out=l1[:], in0=v[:, :, 0], in1=v[:, :, 1],
                                        op=A.add)
                l2 = work.tile([P, TPG, 1], F32, tag="a_l2")
                v = l1[:].rearrange("p (a two) -> p a two", two=2)
                nc.vector.tensor_tensor(out=l2[:, :, 0],
                                        in0=v[:, :, 0], in1=v[:, :, 1],
                                        op=A.add)
                # scatter-add group sums into lo/hi per-atom ncoord tables
                for (sec, t0, ntok, row) in asub[c]:
                    it = axt4[(pos_ * 2 + sec) % 4]
                    nc.sync.dma_start(out=it[0:16, 0:ntok // 16],
                                      in_=aidx_d[row][:, 0:ntok // 16])
                    tab = ncoL_d if sec == 0 else ncoH_d
                    scatter_add(tab[:, 0:1],
                                l2[:, t0 // 128:(t0 + ntok) // 128, :],
                                it[:, 0:ntok // 16], ntok, 1, BTW)

            _s2cm = tc.tile_pool(name="pS2", bufs=2)
            works2 = _s2cm.__enter__()

            # ---------- P2: stage 2 (per-atom A~ rows) ----------
            def emit_s2(k):
                work = works2
                sr = srow_t[k][0][:]
                srb = srow_t[k][1][:]
                nco = work.tile([P, ACH, 1], F32, tag="s2_nco")
                if k * ACH < SECB // P:
                    nsrc = ncoL_d[0:SECB].rearrange("(a p) f -> p a f", p=P)[
                        :, k * ACH:(k + 1) * ACH, 0:1]
                else:
                    k2 = k - SECB // P // ACH
                    nsrc = ncoH_d[0:NA - SECB].rearrange(
                        "(a p) f -> p a f", p=P)[
                        :, k2 * ACH:(k2 + 1) * ACH, 0:1]
                nc.sync.dma_start(out=nco[:], in_=nsrc)

                # gaussian weights gw[P, ACH, NREF]
                gw35 = work.tile([P, ACH, NREF * NC], F32, tag="s2_gw35")
                nc.vector.tensor_tensor(out=gw35[:],
                                        in0=_bc(nco[:, :, 0], NREF * NC),
                                        in1=sr[:, :, 9:44], op=A.subtract)
                nc.vector.tensor_tensor(out=gw35[:], in0=gw35[:], in1=gw35[:],
                                        op=A.mult)
                nc.vector.tensor_tensor(out=gw35[:], in0=gw35[:],
                                        in1=srb[:, :, 161:196], op=A.mult)
                nc.scalar.activation(out=gw35[:], in_=gw35[:], func=AF.Exp,
                                     scale=-6.0)
                nc.vector.tensor_tensor(out=gw35[:], in0=gw35[:],
                                        in1=srb[:, :, 196:231], op=A.mult)
                gw = work.tile([P, ACH, NREF], F32, tag="s2_gw")
                g5 = gw35[:].rearrange("p c (a n) -> p c a n", n=NC)
                nc.vector.tensor_tensor(out=gw[:], in0=g5[:, :, :, 0],
                                        in1=g5[:, :, :, 1], op=A.add)
                for n5 in range(2, NC):
                    nc.vector.tensor_tensor(out=gw[:], in0=gw[:],
                                            in1=g5[:, :, :, n5], op=A.add)
                nrm = work.tile([P, ACH], F32, tag="s2_nrm")
                nc.vector.tensor_reduce(out=nrm[:], in_=gw[:],
                                        axis=mybir.AxisListType.X, op=A.add)
                nc.vector.tensor_scalar(out=nrm[:], in0=nrm[:], scalar1=1e-7,
                                        scalar2=None, op0=A.max)
                nc.vector.reciprocal(out=nrm[:], in_=nrm[:])
                nc.vector.tensor_tensor(out=gw[:], in0=gw[:], in1=_bc(nrm[:], NREF),
                                        op=A.mult)
                # zeta
                chg_t = work.tile([P, ACH], F32, tag="s2_chg")
                nc.sync.dma_start(out=chg_t[:], in_=chg_d[:, k * ACH:(k + 1) * ACH])
                qmod2 = work.tile([P, ACH], F32, tag="s2_qm")
                nc.vector.tensor_tensor(out=qmod2[:], in0=chg_t[:],
                                        in1=sr[:, :, 1], op=A.add)
                msk2 = work.tile([P, ACH], F32, tag="s2_msk")
                nc.vector.tensor_scalar(out=msk2[:], in0=qmod2[:], scalar1=1e-8,
                                        scalar2=None, op0=A.is_gt)
                qs2 = work.tile([P, ACH], F32, tag="s2_qs")
                nc.vector.tensor_scalar(out=qs2[:], in0=qmod2[:], scalar1=1.0,
                                        scalar2=None, op0=A.subtract)
                nc.vector.tensor_tensor(out=qs2[:], in0=qs2[:], in1=msk2[:],
                                        op=A.mult)
                nc.vector.tensor_scalar(out=qs2[:], in0=qs2[:], scalar1=1.0,
                                        scalar2=None, op0=A.add)
                nc.vector.reciprocal(out=qs2[:], in_=qs2[:])
                zt = work.tile([P, ACH, NREF], F32, tag="s2_zt")
                nc.vector.tensor_scalar(out=zt[:], in0=sr[:, :, 2:9],
                                        scalar1=spq, scalar2=None,
                                        op0=A.mult)
                nc.vector.tensor_tensor(out=zt[:], in0=zt[:],
                                        in1=_bc(sr[:, :, 1], NREF), op=A.add)
                nc.vector.tensor_tensor(out=zt[:], in0=zt[:],
                                        in1=_bc(qs2[:], NREF), op=A.mult)
                nc.vector.tensor_tensor(out=zt[:], in0=zt[:],
                                        in1=_bc(sr[:, :, 0], NREF), op=A.mult)
                nc.vector.tensor_tensor(out=zt[:], in0=_bc(sr[:, :, 0], NREF),
                                        in1=zt[:], op=A.subtract)
                nc.scalar.activation(out=zt[:], in_=zt[:], func=AF.Exp, scale=2.0)
                nc.scalar.activation(out=zt[:], in_=zt[:], func=AF.Exp,
                                     scale=-3.0, bias=b3_p[:, 0:1])
                zeta = work.tile([P, ACH, NREF], F32, tag="s2_zeta")
                mb = bass.AP(tensor=msk2[:].tensor, offset=msk2[:].offset,
                             ap=[*msk2[:].ap, [0, NREF]])
                nc.vector.tensor_scalar(out=zeta[:], in0=zt[:], scalar1=E3,
                                        scalar2=None, op0=A.subtract)
                nc.vector.tensor_tensor(out=zeta[:], in0=zeta[:], in1=mb,
                                        op=A.mult)
                nc.vector.tensor_scalar(out=zeta[:], in0=zeta[:], scalar1=E3,
                                        scalar2=None, op0=A.add)
                nc.vector.tensor_tensor(out=zeta[:], in0=zeta[:], in1=gw[:],
                                        op=A.mult)
                # A~_i[w] = sum_a zeta[a]*atil[a,w]; replicate zeta over w
                # and cast atil to bf16 on the Activation engine so the DVE
                # mult/add chain runs in the packed-bf16 2x mode
                zrep = work.tile([P, ACH, NREF, NW], BF16, tag="s2_zrep")
                nc.scalar.activation(out=zrep[:], in_=_bc(zeta[:], NW),
                                     func=AF.Copy)
                av = srb[:, :, 0:161].rearrange("p c (a w) -> p c a w", w=NW)
                t2row = work.tile([P, ACH, NW], BF16, tag="s2_t2row")
                for a_ in range(NREF):
                    if a_ == 0:
                        nc.vector.tensor_tensor(
                            out=t2row[:], in0=av[:, :, 0, :],
                            in1=zrep[:, :, 0, :], op=A.mult)
                    else:
                        tmp_ = work.tile([P, ACH, NW], BF16, tag="s2_tmp")
                        nc.vector.tensor_tensor(
                            out=tmp_[:], in0=av[:, :, a_, :],
                            in1=zrep[:, :, a_, :], op=A.mult)
                        nc.vector.tensor_tensor(out=t2row[:], in0=t2row[:],
                                                in1=tmp_[:], op=A.add)
                nc.sync.dma_start(
                    out=t2sb_d.rearrange("(a p) w -> p a w", p=P)[
                        :, k * ACH:(k + 1) * ACH, :],
                    in_=t2row[:])

            # ---------- driver: pass A round-robin with interleaved s2 ----
            cpb = [meta["NGBS"][b] * GS // CHSLOTS for b in range(NBUCK)]
            coff = [0]
            for b in range(NBUCK):
                coff.append(coff[-1] + cpb[b])
            order = []
            for cc in range(max(cpb)):
                for b in range(NBUCK):
                    if cc < cpb[b]:
                        order.append(coff[b] + cc)
            lo_done_pos = max(
                pos_ for pos_, c in enumerate(order)
                if any(s[0] == 0 for s in meta["asub"][c]))
            for pos_, c in enumerate(order):
                emit_passA(c, pos_)
                if pos_ == lo_done_pos:
                    for k_ in range(SECB // P // ACH):
                        emit_s2(k_)
            for k_ in range(SECB // P // ACH, NACH):
                emit_s2(k_)

            _s2cm.__exit__(None, None, None)
            _wcm.__exit__(None, None, None)
            _srowcm.__exit__(None, None, None)

            # zero the B table now — its only consumers are the pass-B
            # scatter-adds, so this write rides the idle DMA window here
            for z5 in range(ACOLS // ACH):
                nc.sync.dma_start(
                    out=btab_d[0:NA].rearrange("(a p) f -> p a f", p=P)[
                        :, z5 * ACH:(z5 + 1) * ACH, 0:NW],
                    in_=zrow23[:])

            # ---------- P3: AllGather packed bf16 A~ rows into strided table --
            # (emitted before pass B1 in program order so the Pool engine
            # starts the collective while the DVE computes damping factors)
            nc.gpsimd.collective_compute(
                "AllGather", A.bypass,
                replica_groups=[list(range(NCORES))],
                ins=[t2sb_d[:]], outs=[t2f_d[:, 0:NW]])

            _wcm = tc.tile_pool(name="pB", bufs=3)
            work = _wcm.__enter__()

            # ---------- P4a: pass B1 — damping factors (overlaps AllGather) --
            dbts = []
            for c in range(NCH):
                sb = work.tile([P, 3, TCH], F32, tag="b_sb")
                nc.sync.dma_start(
                    out=sb[:], in_=pb_d[c].rearrange("v (t p) -> p v t", p=P))
                r_t, si_t, sj_t = sb[:, 0, :], sb[:, 1, :], sb[:, 2, :]
                r2 = work.tile([P, TCH], F32, tag="b_r2")
                nc.scalar.activation(out=r2[:], in_=r_t, func=AF.Square,
                                     scale=1.0 / BOHR)
                r4 = work.tile([P, TCH], F32, tag="b_r4")
                nc.scalar.activation(out=r4[:], in_=r2[:], func=AF.Square)
                r8 = work.tile([P, TCH], F32, tag="b_r8")
                nc.scalar.activation(out=r8[:], in_=r4[:], func=AF.Square)
                r6 = work.tile([P, TCH], F32, tag="b_r6")
                nc.vector.tensor_tensor(out=r6[:], in0=r4[:], in1=r2[:], op=A.mult)
                R3 = work.tile([P, TCH], F32, tag="b_R3")
                nc.vector.scalar_tensor_tensor(out=R3[:], in0=si_t, scalar=3.0,
                                               in1=sj_t, op0=A.mult,
                                               op1=A.mult)
                r0 = work.tile([P, TCH], F32, tag="b_r0")
                nc.scalar.activation(out=r0[:], in_=R3[:], func=AF.Sqrt)
                nc.vector.tensor_scalar(out=r0[:], in0=r0[:], scalar1=a1p,
                                        scalar2=a2p, op0=A.mult, op1=A.add)
                q2 = work.tile([P, TCH], F32, tag="b_q2")
                nc.scalar.activation(out=q2[:], in_=r0[:], func=AF.Square)
                c4 = work.tile([P, TCH], F32, tag="b_c4")
                nc.scalar.activation(out=c4[:], in_=q2[:], func=AF.Square)
                c3 = work.tile([P, TCH], F32, tag="b_c3")
                nc.vector.tensor_tensor(out=c3[:], in0=c4[:], in1=q2[:], op=A.mult)
                c8 = work.tile([P, TCH], F32, tag="b_c8")
                nc.scalar.activation(out=c8[:], in_=c4[:], func=AF.Square)
                d6 = work.tile([P, TCH], F32, tag="b_d6")
                nc.vector.tensor_tensor(out=d6[:], in0=r6[:], in1=c3[:], op=A.add)
                nc.vector.reciprocal(out=d6[:], in_=d6[:])
                d8 = work.tile([P, TCH], F32, tag="b_d8")
                nc.vector.tensor_tensor(out=d8[:], in0=r8[:], in1=c8[:], op=A.add)
                nc.vector.reciprocal(out=d8[:], in_=d8[:])
                nc.vector.tensor_tensor(out=d8[:], in0=d8[:], in1=R3[:], op=A.mult)
                nc.vector.tensor_scalar(out=d8[:], in0=d8[:], scalar1=s8p,
                                        scalar2=None, op0=A.mult)
                d6c = const.tile([P, TCH], F32, tag=f"b_d6_{c}")
                nc.vector.scalar_tensor_tensor(out=d6c[:], in0=d6[:], scalar=s6p,
                                               in1=d8[:], op0=A.mult, op1=A.add)
                dbts.append(d6c)

            # ---------- P4b: pass B2 (gather + scale + reduce + scatter) -----
            cpb = [meta["NGBS"][b] * GS // CHSLOTS for b in range(NBUCK)]
            coff = [0]
            for b in range(NBUCK):
                coff.append(coff[-1] + cpb[b])
            order = []
            for cc in range(max(cpb)):
                for b in range(NBUCK):
                    if cc < cpb[b]:
                        order.append(coff[b] + cc)
            lo_done_pos = max(
                pos_ for pos_, c in enumerate(order)
                if any(s[0] == 0 for s in meta["asub"][c]))

            def emit_passB2(c, pos_):
                b = ch_bucket[c]
                sidx_t = ldsidx(c, work, "b_sidx")
                jw_t = jwt2[pos_ % 2]
                nc.sync.dma_start(out=jw_t[0:16, :], in_=jw_d[c])
                gt = work.tile([P, TCH, NW], BF16, tag="b_g")
                _dma_gather_raw(
                    nc, gt[:],
                    t2f_d[BBASE[b]:BBASE[b] + BSIZE[b], 0:NW],
                    jw_t[:], CALL, NW, P)
                d6c = dbts[c]
                # replicate D over the 23 w-columns on the Activation engine
                # (bf16 cast + broadcast), keeping the DVE mult in 2x mode
                db = bass.AP(tensor=d6c[:].tensor, offset=d6c[:].offset,
                             ap=[*d6c[:].ap, [0, NW]])
                drep = work.tile([P, TCH, NW], BF16, tag="b_drep")
                nc.scalar.activation(out=drep[:], in_=db, func=AF.Copy)
                nc.vector.tensor_tensor(out=gt[:], in0=gt[:], in1=drep[:],
                                        op=A.mult)
                # tree reduce over GS slots
                m1 = work.tile([P, TCH // 2, NW], BF16, tag="b_m1")
                v = gt[:].rearrange("p (a two) f -> p a two f", two=2)
                nc.vector.tensor_tensor(out=m1[:], in0=v[:, :, 0, :],
                                        in1=v[:, :, 1, :], op=A.add)
                m2 = work.tile([P, TPG, NW], BF16, tag="b_m2")
                v = m1[:].rearrange("p (a two) f -> p a two f", two=2)
                nc.vector.tensor_tensor(out=m2[:], in0=v[:, :, 0, :],
                                        in1=v[:, :, 1, :], op=A.add)
                # scatter-add group rows into the per-atom B table
                scatter_add(btab_d[:, 0:NW], m2[:], sidx_t[:], GCH, NW, 2 * BTW)

            _ecm = tc.tile_pool(name="pE", bufs=2)
            worke = _ecm.__enter__()

            # ---------- P5: assemble E ----------
            def emit_p5(k):
                work = worke
                bsum = work.tile([P, ACH, NW], BF16, tag="e_bsum")
                nc.sync.dma_start(
                    out=bsum[:],
                    in_=btab_d[0:NA].rearrange("(a p) f -> p a f", p=P)[
                        :, k * ACH:(k + 1) * ACH, 0:NW])
                ai = work.tile([P, ACH, NW], BF16, tag="e_ai")
                nc.sync.dma_start(
                    out=ai[:],
                    in_=t2sb_d.rearrange("(a p) w -> p a w", p=P)[
                        :, k * ACH:(k + 1) * ACH, :])
                prod = work.tile([P, ACH, NW], F32, tag="e_prod")
                nc.vector.tensor_tensor(out=prod[:], in0=ai[:],
                                        in1=bsum[:], op=A.mult)
                ev = work.tile([P, ACH], F32, tag="e_ev")
                nc.vector.tensor_reduce(out=ev[:], in_=prod[:],
                                        axis=mybir.AxisListType.X, op=A.add)
                nc.vector.tensor_scalar(out=ev[:], in0=ev[:],
                                        scalar1=-0.5 * HARTREE, scalar2=None,
                                        op0=A.mult)
                nc.sync.dma_start(
                    out=e_d.rearrange("(a p) -> p a", p=P)[:, k * ACH:(k + 1) * ACH],
                    in_=ev[:])

            for c in range(NCH):
                emit_passB2(c, c)
            for k_ in range(NACH):
                emit_p5(k_)

            _ecm.__exit__(None, None, None)
            _wcm.__exit__(None, None, None)
    return nc


_PROG_CACHE = {}


def kernel(**inputs):
    species = np.asarray(inputs["species"])
    per_core, meta = preprocess(species, inputs["edge_index"],
                                inputs["lengths"], inputs["partial_charges"])
    rcov = np.asarray(inputs["rcov"], np.float32)
    en = np.asarray(inputs["en"], np.float32)
    sr4 = np.asarray(inputs["sqrt_r4r2"], np.float32)
    refsys = np.asarray(inputs["refsys"]).astype(np.int64)

    # refsys-expanded tables (pure host-side permutation of inputs)
    zeff = np.asarray(inputs["zeff"], np.float32)
    sscale = np.asarray(inputs["sscale"], np.float32)
    gam = np.asarray(inputs["gam"], np.float32)
    secaiw = np.asarray(inputs["secaiw"], np.float32)
    zeff_r = zeff[refsys]
    sscale_r = sscale[refsys]
    gam_r = gam[refsys]
    secaiw_r = secaiw[refsys].reshape(Z, NREF * NW)

    import os as _os
    _bedrock = _os.environ.get("BEDROCK") == "1"
    if not _bedrock:
        key = (tuple(meta["NGBS"]),
               tuple(tuple(s) for ss in meta["asub"] for s in ss))
        if key not in _PROG_CACHE:
            nc = build_program(meta)
            nc.finalize()
            _PROG_CACHE[key] = nc
        nc = _PROG_CACHE[key]

    tab87 = np.concatenate([
        zeff_r, sscale_r, gam_r,
        np.asarray(inputs["refh"], np.float32),
        np.asarray(inputs["ascale"], np.float32),
        np.asarray(inputs["hcount"], np.float32),
        np.asarray(inputs["refq"], np.float32),
        secaiw_r,
        np.asarray(inputs["alphaiw"], np.float32).reshape(Z, NREF * NW),
        gam[:, None], zeff[:, None],
        np.asarray(inputs["ncount_weight"], np.float32).reshape(Z, -1),
        np.asarray(inputs["cn"], np.float32).reshape(Z, -1),
        np.asarray(inputs["ncount_mask"], np.float32).reshape(Z, -1),
    ], axis=1).astype(np.float32)
    shared = dict(
        tab87=tab87,
        cpw=np.asarray(inputs["cpw"], np.float32),
        s6_raw=np.asarray(inputs["s6_raw"], np.float32),
        s8_raw=np.asarray(inputs["s8_raw"], np.float32),
        a1_raw=np.asarray(inputs["a1_raw"], np.float32),
        a2_raw=np.asarray(inputs["a2_raw"], np.float32),
        scale_q_raw=np.asarray(inputs["scale_q_raw"], np.float32),
    )
    in_maps = []
    for c in range(NCORES):
        ci = build_core_inputs(per_core[c], meta, rcov, en, sr4)
        m = dict(shared)
        m.update(
            pa_pack=ci["pa_pack"], pa2_pack=ci["pa2_pack"],
            pb_pack=ci["pb_pack"],
            jw=ci["jw"].reshape(meta["NCH"], 16, CALL // 16),
            spw=ci["spw"].reshape(NACH, 16, (ACH * P) // 16),
            sidx=ci["sidx"].reshape(meta["NCH"], 16, GCH // 16),
            aidx=ci["aidx"].reshape(meta["NASC"], 16, GCH // 16),
            chg=ci["chg"],
        )
        in_maps.append(m)

    if _bedrock:
        outs = _sim_fallback(build_program(meta), in_maps)
    else:
        try:
            from concourse.bass_utils import run_bass_kernel_spmd
            res = run_bass_kernel_spmd(nc, in_maps, list(range(NCORES)))
            outs = [res.results[c]["e_out"] for c in range(NCORES)]
        except Exception:
            outs = _sim_fallback(build_program(meta), in_maps)
    e = np.concatenate(outs)
    return e[: species.shape[0]].astype(np.float32)


def _sim_fallback(nc, in_maps):
    import inspect
    import textwrap
    from scipy.special import erf as _scipy_erf
    from concourse import bass_interp
    src = textwrap.dedent(inspect.getsource(
        bass_interp.InstructionExecutor.visit_InstActivation))
    if "_scipy_erf" not in src:
        pat = ("    else:\n"
               "        # NOTE: If you are adding a new activation instruction")
        rep = ("    elif instruction.func == mb.ActivationFunctionType.Erf:\n"
               "        acted = _scipy_erf(scaled_and_biased)\n"
               "    else:\n"
               "        # NOTE: If you are adding a new activation instruction")
        assert pat in src
        src = src.replace(pat, rep)
        ns = dict(bass_interp.__dict__)
        ns["_scipy_erf"] = _scipy_erf
        exec(compile(src, "<erfpatch>", "exec"), ns)
        bass_interp.InstructionExecutor.visit_InstActivation = ns[
            "visit_InstActivation"]
    sim = bass_interp.MultiCoreSim(nc, NCORES, num_workers=1)
    for c in range(NCORES):
        for k, v in in_maps[c].items():
            sim.cores[c].tensor(k)[:] = v
    sim.simulate()
    global LAST_EXEC_TIME_NS
    LAST_EXEC_TIME_NS = int(getattr(sim, "global_time", 0))
    return [np.array(sim.cores[c].tensor("e_out")) for c in range(NCORES)]


LAST_EXEC_TIME_NS = None
